# revision 1
# baseline (speedup 1.0000x reference)
"""Trainium2 Bass kernel for deterministic NeuralSort soft-kNN (DKNN).

Math (per query b over N neighbors):
    s_j   = -||q_b - x_j||^2
    A_j   = sum_i |s_j - s_i|
    P[r,j]= softmax_j(scaling[r] * s_j - A_j),  r = 0..K-1, scaling[r] = N+1-2(r+1)
    out_j = sum_r P[r,j]

Key reduction: s_j = u_j - ||q_b||^2 with u_j = 2 q_b.x_j - ||x_j||^2.  The
||q||^2 term is constant in j, so it cancels in A (pairwise diffs) and shifts
every softmax row by a constant (scaling[r]*||q||^2) which softmax ignores.
So we never compute ||q||^2.

Sharding: data-parallel over the B=128 queries across 8 cores (16 each);
neighbors replicated.

Per-core hot loop (the O(B_local * N^2) part): for each query b, broadcast
u_b to 128 partitions (DMA), then for each 128-row block of pairwise rows,
one fused op produces |u_j - u_p| with the row-sum accumulated on the fly:
  - ScalarE:  activation(Abs, bias=-u_p, accum_out)      (3 blocks / query)
  - VectorE:  tensor_scalar(add -u_p, abs_max 0, accum)  (5 blocks / query)
By symmetry of |u_j - u_i| the free-dim row sums ARE A_sum for the block's
partition indices, so no cross-partition reduce is needed.
"""

import numpy as np

import concourse.bass as bass
import concourse.bacc as bacc
import concourse.tile as tile
from concourse import mybir
from concourse.masks import make_identity
from concourse.bass_utils import run_bass_kernel_spmd

AFT = mybir.ActivationFunctionType
ALU = mybir.AluOpType
FP32 = mybir.dt.float32
BF16 = mybir.dt.bfloat16

B, N, D, TOPK = 128, 1024, 128, 10
NCORES = 8
BL = B // NCORES          # 16 queries per core
NBLK = N // 128           # 8 row-blocks of the pairwise matrix
GROUPS = 2                # softmax groups (8 queries x 10 rows = 80 partitions)
GB = BL // GROUPS         # 8

# Static engine split of the queries (ratio ~ ACT vs DVE+PE per-query cost).
# ACT queries: fused Abs+accum on ScalarE (self-contained, A in transposed form).
# DVE queries: tensor_scalar |diff| tiles on VectorE, row-reduced by TensorE
# selector-matmuls straight into a row-form PSUM accumulator.
# ACT queries spread across both softmax groups so ScalarE stays busy through
# the whole pairwise phase; groups are {0..7} and {8..15}, group 0's queries
# are scheduled first so its softmax overlaps group 1's pairwise work.
ACT_SET = (0, 1, 2, 8, 9)
G0_DVE = (3, 4, 5, 6, 7)
G1_DVE = (10, 11, 12, 13, 14, 15)
GPS_SET = (10, 12)   # whole queries whose G-pass runs on GpSimd
TAIL_SPLIT = (13, 14, 15)  # final-pair queries: blocks t>=5 go to GpSimd too
PAIRS = ((0, 3), (1, 4), (2, 5), (6, 7), (8, 10), (9, 11), (12, 13), (14, 15))


def _host_consts():
    scaling = (N + 1 - 2.0 * (np.arange(TOPK) + 1)).astype(np.float32)
    E = np.zeros((BL, GROUPS, GB, TOPK), np.float32)
    F = np.zeros((BL, GROUPS, GB, TOPK), np.float32)
    for g in range(GROUPS):
        for bl in range(GB):
            E[g * GB + bl, g, bl, :] = scaling
            F[g * GB + bl, g, bl, :] = -1.0
    G = np.zeros((GB * TOPK, GB), np.float32)
    for bl in range(GB):
        G[bl * TOPK : (bl + 1) * TOPK, bl] = 1.0
    F = F.reshape(BL, -1)
    Fa = F.copy()
    Fd = F.copy()
    for b in range(BL):
        (Fd if b in ACT_SET else Fa)[b, :] = 0.0
    return E.reshape(BL, -1), Fa, Fd, G


def _build_nc(debug_taps=False):
    nc = bacc.Bacc(None, target_bir_lowering=False)

    q_in = nc.dram_tensor("query", [BL, D], FP32, kind="ExternalInput")
    x_in = nc.dram_tensor("neighbors", [N, D], FP32, kind="ExternalInput")
    out_t = nc.dram_tensor("out", [BL, N], FP32, kind="ExternalOutput")
    if debug_taps:
        dbg_u = nc.dram_tensor("dbg_u", [BL, N], FP32, kind="ExternalOutput")
        dbg_a = nc.dram_tensor("dbg_a", [BL, N], FP32, kind="ExternalOutput")
        dbg_nut = nc.dram_tensor("dbg_nut", [128, NBLK * BL], FP32, kind="ExternalOutput")
        dbg_paw = nc.dram_tensor("dbg_paw", [80, N], FP32, kind="ExternalOutput")

    E, Fa, Fd, G = _host_consts()
    e_in = nc.inline_tensor(E, "lhs_e")
    fa_in = nc.inline_tensor(Fa, "lhs_fa")
    fd_in = nc.inline_tensor(Fd, "lhs_fd")
    g_in = nc.inline_tensor(G, "lhs_g")

    with tile.TileContext(nc) as tc:
        with (
            tc.tile_pool(name="consts", bufs=1) as consts,
            tc.tile_pool(name="xp", bufs=1) as xp,
            tc.tile_pool(name="bcast", bufs=4) as bcast,
            tc.tile_pool(name="scrA", bufs=2) as scrA,
            tc.tile_pool(name="scrD", bufs=3) as scrD,
            tc.tile_pool(name="scrP", bufs=2) as scrP,
            tc.tile_pool(name="cmbp", bufs=2) as cmbp,
            tc.tile_pool(name="expp", bufs=2) as expp,
            tc.tile_pool(name="small", bufs=8) as small,
            tc.tile_pool(name="dramp", bufs=1, space="DRAM") as dramp,
        ):
            ident = consts.tile([128, 128], FP32)
            make_identity(nc, ident)
            ones128 = consts.tile([128, 1], FP32)
            nc.vector.memset(ones128, 1.0)
            ones1xb = consts.tile([1, BL], FP32)
            nc.vector.memset(ones1xb, 1.0)

            # ---- Phase A: neighbors in, transpose to [d, j]; row norms ----
            x_sb = xp.tile([128, NBLK, D], FP32)
            xv = x_in[:].rearrange("(t p) d -> p t d", p=128)
            half = NBLK // 2
            nc.default_dma_engine.dma_start(out=x_sb[:, :half, :], in_=xv[:, :half, :])
            q_sb = small.tile([BL, D], FP32)
            nc.default_dma_engine.dma_start(out=q_sb, in_=q_in[:])
            nc.default_dma_engine.dma_start(out=x_sb[:, half:, :], in_=xv[:, half:, :])

            xT = xp.tile([128, N], FP32)  # xT[d, j] = X[j, d]
            with tc.tile_pool(name="ps_tr", bufs=2, space="PSUM") as ps_tr:
                for t in range(NBLK):
                    ptr = ps_tr.tile([128, 128], FP32)
                    nc.tensor.transpose(ptr, x_sb[:, t, :], ident)
                    nc.any.tensor_copy(xT[:, t * 128 : (t + 1) * 128], ptr)

            sq = xp.tile([128, N], FP32)
            negx2 = consts.tile([1, N], FP32)  # -||x_j||^2
            with tc.tile_pool(name="ps_x2", bufs=1, space="PSUM") as ps_x2:
                px2 = ps_x2.tile([1, N], FP32)
                for c in range(2):
                    cs = slice(c * 512, (c + 1) * 512)
                    nc.scalar.activation(out=sq[:, cs], in_=xT[:, cs], func=AFT.Square)
                    nc.tensor.matmul(
                        px2[:, cs], lhsT=ones128, rhs=sq[:, cs], start=True, stop=True
                    )
                    nc.scalar.activation(
                        out=negx2[:, cs], in_=px2[:, cs], func=AFT.Copy, scale=-1.0
                    )

            e_sb = consts.tile([BL, GROUPS * GB * TOPK], FP32)
            nc.default_dma_engine.dma_start(out=e_sb, in_=e_in[:])
            fa_sb = consts.tile([BL, GROUPS * GB * TOPK], FP32)
            nc.default_dma_engine.dma_start(out=fa_sb, in_=fa_in[:])
            fd_sb = consts.tile([BL, GROUPS * GB * TOPK], FP32)
            nc.default_dma_engine.dma_start(out=fd_sb, in_=fd_in[:])
            g_sb = consts.tile([GB * TOPK, GB], FP32)
            nc.default_dma_engine.dma_start(out=g_sb, in_=g_in[:])

            # ---- Phase B: u = 2 Q X^T - ||x||^2, plus -u^T columns ----
            q2T = consts.tile([128, BL], FP32)   # (2Q)^T
            u_sb = consts.tile([BL, N], FP32)
            nuT = consts.tile([128, NBLK, BL], FP32)  # nuT[p, t, b] = -u[b, t*128+p]
            u_dram = dramp.tile([BL, N], FP32)
            with tc.tile_pool(name="ps_qt", bufs=2, space="PSUM") as ps_qt:
                pqt = ps_qt.tile([128, BL], FP32)
                nc.tensor.transpose(pqt, q_sb, ident[:BL, :BL])
                nc.scalar.activation(out=q2T, in_=pqt, func=AFT.Copy, scale=2.0)
                # nuT[:, t, b] = -u[b, t*128+p] computed directly:
                # uT_blk = xT_blk^T @ q2T + negx2_blk^T @ ones  (bit-identical
                # to the u_sb path: same products, same accumulation order).
                for t in range(NBLK):
                    put = ps_qt.tile([128, BL], FP32, tag="put")
                    nc.tensor.matmul(
                        put, lhsT=xT[:, t * 128 : (t + 1) * 128], rhs=q2T,
                        start=True, stop=False,
                    )
                    nc.tensor.matmul(
                        put, lhsT=negx2[:, t * 128 : (t + 1) * 128], rhs=ones1xb,
                        start=False, stop=True,
                    )
                    nc.vector.tensor_scalar(
                        out=nuT[:, t, :], in0=put, scalar1=-1.0, scalar2=None,
                        op0=ALU.mult,
                    )
                with tc.tile_pool(name="ps_u", bufs=1, space="PSUM") as ps_u:
                    pu = ps_u.tile([BL, N], FP32)
                    for c in range(2):
                        cs = slice(c * 512, (c + 1) * 512)
                        nc.tensor.matmul(
                            pu[:, cs], lhsT=q2T, rhs=xT[:, cs], start=True, stop=False
                        )
                        nc.tensor.matmul(
                            pu[:, cs], lhsT=ones1xb, rhs=negx2[:, cs],
                            start=False, stop=True,
                        )
                    nc.any.tensor_copy(u_sb, pu)
            nc.default_dma_engine.dma_start(out=u_dram[:4, :], in_=u_sb[:4, :])
            nc.scalar.dma_start(out=u_dram[4:, :], in_=u_sb[4:, :])
            Tb = small.tile([BL, 1], FP32, tag="Tb")
            nc.vector.tensor_reduce(
                out=Tb, in_=u_sb, axis=mybir.AxisListType.X, op=ALU.add
            )

            # Three-limb bf16 split of -nuT = u^T: u = hi + mid + lo exactly
            # (3 x 8 mantissa bits cover fp32's 24).  The 0/1 compare matrix G
            # is exact in bf16, so the TensorE reduce of (counts, L_hi, L_mid,
            # L_lo) runs at bf16 rate (1 col/cycle) instead of fp32's 4.
            hi_bf = consts.tile([128, NBLK, BL], BF16)
            nc.vector.tensor_scalar(out=hi_bf[:].rearrange("p t b -> p (t b)"),
                                    in0=nuT[:].rearrange("p t b -> p (t b)"),
                                    scalar1=-1.0, scalar2=None, op0=ALU.mult)
            hi32 = consts.tile([128, NBLK * BL], FP32)
            nc.vector.tensor_copy(hi32, hi_bf[:].rearrange("p t b -> p (t b)"))
            r1 = consts.tile([128, NBLK * BL], FP32)  # = hi - u
            nc.vector.tensor_tensor(out=r1, in0=nuT[:].rearrange("p t b -> p (t b)"),
                                    in1=hi32, op=ALU.add)
            mid_bf = consts.tile([128, NBLK, BL], BF16)
            nc.vector.tensor_scalar(out=mid_bf[:].rearrange("p t b -> p (t b)"),
                                    in0=r1, scalar1=-1.0, scalar2=None, op0=ALU.mult)
            mid32 = consts.tile([128, NBLK * BL], FP32)
            nc.vector.tensor_copy(mid32, mid_bf[:].rearrange("p t b -> p (t b)"))
            r2 = consts.tile([128, NBLK * BL], FP32)  # = hi + mid - u
            nc.vector.tensor_tensor(out=r2, in0=r1, in1=mid32, op=ALU.add)
            lo_bf = consts.tile([128, NBLK, BL], BF16)
            nc.vector.tensor_scalar(out=lo_bf[:].rearrange("p t b -> p (t b)"),
                                    in0=r2, scalar1=-1.0, scalar2=None, op0=ALU.mult)

            # W[p, t, b, m] (bf16): lhsT for the TensorE reduce of G.
            # col m==b: 1.0 -> counts r_j; m==32+b: hi -> L_hi row 32+b;
            # m==48+b: mid -> L_mid row 48+b; m==64+b: lo -> L_lo row 64+b.
            MW = 80
            W = consts.tile([128, NBLK, BL, MW], BF16)
            zb = consts.tile([128, 1], BF16)
            nc.vector.memset(zb, 0.0)
            zv = zb[:]
            zap = bass.AP(tensor=zv.tensor, offset=zv.offset,
                          ap=[zv.ap[0], [0, NBLK * BL * MW]])
            nc.gpsimd.affine_select(
                out=W[:].rearrange("p t b m -> p (t b m)"), in_=zap,
                compare_op=ALU.not_equal, fill=1.0, base=0,
                pattern=[[0, NBLK], [1, BL], [-1, MW]], channel_multiplier=0,
            )
            for t in range(NBLK):
                for lane, limb in ((32, hi_bf), (48, mid_bf), (64, lo_bf)):
                    lv = limb[:, t, :]
                    lb = bass.AP(tensor=lv.tensor, offset=lv.offset,
                                 ap=[lv.ap[0], list(lv.ap[1]), [0, BL]])
                    nc.gpsimd.tensor_tensor(
                        out=W[:, t, :, lane : lane + BL],
                        in0=W[:, t, :, 0:BL], in1=lb, op=ALU.mult,
                    )

            # ---- Phases C/D/E: pairwise passes, per-group overlap ----
            asumT = consts.tile([128, BL, NBLK], FP32)  # A_sum[b][t*128+p], ACT qs
            Ag0 = consts.tile([BL, N], FP32)
            Ag1 = consts.tile([BL, N], FP32)
            nc.gpsimd.memset(Ag0[:], 0.0)
            nc.gpsimd.memset(Ag1[:], 0.0)

            pm_cm = tc.tile_pool(name="ps_pm", bufs=1, space="PSUM")
            ps_pm = pm_cm.__enter__()
            po_cm = tc.tile_pool(name="ps_out", bufs=1, space="PSUM")
            ps_out = po_cm.__enter__()
            pm_tile = {}

            def phase_e_prelude(g):
                # pm = E . u can run as soon as u_sb is ready; F . A joins later
                pm = ps_pm.tile([GB * TOPK, N], FP32, tag="pm", name=f"pm{g}")
                pm_tile[g] = pm
                for c in range(2):
                    cs = slice(c * 512, (c + 1) * 512)
                    nc.tensor.matmul(
                        pm[:, cs], lhsT=e_sb[:, g * 80 : (g + 1) * 80],
                        rhs=u_sb[:, cs], start=True, stop=False,
                    )

            pa_cm, pa_tile, first = {}, {}, {}
            remaining = {0: len(G0_DVE) * NBLK, 1: len(G1_DVE) * NBLK}
            for g in (1, 0):  # stack allocator: group 0's pool closes first
                pa_cm[g] = tc.tile_pool(name=f"ps_pa{g}", bufs=1, space="PSUM")
                pool = pa_cm[g].__enter__()
                pa_tile[g] = pool.tile([MW, N], FP32, tag=f"pa{g}", name=f"pa{g}")
                first[g] = [True, True]

            def emit_act_query(b, ub):
                for t in range(NBLK):
                    sa = scrA.tile([128, N], FP32, tag="sa")
                    nc.scalar.activation(
                        out=sa, in_=ub, func=AFT.Abs,
                        bias=nuT[:, t, b : b + 1], scale=1.0,
                        accum_out=asumT[:, b, t : t + 1],
                    )

            def emit_dve_query(b, ub):
                g = 0 if b < GB else 1
                pa = pa_tile[g]
                for t in range(NBLK):
                    gps = b in GPS_SET or (b in TAIL_SPLIT and t >= 5)
                    eng = nc.gpsimd if gps else nc.vector
                    pool = scrP if gps else scrD
                    sd = pool.tile([128, N], BF16, tag="sdp" if gps else "sd", name="sd")
                    eng.tensor_scalar(
                        out=sd, in0=ub, scalar1=nuT[:, t, b : b + 1], scalar2=0.0,
                        op0=ALU.add, op1=ALU.is_gt,
                    )
                    remaining[g] -= 1
                    for c in range(2):
                        cs = slice(c * 512, (c + 1) * 512)
                        nc.tensor.matmul(
                            pa[:, cs], lhsT=W[:, t, b], rhs=sd[:, cs],
                            start=first[g][c], stop=remaining[g] == 0,
                        )
                        first[g][c] = False

            def combine_half(g, paw_g):
                # A[b,j] = u*(2r - N) + (T_b - 2(L_hi+L_mid+L_lo)), all 16 rows
                # (rows not hosted in this half read accumulated zeros -> junk,
                # only the hosted rows are consumed).  Group 0 (overlapped, not
                # latency-critical) sums the limb rows with accumulating SWDGE
                # DMAs; group 1 (the tail) uses three parallel DMAs on separate
                # issuers plus DVE adds.
                Lsum = consts.tile([BL, N], FP32, tag="Lsum")
                Lhi = consts.tile([BL, N], FP32, tag="Lhi")
                nc.sync.dma_start(out=Lhi, in_=paw_g[32 : 32 + BL, :])
                Lmid = consts.tile([BL, N], FP32, tag="Lmid")
                nc.scalar.dma_start(out=Lmid, in_=paw_g[48 : 48 + BL, :])
                Llo = consts.tile([BL, N], FP32, tag="Llo")
                nc.gpsimd.dma_start(out=Llo, in_=paw_g[64 : 64 + BL, :])
                tLa = cmbp.tile([BL, N], FP32, tag="cmbLa")
                nc.vector.tensor_tensor(out=tLa, in0=Lhi, in1=Lmid, op=ALU.add)
                nc.vector.tensor_tensor(out=Lsum, in0=tLa, in1=Llo, op=ALU.add)
                t1 = cmbp.tile([BL, N], FP32, tag="cmb1")
                nc.vector.tensor_scalar(
                    out=t1, in0=pa_tile[g][:BL, :], scalar1=2.0, scalar2=-float(N),
                    op0=ALU.mult, op1=ALU.add,
                )
                t2 = cmbp.tile([BL, N], FP32, tag="cmb2")
                nc.vector.tensor_tensor(out=t2, in0=t1, in1=u_sb, op=ALU.mult)
                t3 = cmbp.tile([BL, N], FP32, tag="cmb3")
                nc.vector.tensor_scalar(
                    out=t3, in0=Lsum, scalar1=-2.0, scalar2=Tb,
                    op0=ALU.mult, op1=ALU.add,
                )
                cmb = consts.tile([BL, N], FP32, tag=f"cmb{g}")
                nc.vector.tensor_tensor(out=cmb, in0=t2, in1=t3, op=ALU.add)
                return cmb

            def act_rows_to_ag(g, b0, nq, Ag):
                # DMA-transpose asumT[:, b0:b0+nq, :] into row form via a DRAM
                # bounce (src contiguity is 8-element runs along t -> cheap).
                adr = dramp.tile([nq, N], FP32, tag=f"adr{g}", name=f"adr{g}")
                dst = bass.AP(
                    tensor=adr[:].tensor, offset=adr[:].offset,
                    ap=[[1, 128], [N, nq], [128, NBLK]],
                )
                nc.sync.dma_start(out=dst, in_=asumT[:, b0 : b0 + nq, :])
                nc.scalar.dma_start(out=Ag[b0 : b0 + nq, :], in_=adr[:])

            def phase_e_act_part(g, Ag):
                # F.A contribution of the ACT-path rows; can run as soon as the
                # transposed ACT A-rows land, well before the DVE combine.
                pm = pm_tile[g]
                for c in range(2):
                    cs = slice(c * 512, (c + 1) * 512)
                    nc.tensor.matmul(
                        pm[:, cs], lhsT=fa_sb[:, g * 80 : (g + 1) * 80],
                        rhs=Ag[:, cs], start=False, stop=False,
                    )

            def phase_e(g, cmb):
                pm = pm_tile[g]
                for c in range(2):
                    cs = slice(c * 512, (c + 1) * 512)
                    nc.tensor.matmul(
                        pm[:, cs], lhsT=fd_sb[:, g * 80 : (g + 1) * 80],
                        rhs=cmb[:, cs], start=False, stop=True,
                    )
                nmx = small.tile([GB * TOPK, 1], FP32, tag="nmx")
                nc.vector.tensor_reduce(
                    out=nmx, in_=pm, axis=mybir.AxisListType.X,
                    op=ALU.max, negate=True,
                )
                exps = expp.tile([GB * TOPK, N], BF16, tag="exps")
                den = small.tile([GB * TOPK, 1], FP32, tag="den")
                nc.scalar.activation(
                    out=exps, in_=pm, func=AFT.Exp, bias=nmx, scale=1.0,
                    accum_out=den,
                )
                rden = small.tile([GB * TOPK, 1], FP32, tag="rden")
                nc.vector.reciprocal(rden, den)
                gr = small.tile([GB * TOPK, GB], BF16, tag="gr")
                nc.vector.tensor_scalar(
                    out=gr, in0=g_sb, scalar1=rden, scalar2=None, op0=ALU.mult
                )
                po = ps_out.tile([GB, N], FP32, tag="po", name=f"po{g}")
                og = expp.tile([GB, N], FP32, tag="og")
                for c in range(2):
                    cs = slice(c * 512, (c + 1) * 512)
                    nc.tensor.matmul(
                        po[:, cs], lhsT=gr, rhs=exps[:, cs], start=True, stop=True
                    )
                    nc.any.tensor_copy(og[:, cs], po[:, cs])
                    eng = nc.sync if c == 0 else nc.scalar
                    eng.dma_start(
                        out=out_t[g * GB : (g + 1) * GB, cs], in_=og[:, cs]
                    )

            def finalize_group(g):
                paw_g = consts.tile([MW, N], FP32, tag=f"paw{g}")
                nc.any.tensor_copy(paw_g, pa_tile[g])
                cmb = combine_half(g, paw_g)
                pa_cm[g].__exit__(None, None, None)
                phase_e(g, cmb)
                return paw_g

            paw0 = None
            for pi, (b0, b1) in enumerate(PAIRS):
                ub2 = bcast.tile([128, 2, N], FP32)
                base = u_dram[b0 : b0 + 1, :]
                src = bass.AP(
                    tensor=base.tensor, offset=base.offset,
                    ap=[[0, 128], [(b1 - b0) * N, 2], [1, N]],
                )
                nc.default_dma_engine.dma_start(out=ub2, in_=src)
                for k, b in enumerate((b0, b1)):
                    ub = ub2[:, k, :]
                    if b in ACT_SET:
                        emit_act_query(b, ub)
                    else:
                        emit_dve_query(b, ub)
                if pi == 2:
                    phase_e_prelude(0)
                    act_rows_to_ag(0, 0, 3, Ag0)
                    phase_e_act_part(0, Ag0)
                if pi == 3:
                    paw0 = finalize_group(0)
                if pi == 5:
                    phase_e_prelude(1)
                    act_rows_to_ag(1, 8, 2, Ag1)
                    phase_e_act_part(1, Ag1)
            finalize_group(1)
            po_cm.__exit__(None, None, None)
            pm_cm.__exit__(None, None, None)

            if debug_taps:
                nc.default_dma_engine.dma_start(out=dbg_u[:], in_=u_sb)
                nc.default_dma_engine.dma_start(out=dbg_a[:8], in_=Ag0[:8, :])
                nc.default_dma_engine.dma_start(out=dbg_a[8:], in_=Ag1[8:, :])
                nc.default_dma_engine.dma_start(
                    out=dbg_nut[:], in_=nuT[:].rearrange("p t b -> p (t b)")
                )
                nc.default_dma_engine.dma_start(out=dbg_paw[:], in_=paw0)

    nc.compile()
    return nc


_CACHE = {}


def _get_nc():
    if "nc" not in _CACHE:
        _CACHE["nc"] = _build_nc()
    return _CACHE["nc"]


def _in_maps(query, neighbors):
    query = np.ascontiguousarray(query, dtype=np.float32)
    neighbors = np.ascontiguousarray(neighbors, dtype=np.float32)
    return [
        {"query": query[c * BL : (c + 1) * BL], "neighbors": neighbors}
        for c in range(NCORES)
    ]


def _run(query, neighbors, **kw):
    nc = _get_nc()
    res = run_bass_kernel_spmd(nc, _in_maps(query, neighbors), list(range(NCORES)), **kw)
    out = np.concatenate([res.results[c]["out"] for c in range(NCORES)], axis=0)
    return out, res


def kernel(query, neighbors):
    out, _ = _run(query, neighbors)
    return out


def run_profiled(query, neighbors, **kw):
    out, res = _run(query, neighbors, trace=True, **kw)
    return out, res



# revision 2
# speedup vs baseline: 1.2472x; 1.2472x over previous
"""Trainium2 Bass kernel for deterministic NeuralSort soft-kNN (DKNN), v2.

Math (per query b over N neighbors):
    s_j   = -||q_b - x_j||^2
    A_j   = sum_i |s_j - s_i|
    P[r,j]= softmax_j(scaling[r] * s_j - A_j),  r = 0..K-1, scaling[r] = N+1-2(r+1)
    out_j = sum_r P[r,j]

Reductions:
  * s_j = u_j - ||q_b||^2 with u_j = 2 q_b.x_j - ||x_j||^2; the ||q||^2 term
    cancels (constant in j for A; constant per softmax row otherwise).
  * A_j = u_j*(2 r_j - N) - 2 L_j + T with r_j = #{i: u_i < u_j} and
    L_j = sum_{u_i < u_j} u_i.  T is constant per query => cancels in the
    row softmax and is dropped.
  * r_j / L_j come from a 0/1 compare matrix reduced on TensorE: the compare
    tiles are fp8 (exact 0/1 or +-1), u_i is decomposed into 5 exact fp8e4m3
    limbs (scales 4*16^-m), and the reduce runs as fp8 DoubleRow matmuls
    (2 k-tiles per pass, 0.5 cyc/col) accumulating counts + limb sums in
    fp32 PSUM.
  * ScalarE generates sign(u_i - u_j) tiles (func=Sign, scale=-1), Vector/
    GpSimd generate is_gt tiles; the convention difference is absorbed in
    host-constant coefficients of the final fp32 matmul.
  * P_max[80, N] for a group of 8 queries is ONE fp32 matmul per half:
    lhsT [56, 80] host consts over a stacked rhs [c2(8); Lsc(40); u(8)]
    where c2 = u .* counts, Lsc = s_m * limb-sum rows.

Sharding: data-parallel over B=128 queries across 8 cores (16 each, two
groups of 8 for softmax finalization); neighbors replicated.
"""

import numpy as np
import ml_dtypes

import concourse.bass as bass
import concourse.bacc as bacc
import concourse.tile as tile
from concourse import mybir
from concourse.masks import make_identity
from concourse.bass_utils import run_bass_kernel_spmd

AFT = mybir.ActivationFunctionType
ALU = mybir.AluOpType
FP32 = mybir.dt.float32
BF16 = mybir.dt.bfloat16
FP8 = mybir.dt.float8e4
PM = mybir.MatmulPerfMode

B, N, D, TOPK = 128, 1024, 128, 10
NCORES = 8
BL = B // NCORES          # 16 queries per core
NBLK = N // 128           # 8 row-blocks of the pairwise matrix
GROUPS = 2
GQ = BL // GROUPS         # 8 queries per softmax group
NL = 5                    # fp8 limbs per u value
LANES = 104               # pa rows: counts 0:8, (pad), limbs 64:104
LIMB0 = 64                # first limb lane
LSC = [4.0 * 2.0 ** (-4 * m) for m in range(NL)]

# Per-(query, t-block) engine for the compare-tile generation.
# A = ScalarE (Sign, +-1), D = VectorE (is_gt 0/1), P = GpSimd (is_gt 0/1).
ACT_Q = (0, 1, 8, 9)
SIGN_BL = {0: (0, 1), 1: (0, 1)}   # sign-convention bl per group
_NDH = 2  # number of DVE-heavy (5/3) queries
_DH_POOL = [2, 10, 3, 11, 5, 13, 6, 14]
ENG_T = {}
for _q in range(BL):
    if _q in ACT_Q:
        ENG_T[_q] = "AAAAAAAA"
    elif _q in _DH_POOL[:_NDH]:
        ENG_T[_q] = "DDDDDPPP"
    else:
        ENG_T[_q] = "DDDDPPPP"

PAIRS = ((0, 2), (1, 3), (4, 5), (6, 7), (8, 10), (9, 11), (12, 13), (14, 15))
FIN_A, FIN_B = 4, 6
# DMA queue for each query's u broadcast: S = sync, P = gpsimd
UBQ = {q: "S" for q in range(BL)}


def _host_consts():
    scaling = (N + 1 - 2.0 * (np.arange(TOPK) + 1)).astype(np.float64)
    # FdL [56, 80]: stack rows 0:8 u, 8:16 c2 (u .* counts), 16:56 Lsc (limb
    # sums, lane m,b at 16+8m+b).  Column (bl*10+r) computes
    # scaling[r]*u_j - A_j (+ const, dropped).
    # gt-convention (counts r_j, limb sums L):    -A = -2*(u.*r) + N*u + 2*Lsum
    # sign-convention (C' = -sum sgn, S' = -S):   -A = +(u.*C') - Lsum'
    fdl = np.zeros((GROUPS, 104, 80), np.float64)
    for g in range(GROUPS):
        for bl in range(GQ):
            is_sign = bl in SIGN_BL[g]
            for r in range(TOPK):
                col = bl * TOPK + r
                fdl[g, bl, col] = scaling[r] + (0.0 if is_sign else float(N))
                fdl[g, 32 + bl, col] = 1.0 if is_sign else -2.0
                for m in range(NL):
                    fdl[g, 64 + 8 * m + bl, col] = -1.0 if is_sign else 2.0
    # lscalev [40, 1]: Lsc = s_m * pa_limb_row
    lsc = np.zeros((40, 1), np.float32)
    for m in range(NL):
        for bl in range(GQ):
            lsc[8 * m + bl, 0] = LSC[m]
    G = np.zeros((GQ * TOPK, GQ), np.float32)
    for bl in range(GQ):
        G[bl * TOPK:(bl + 1) * TOPK, bl] = 1.0
    zeros_w3 = np.zeros((1, 4 * 2 * LANES * BL), ml_dtypes.float8_e4m3)
    return fdl.astype(np.float32), lsc, G, zeros_w3


def _build_nc(debug_taps=False):
    nc = bacc.Bacc(None, target_bir_lowering=False)

    q_in = nc.dram_tensor("query", [BL, D], FP32, kind="ExternalInput")
    x_in = nc.dram_tensor("neighbors", [N, D], FP32, kind="ExternalInput")
    out_t = nc.dram_tensor("out", [BL, N], FP32, kind="ExternalOutput")
    if debug_taps:
        dbg_u = nc.dram_tensor("dbg_u", [BL, N], FP32, kind="ExternalOutput")
        dbg_pa = nc.dram_tensor("dbg_pa", [GROUPS * LANES, N], FP32, kind="ExternalOutput")
        dbg_stk = nc.dram_tensor("dbg_stk", [GROUPS * 104, N], FP32, kind="ExternalOutput")
        dbg_pm = nc.dram_tensor("dbg_pm", [GROUPS * 80, N], FP32, kind="ExternalOutput")

    FdL, lscv, G, zw3 = _host_consts()
    fdl_in = [nc.inline_tensor(np.ascontiguousarray(FdL[g]), f"fdl{g}")
              for g in range(GROUPS)]
    lsc_in = nc.inline_tensor(lscv, "lscv")
    g_in = nc.inline_tensor(G, "lhs_g")
    zw3_in = nc.inline_tensor(zw3, "zw3")

    with tile.TileContext(nc) as tc:
        with (
            tc.tile_pool(name="consts", bufs=1) as consts,
            tc.tile_pool(name="xp", bufs=1) as xp,
            tc.tile_pool(name="bcast", bufs=16) as bcast,
            tc.tile_pool(name="scrS", bufs=16) as scrS,
            tc.tile_pool(name="small", bufs=8) as small,
            tc.tile_pool(name="dramp", bufs=1, space="DRAM") as dramp,
        ):
            ident = consts.tile([128, 128], FP32)
            make_identity(nc, ident)
            ones128 = consts.tile([128, 1], FP32)
            nc.vector.memset(ones128, 1.0)
            ones1xb = consts.tile([1, BL], FP32)
            nc.vector.memset(ones1xb, 1.0)
            ones1x128 = consts.tile([1, 128], FP32)
            nc.vector.memset(ones1x128, 1.0)
            ones1x72 = consts.tile([1, 72], FP32)
            nc.vector.memset(ones1x72, 0.0)
            nc.vector.memset(ones1x72[:, 0:8], 1.0)
            nc.vector.memset(ones1x72[:, 32:40], 1.0)
            nc.vector.memset(ones1x72[:, 64:65], 1.0)

            # ---- inputs in; W3 zero skeleton on Pool queue ----
            x_sb = xp.tile([128, NBLK, D], FP32)
            xv = x_in[:].rearrange("(t p) d -> p t d", p=128)
            half = NBLK // 2
            nc.sync.dma_start(out=x_sb[:, :half, :], in_=xv[:, :half, :])
            q_sb = small.tile([BL, D], FP32)
            nc.sync.dma_start(out=q_sb, in_=q_in[:])
            nc.sync.dma_start(out=x_sb[:, half:, :], in_=xv[:, half:, :])

            # W3 [128, tpair(4), tp(2), lane(48), q(16)] fp8
            W3 = consts.tile([128, 4, 2, LANES, BL], FP8)
            zsrc = zw3_in[:]
            zap = bass.AP(tensor=zsrc.tensor, offset=zsrc.offset,
                          ap=[[0, 128], [1, 4 * 2 * LANES * BL]])
            nc.sync.dma_start(
                out=W3[:].rearrange("p a b l q -> p (a b l q)"), in_=zap)

            fdl_sb = [consts.tile([104, 80], FP32, name=f"fdl_sb{g}")
                      for g in range(GROUPS)]
            for g in range(GROUPS):
                nc.sync.dma_start(out=fdl_sb[g], in_=fdl_in[g][:])
            lsc_sb = consts.tile([40, 1], FP32)
            nc.sync.dma_start(out=lsc_sb, in_=lsc_in[:])
            lscB = consts.tile([8, 1], FP32)
            nc.sync.dma_start(out=lscB, in_=lsc_in[32:40])
            g_sb = consts.tile([GQ * TOPK, GQ], FP32)
            nc.sync.dma_start(out=g_sb, in_=g_in[:])

            # ---- transposes: xT[d, j], q2T = (2Q)^T ----
            xT = xp.tile([128, N], FP32)
            q2T = consts.tile([128, BL], FP32)
            q2Tw = consts.tile([128, 72], FP32)
            with tc.tile_pool(name="ps_tr", bufs=2, space="PSUM") as ps_tr:
                for t in range(half):
                    ptr = ps_tr.tile([128, 128], FP32)
                    nc.tensor.transpose(ptr, x_sb[:, t, :], ident)
                    nc.vector.tensor_copy(xT[:, t * 128:(t + 1) * 128], ptr)
                pqt = ps_tr.tile([128, BL], FP32, tag="pqt")
                nc.tensor.transpose(pqt, q_sb, ident[:BL, :BL])
                nc.scalar.activation(out=q2T, in_=pqt, func=AFT.Copy, scale=2.0)
                nc.vector.memset(q2Tw[:, 8:32], 0.0)
                nc.vector.memset(q2Tw[:, 40:64], 0.0)
                nc.vector.memset(q2Tw[:, 65:72], 0.0)
                nc.scalar.activation(out=q2Tw[:, 0:8], in_=pqt[:, 0:8],
                                     func=AFT.Copy, scale=2.0)
                nc.scalar.activation(out=q2Tw[:, 32:40], in_=pqt[:, 8:16],
                                     func=AFT.Copy, scale=2.0)
                nc.scalar.activation(out=q2Tw[:, 64:65], in_=pqt[:, 2:3],
                                     func=AFT.Copy, scale=2.0)
                for t in range(half, NBLK):
                    ptr = ps_tr.tile([128, 128], FP32)
                    nc.tensor.transpose(ptr, x_sb[:, t, :], ident)
                    nc.vector.tensor_copy(xT[:, t * 128:(t + 1) * 128], ptr)

            # ---- row norms: negx2[1, j] = -||x_j||^2 ----
            sq = xp.tile([128, N], FP32)
            negx2 = consts.tile([1, N], FP32)
            with tc.tile_pool(name="ps_x2", bufs=1, space="PSUM") as ps_x2:
                px2 = ps_x2.tile([1, N], FP32)
                for c in range(2):
                    cs = slice(c * 512, (c + 1) * 512)
                    nc.scalar.activation(out=sq[:, cs], in_=xT[:, cs], func=AFT.Square)
                    nc.tensor.matmul(
                        px2[:, cs], lhsT=ones128, rhs=sq[:, cs], start=True, stop=True)
                    nc.scalar.activation(
                        out=negx2[:, cs], in_=px2[:, cs], func=AFT.Copy, scale=-1.0)

            # ---- u row form (u_sb -> u_dram) and uT column form ----
            u_sb = consts.tile([40, N], FP32)   # rows 0:8 = q0..7, 32:40 = q8..15
            uT = consts.tile([128, NBLK, BL], FP32)   # uT[p, t, b] = u[b, t*128+p]
            u_dram = dramp.tile([40, N], FP32)
            with tc.tile_pool(name="ps_u", bufs=1, space="PSUM") as ps_u, \
                 tc.tile_pool(name="ps_qt", bufs=2, space="PSUM") as ps_qt:
                pu = ps_u.tile([72, N], FP32)
                for c in range(2):
                    cs = slice(c * 512, (c + 1) * 512)
                    nc.tensor.matmul(
                        pu[:, cs], lhsT=q2Tw, rhs=xT[:, cs], start=True, stop=False)
                    nc.tensor.matmul(
                        pu[:, cs], lhsT=ones1x72, rhs=negx2[:, cs],
                        start=False, stop=True)
                    nc.scalar.activation(out=u_sb[:, cs], in_=pu[0:40, cs], func=AFT.Copy)
                    nc.sync.dma_start(out=u_dram[:, cs], in_=u_sb[:, cs])
                u2row = consts.tile([1, N], FP32, name="u2row")
                nc.vector.tensor_copy(u2row, pu[64:65, :])
                for t in range(NBLK):
                    put = ps_qt.tile([128, BL], FP32, tag="put")
                    nc.tensor.matmul(
                        put, lhsT=xT[:, t * 128:(t + 1) * 128], rhs=q2T,
                        start=True, stop=False)
                    nc.tensor.matmul(
                        put, lhsT=negx2[:, t * 128:(t + 1) * 128], rhs=ones1xb,
                        start=False, stop=True)
                    nc.vector.tensor_copy(uT[:, t, :], put)

            # ---- stacked rhs tiles (u rows placed early) ----
            stack = [[consts.tile([104, 512], FP32, name=f"stack{g}_{c}")
                      for c in range(2)] for g in range(GROUPS)]
            for g in range(GROUPS):
                for c in range(2):
                    cs = slice(c * 512, (c + 1) * 512)
                    nc.gpsimd.memset(stack[g][c][0:32, :], 0.0)
                    nc.gpsimd.memset(stack[g][c][32:64, :], 0.0)
                    nc.gpsimd.tensor_copy(stack[g][c][0:8, :],
                                          u_sb[g * 32:g * 32 + 8, cs])

            # ---- W3 diagonal lanes: counts + 5 fp8 limbs of uT ----
            # diag view for (lane-base lb, group g): [128, t(8), b(8)]
            def w3diag(lb, g):
                v = W3[:]
                return bass.AP(tensor=v.tensor, offset=v.offset + lb * BL + g * GQ,
                               ap=[list(v.ap[0]), [LANES * BL, NBLK], [BL + 1, GQ]])

            for g in range(GROUPS):
                nc.gpsimd.memset(w3diag(0, g), 1.0)
            for g in range(GROUPS):
                uTg = uT[:, :, g * GQ:(g + 1) * GQ]
                resid = bass.AP(tensor=uTg.tensor, offset=uTg.offset,
                                ap=[list(uTg.ap[0]), [BL, NBLK], [1, GQ]])
                for m in range(NL):
                    lane = w3diag(LIMB0 + 8 * m, g)
                    nc.vector.tensor_scalar(
                        out=lane, in0=resid, scalar1=1.0 / LSC[m], scalar2=None,
                        op0=ALU.mult)
                    if m < NL - 1:
                        back = small.tile([128, NBLK, GQ], FP32, tag="lback")
                        nc.vector.tensor_scalar(
                            out=back, in0=lane, scalar1=LSC[m], scalar2=None,
                            op0=ALU.mult)
                        nres = small.tile([128, NBLK, GQ], FP32, tag=f"lres{m % 2}")
                        nc.vector.tensor_tensor(
                            out=nres, in0=resid, in1=back, op=ALU.subtract)
                        resid = nres[:]

            # ---- pairwise phase ----
            pa_cm, pa_tile = {}, {}
            for g in (1, 0):
                pa_cm[g] = tc.tile_pool(name=f"ps_pa{g}", bufs=1, space="PSUM")
                pool = pa_cm[g].__enter__()
                pa_tile[g] = pool.tile([LANES, N], FP32, tag=f"pa{g}", name=f"pa{g}")
            first = {(g, c): True for g in range(GROUPS) for c in range(2)}
            left = {0: GQ * 4, 1: GQ * 4}

            ub_tile = {}

            def issue_ub(q):
                ub = bcast.tile([128, N], FP32, tag="ub", name=f"ub{q}")
                row = (q // GQ) * 32 + (q % GQ)
                eng = {"S": nc.sync, "P": nc.gpsimd, "A": nc.scalar}[UBQ[q]]
                for c in range(2):
                    cs = slice(c * 512, (c + 1) * 512)
                    base = u_dram[row:row + 1, cs]
                    src = bass.AP(tensor=base.tensor, offset=base.offset,
                                  ap=[[0, 128], [1, 512]])
                    eng.dma_start(out=ub[:, cs], in_=src)
                ub_tile[q] = ub

            def emit_query(q):
                g = q // GQ
                ub = ub_tile[q]
                for tp in range(4):
                    sd2 = scrS.tile([128, 2, N], FP8, tag="sd2", name=f"sd2_{q}_{tp}")
                    for th in range(2):
                        t = 2 * tp + th
                        e = ENG_T[q][t]
                        if e == "A":
                            nc.scalar.activation(
                                out=sd2[:, th, :], in_=ub, func=AFT.Sign,
                                bias=uT[:, t, q:q + 1], scale=-1.0)
                        else:
                            eng = nc.vector if e == "D" else nc.gpsimd
                            eng.tensor_scalar(
                                out=sd2[:, th, :], in0=ub,
                                scalar1=uT[:, t, q:q + 1], scalar2=0.0,
                                op0=ALU.subtract, op1=ALU.is_gt)
                    left[g] -= 1
                    for c in range(2):
                        cs = slice(c * 512, (c + 1) * 512)
                        nc.tensor.matmul(
                            pa_tile[g][:, cs], lhsT=W3[:, tp, :, :, q],
                            rhs=sd2[:, :, cs],
                            start=first[(g, c)], stop=left[g] == 0,
                            perf_mode=PM.DoubleRow)
                        first[(g, c)] = False

            pm_cm, pm_tile, po_cm, fin_state = {}, {}, {}, {}

            def finalize(g):
                pa = pa_tile[g]
                h0, h1 = slice(0, 512), slice(512, 1024)
                # c2 = u .* counts (rows 32:40); Lsc = s_m * limb rows (64:104)
                # halves in separate stack tiles so FdL h0 starts early.
                nc.vector.tensor_tensor(
                    out=stack[g][0][32:40, :], in0=pa[0:8, h0],
                    in1=u_sb[g * 32:g * 32 + 8, h0], op=ALU.mult)
                nc.scalar.activation(
                    out=stack[g][0][64:104, :], in_=pa[64:104, h0], func=AFT.Copy,
                    scale=lsc_sb)
                nc.vector.tensor_tensor(
                    out=stack[g][1][32:40, :], in0=pa[0:8, h1],
                    in1=u_sb[g * 32:g * 32 + 8, h1], op=ALU.mult)
                nc.scalar.activation(
                    out=stack[g][1][64:104, :], in_=pa[64:104, h1], func=AFT.Copy,
                    scale=lsc_sb)
                pa_cm[g].__exit__(None, None, None)
                pm_cm[g] = tc.tile_pool(name=f"ps_pm{g}", bufs=1, space="PSUM")
                pmp = pm_cm[g].__enter__()
                pm = pmp.tile([GQ * TOPK, N], FP32, tag=f"pm{g}", name=f"pm{g}")
                pm_tile[g] = pm
                # 128-col chunks: PE ramps on useful work; chunk 0/4 carry
                # start=True (pending-zero covers the whole 2KB region).
                for c in range(8):
                    cs = slice(c * 128, (c + 1) * 128)
                    hcs = slice((c % 4) * 128, (c % 4) * 128 + 128)
                    nc.tensor.matmul(
                        pm[:, cs], lhsT=fdl_sb[g], rhs=stack[g][c // 4][:, hcs],
                        start=(c % 4 == 0), stop=(c % 4 == 3),
                        skip_group_check=True)
                nmx0 = small.tile([GQ * TOPK, 1], FP32, tag="nmx0")
                nc.vector.tensor_reduce(
                    out=nmx0, in_=pm[:, h0], axis=mybir.AxisListType.X, op=ALU.max,
                    negate=True)
                nmx1 = small.tile([GQ * TOPK, 1], FP32, tag="nmx1")
                nc.vector.tensor_reduce(
                    out=nmx1, in_=pm[:, h1], axis=mybir.AxisListType.X, op=ALU.max,
                    negate=True)
                nmx = small.tile([GQ * TOPK, 1], FP32, tag="nmx")
                nc.vector.tensor_tensor(out=nmx, in0=nmx0, in1=nmx1, op=ALU.min)
                exps = consts.tile([GQ * TOPK, N], BF16, name=f"exps{g}")
                den = small.tile([GQ * TOPK, 1], FP32, tag="den")
                nc.scalar.activation(
                    out=exps, in_=pm, func=AFT.Exp, bias=nmx, scale=1.0,
                    accum_out=den)
                pm_cm[g].__exit__(None, None, None)
                fin_state[g] = (exps, den)

            def finalize_b(g):
                exps, den = fin_state[g]
                rden = small.tile([GQ * TOPK, 1], FP32, tag="rden")
                nc.vector.reciprocal(rden, den)
                gr = small.tile([GQ * TOPK, GQ], BF16, tag="gr")
                nc.vector.tensor_scalar(
                    out=gr, in0=g_sb, scalar1=rden, scalar2=None, op0=ALU.mult)
                po_cm[g] = tc.tile_pool(name=f"ps_po{g}", bufs=1, space="PSUM")
                pop = po_cm[g].__enter__()
                og = consts.tile([GQ, N], FP32, name=f"og{g}")
                for c in range(2):
                    cs = slice(c * 512, (c + 1) * 512)
                    po = pop.tile([GQ, 512], FP32, tag=f"po{g}_{c}", name=f"po{g}_{c}")
                    nc.tensor.matmul(
                        po, lhsT=gr, rhs=exps[:, cs], start=True, stop=True)
                    if g == 0 or c == 0:
                        nc.vector.tensor_copy(og[:, cs], po)
                    else:
                        nc.scalar.activation(out=og[:, cs], in_=po, func=AFT.Copy)
                    deng = nc.sync if (g == 0 or c == 0) else nc.gpsimd
                    deng.dma_start(
                        out=out_t[g * GQ:(g + 1) * GQ, cs], in_=og[:, cs])
                po_cm[g].__exit__(None, None, None)

            # queries 0 and 2 bootstrap via PE broadcast (no DRAM round-trip)
            bc_cm = tc.tile_pool(name="ps_bc", bufs=1, space="PSUM")
            bc_pool = bc_cm.__enter__()
            for q, ceng in ((0, nc.scalar), (2, nc.vector)):
                bc = bc_pool.tile([128, N], FP32, tag="bc", name=f"bc{q}")
                ub = bcast.tile([128, N], FP32, tag="ub", name=f"ub{q}")
                for c in range(2):
                    cs = slice(c * 512, (c + 1) * 512)
                    urow = u_sb[0:1, cs] if q == 0 else u2row[:, cs]
                    nc.tensor.matmul(
                        bc[:, cs], lhsT=ones1x128, rhs=urow,
                        start=True, stop=True)
                    if ceng is nc.scalar:
                        nc.scalar.activation(out=ub[:, cs], in_=bc[:, cs], func=AFT.Copy)
                    else:
                        ceng.tensor_copy(ub[:, cs], bc[:, cs])
                ub_tile[q] = ub
            bc_cm.__exit__(None, None, None)
            issue_ub(1)
            issue_ub(3)
            for pi, pr in enumerate(PAIRS):
                if pi + 2 < len(PAIRS):
                    for q in PAIRS[pi + 2]:
                        issue_ub(q)
                for q in pr:
                    emit_query(q)
                if pi == FIN_A:
                    finalize(0)
                if pi == FIN_B:
                    finalize_b(0)
            finalize(1)
            finalize_b(1)

            if debug_taps:
                nc.sync.dma_start(out=dbg_u[0:8, :], in_=u_sb[0:8, :])
                nc.sync.dma_start(out=dbg_u[8:16, :], in_=u_sb[32:40, :])
                for g in range(GROUPS):
                    for c in range(2):
                        cs = slice(c * 512, (c + 1) * 512)
                        nc.sync.dma_start(
                            out=dbg_stk[g * 104:(g + 1) * 104, cs], in_=stack[g][c])

    nc.compile()
    return nc


_CACHE = {}


def _get_nc():
    if "nc" not in _CACHE:
        _CACHE["nc"] = _build_nc()
    return _CACHE["nc"]


def _in_maps(query, neighbors):
    query = np.ascontiguousarray(query, dtype=np.float32)
    neighbors = np.ascontiguousarray(neighbors, dtype=np.float32)
    return [
        {"query": query[c * BL:(c + 1) * BL], "neighbors": neighbors}
        for c in range(NCORES)
    ]


def _run(query, neighbors, **kw):
    nc = _get_nc()
    res = run_bass_kernel_spmd(nc, _in_maps(query, neighbors), list(range(NCORES)), **kw)
    out = np.concatenate([res.results[c]["out"] for c in range(NCORES)], axis=0)
    return out, res


def kernel(query, neighbors):
    out, _ = _run(query, neighbors)
    return out


def run_profiled(query, neighbors, **kw):
    out, res = _run(query, neighbors, trace=True, **kw)
    return out, res


# revision 3
# speedup vs baseline: 1.2638x; 1.0133x over previous
"""Trainium2 Bass kernel for deterministic NeuralSort soft-kNN (DKNN), v2.

Math (per query b over N neighbors):
    s_j   = -||q_b - x_j||^2
    A_j   = sum_i |s_j - s_i|
    P[r,j]= softmax_j(scaling[r] * s_j - A_j),  r = 0..K-1, scaling[r] = N+1-2(r+1)
    out_j = sum_r P[r,j]

Reductions:
  * s_j = u_j - ||q_b||^2 with u_j = 2 q_b.x_j - ||x_j||^2; the ||q||^2 term
    cancels (constant in j for A; constant per softmax row otherwise).
  * A_j = u_j*(2 r_j - N) - 2 L_j + T with r_j = #{i: u_i < u_j} and
    L_j = sum_{u_i < u_j} u_i.  T is constant per query => cancels in the
    row softmax and is dropped.
  * r_j / L_j come from a 0/1 compare matrix reduced on TensorE: the compare
    tiles are fp8 (exact 0/1 or +-1), u_i is decomposed into 5 exact fp8e4m3
    limbs (scales 4*16^-m), and the reduce runs as fp8 DoubleRow matmuls
    (2 k-tiles per pass, 0.5 cyc/col) accumulating counts + limb sums in
    fp32 PSUM.
  * ScalarE generates sign(u_i - u_j) tiles (func=Sign, scale=-1), Vector/
    GpSimd generate is_gt tiles; the convention difference is absorbed in
    host-constant coefficients of the final fp32 matmul.
  * P_max[80, N] for a group of 8 queries is ONE fp32 matmul per half:
    lhsT [56, 80] host consts over a stacked rhs [c2(8); Lsc(40); u(8)]
    where c2 = u .* counts, Lsc = s_m * limb-sum rows.

Sharding: data-parallel over B=128 queries across 8 cores (16 each, two
groups of 8 for softmax finalization); neighbors replicated.
"""

import numpy as np
import ml_dtypes

import concourse.bass as bass
import concourse.bacc as bacc
import concourse.tile as tile
from concourse import mybir
from concourse.masks import make_identity
from concourse.bass_utils import run_bass_kernel_spmd

AFT = mybir.ActivationFunctionType
ALU = mybir.AluOpType
FP32 = mybir.dt.float32
BF16 = mybir.dt.bfloat16
FP8 = mybir.dt.float8e4
PM = mybir.MatmulPerfMode

B, N, D, TOPK = 128, 1024, 128, 10
NCORES = 8
BL = B // NCORES          # 16 queries per core
NBLK = N // 128           # 8 row-blocks of the pairwise matrix
GROUPS = 2
GQ = BL // GROUPS         # 8 queries per softmax group
NL = 5                    # fp8 limbs per u value
LANES = 104               # pa rows: counts 0:8, (pad), limbs 64:104
LIMB0 = 64                # first limb lane
LSC = [4.0 * 2.0 ** (-4 * m) for m in range(NL)]

# Per-(query, t-block) engine for the compare-tile generation.
# A = ScalarE (Sign, +-1), D = VectorE (is_gt 0/1), P = GpSimd (is_gt 0/1).
ACT_Q = (0, 1, 8, 9, 15)
SIGN_BL = {0: (0, 1), 1: (0, 1, 7)}   # sign-convention bl per group
_NDH = 2  # number of DVE-heavy (5/3) queries
_DH_POOL = [2, 10, 3, 11, 5, 13, 6, 14]
ENG_T = {}
for _q in range(BL):
    if _q in ACT_Q:
        ENG_T[_q] = "AAAAAAAA"
    elif _q in _DH_POOL[:_NDH]:
        ENG_T[_q] = "DDDDDPPP"
    else:
        ENG_T[_q] = "DDDDPPPP"

PAIRS = ((0, 2), (1, 3), (4, 5), (6, 7), (8, 10), (9, 11), (12, 13), (14, 15))
FIN_A, FIN_B = 4, 6
# DMA queue for each query's u broadcast: S = sync, P = gpsimd
UBQ = {q: "S" for q in range(BL)}


def _host_consts():
    scaling = (N + 1 - 2.0 * (np.arange(TOPK) + 1)).astype(np.float64)
    # FdL [56, 80]: stack rows 0:8 u, 8:16 c2 (u .* counts), 16:56 Lsc (limb
    # sums, lane m,b at 16+8m+b).  Column (bl*10+r) computes
    # scaling[r]*u_j - A_j (+ const, dropped).
    # gt-convention (counts r_j, limb sums L):    -A = -2*(u.*r) + N*u + 2*Lsum
    # sign-convention (C' = -sum sgn, S' = -S):   -A = +(u.*C') - Lsum'
    fdl = np.zeros((GROUPS, 104, 80), np.float64)
    for g in range(GROUPS):
        for bl in range(GQ):
            is_sign = bl in SIGN_BL[g]
            for r in range(TOPK):
                col = bl * TOPK + r
                fdl[g, bl, col] = scaling[r] + (0.0 if is_sign else float(N))
                fdl[g, 32 + bl, col] = 1.0 if is_sign else -2.0
                for m in range(NL):
                    fdl[g, 64 + 8 * m + bl, col] = -1.0 if is_sign else 2.0
    # lscalev [40, 1]: Lsc = s_m * pa_limb_row
    lsc = np.zeros((40, 1), np.float32)
    for m in range(NL):
        for bl in range(GQ):
            lsc[8 * m + bl, 0] = LSC[m]
    G = np.zeros((GQ * TOPK, GQ), np.float32)
    for bl in range(GQ):
        G[bl * TOPK:(bl + 1) * TOPK, bl] = 1.0
    zeros_w3 = np.zeros((1, 4 * 2 * LANES * BL), ml_dtypes.float8_e4m3)
    return fdl.astype(np.float32), lsc, G, zeros_w3


def _build_nc(debug_taps=False):
    nc = bacc.Bacc(None, target_bir_lowering=False)

    q_in = nc.dram_tensor("query", [BL, D], FP32, kind="ExternalInput")
    x_in = nc.dram_tensor("neighbors", [N, D], FP32, kind="ExternalInput")
    out_t = nc.dram_tensor("out", [BL, N], FP32, kind="ExternalOutput")
    if debug_taps:
        dbg_u = nc.dram_tensor("dbg_u", [BL, N], FP32, kind="ExternalOutput")
        dbg_pa = nc.dram_tensor("dbg_pa", [GROUPS * LANES, N], FP32, kind="ExternalOutput")
        dbg_stk = nc.dram_tensor("dbg_stk", [GROUPS * 104, N], FP32, kind="ExternalOutput")
        dbg_pm = nc.dram_tensor("dbg_pm", [GROUPS * 80, N], FP32, kind="ExternalOutput")

    FdL, lscv, G, zw3 = _host_consts()
    fdl_in = [nc.inline_tensor(np.ascontiguousarray(FdL[g]), f"fdl{g}")
              for g in range(GROUPS)]
    lsc_in = nc.inline_tensor(lscv, "lscv")
    g_in = nc.inline_tensor(G, "lhs_g")
    zw3_in = nc.inline_tensor(zw3, "zw3")

    with tile.TileContext(nc) as tc:
        with (
            tc.tile_pool(name="consts", bufs=1) as consts,
            tc.tile_pool(name="xp", bufs=1) as xp,
            tc.tile_pool(name="bcast", bufs=16) as bcast,
            tc.tile_pool(name="scrS", bufs=16) as scrS,
            tc.tile_pool(name="small", bufs=8) as small,
            tc.tile_pool(name="dramp", bufs=1, space="DRAM") as dramp,
        ):
            ident = consts.tile([128, 128], FP32)
            make_identity(nc, ident)
            ones128 = consts.tile([128, 1], FP32)
            nc.vector.memset(ones128, 1.0)
            ones1xb = consts.tile([1, BL], FP32)
            nc.vector.memset(ones1xb, 1.0)
            ones1x128 = consts.tile([1, 128], FP32)
            nc.vector.memset(ones1x128, 1.0)
            ones1x72 = consts.tile([1, 72], FP32)
            nc.vector.memset(ones1x72, 0.0)
            nc.vector.memset(ones1x72[:, 0:8], 1.0)
            nc.vector.memset(ones1x72[:, 32:40], 1.0)
            nc.vector.memset(ones1x72[:, 64:65], 1.0)

            # ---- inputs in; W3 zero skeleton on Pool queue ----
            x_sb = xp.tile([128, NBLK, D], FP32)
            xv = x_in[:].rearrange("(t p) d -> p t d", p=128)
            half = NBLK // 2
            nc.sync.dma_start(out=x_sb[:, :half, :], in_=xv[:, :half, :])
            q_sb = small.tile([BL, D], FP32)
            nc.sync.dma_start(out=q_sb, in_=q_in[:])
            nc.sync.dma_start(out=x_sb[:, half:, :], in_=xv[:, half:, :])

            # W3 [128, tpair(4), tp(2), lane(48), q(16)] fp8
            W3 = consts.tile([128, 4, 2, LANES, BL], FP8)
            zsrc = zw3_in[:]
            zap = bass.AP(tensor=zsrc.tensor, offset=zsrc.offset,
                          ap=[[0, 128], [1, 4 * 2 * LANES * BL]])
            nc.sync.dma_start(
                out=W3[:].rearrange("p a b l q -> p (a b l q)"), in_=zap)

            fdl_sb = [consts.tile([104, 80], FP32, name=f"fdl_sb{g}")
                      for g in range(GROUPS)]
            for g in range(GROUPS):
                nc.sync.dma_start(out=fdl_sb[g], in_=fdl_in[g][:])
            lsc_sb = consts.tile([40, 1], FP32)
            nc.sync.dma_start(out=lsc_sb, in_=lsc_in[:])
            lscB = consts.tile([8, 1], FP32)
            nc.sync.dma_start(out=lscB, in_=lsc_in[32:40])
            g_sb = consts.tile([GQ * TOPK, GQ], FP32)
            nc.sync.dma_start(out=g_sb, in_=g_in[:])

            # ---- transposes: xT[d, j], q2T = (2Q)^T ----
            xT = xp.tile([128, N], FP32)
            q2T = consts.tile([128, BL], FP32)
            q2Tw = consts.tile([128, 72], FP32)
            with tc.tile_pool(name="ps_tr", bufs=2, space="PSUM") as ps_tr:
                for t in range(half):
                    ptr = ps_tr.tile([128, 128], FP32)
                    nc.tensor.transpose(ptr, x_sb[:, t, :], ident)
                    nc.vector.tensor_copy(xT[:, t * 128:(t + 1) * 128], ptr)
                pqt = ps_tr.tile([128, BL], FP32, tag="pqt")
                nc.tensor.transpose(pqt, q_sb, ident[:BL, :BL])
                nc.scalar.activation(out=q2T, in_=pqt, func=AFT.Copy, scale=2.0)
                nc.vector.memset(q2Tw[:, 8:32], 0.0)
                nc.vector.memset(q2Tw[:, 40:64], 0.0)
                nc.vector.memset(q2Tw[:, 65:72], 0.0)
                nc.scalar.activation(out=q2Tw[:, 0:8], in_=pqt[:, 0:8],
                                     func=AFT.Copy, scale=2.0)
                nc.scalar.activation(out=q2Tw[:, 32:40], in_=pqt[:, 8:16],
                                     func=AFT.Copy, scale=2.0)
                nc.scalar.activation(out=q2Tw[:, 64:65], in_=pqt[:, 2:3],
                                     func=AFT.Copy, scale=2.0)
                for t in range(half, NBLK):
                    ptr = ps_tr.tile([128, 128], FP32)
                    nc.tensor.transpose(ptr, x_sb[:, t, :], ident)
                    nc.vector.tensor_copy(xT[:, t * 128:(t + 1) * 128], ptr)

            # ---- row norms: negx2[1, j] = -||x_j||^2 ----
            sq = xp.tile([128, N], FP32)
            negx2 = consts.tile([1, N], FP32)
            with tc.tile_pool(name="ps_x2", bufs=1, space="PSUM") as ps_x2:
                px2 = ps_x2.tile([1, N], FP32)
                for c in range(2):
                    cs = slice(c * 512, (c + 1) * 512)
                    nc.scalar.activation(out=sq[:, cs], in_=xT[:, cs], func=AFT.Square)
                    nc.tensor.matmul(
                        px2[:, cs], lhsT=ones128, rhs=sq[:, cs], start=True, stop=True)
                    nc.scalar.activation(
                        out=negx2[:, cs], in_=px2[:, cs], func=AFT.Copy, scale=-1.0)

            # ---- u row form (u_sb -> u_dram) and uT column form ----
            u_sb = consts.tile([40, N], FP32)   # rows 0:8 = q0..7, 32:40 = q8..15
            uT = consts.tile([128, NBLK, BL], FP32)   # uT[p, t, b] = u[b, t*128+p]
            u_dram = dramp.tile([40, N], FP32)
            with tc.tile_pool(name="ps_u", bufs=1, space="PSUM") as ps_u, \
                 tc.tile_pool(name="ps_qt", bufs=2, space="PSUM") as ps_qt:
                pu = ps_u.tile([72, N], FP32)
                for c in range(2):
                    cs = slice(c * 512, (c + 1) * 512)
                    nc.tensor.matmul(
                        pu[:, cs], lhsT=q2Tw, rhs=xT[:, cs], start=True, stop=False)
                    nc.tensor.matmul(
                        pu[:, cs], lhsT=ones1x72, rhs=negx2[:, cs],
                        start=False, stop=True)
                    nc.scalar.activation(out=u_sb[:, cs], in_=pu[0:40, cs], func=AFT.Copy)
                    nc.sync.dma_start(out=u_dram[:, cs], in_=u_sb[:, cs])
                u2row = consts.tile([1, N], FP32, name="u2row")
                nc.vector.tensor_copy(u2row, pu[64:65, :])
                for t in range(NBLK):
                    put = ps_qt.tile([128, BL], FP32, tag="put")
                    nc.tensor.matmul(
                        put, lhsT=xT[:, t * 128:(t + 1) * 128], rhs=q2T,
                        start=True, stop=False)
                    nc.tensor.matmul(
                        put, lhsT=negx2[:, t * 128:(t + 1) * 128], rhs=ones1xb,
                        start=False, stop=True)
                    nc.vector.tensor_copy(uT[:, t, :], put)

            # ---- stacked rhs tiles (u rows placed early) ----
            stack = [[consts.tile([104, 512], FP32, name=f"stack{g}_{c}")
                      for c in range(2)] for g in range(GROUPS)]
            for g in range(GROUPS):
                for c in range(2):
                    cs = slice(c * 512, (c + 1) * 512)
                    nc.gpsimd.memset(stack[g][c][0:32, :], 0.0)
                    nc.gpsimd.memset(stack[g][c][32:64, :], 0.0)
                    nc.gpsimd.tensor_copy(stack[g][c][0:8, :],
                                          u_sb[g * 32:g * 32 + 8, cs])

            # ---- W3 diagonal lanes: counts + 5 fp8 limbs of uT ----
            # diag view for (lane-base lb, group g): [128, t(8), b(8)]
            def w3diag(lb, g):
                v = W3[:]
                return bass.AP(tensor=v.tensor, offset=v.offset + lb * BL + g * GQ,
                               ap=[list(v.ap[0]), [LANES * BL, NBLK], [BL + 1, GQ]])

            for g in range(GROUPS):
                nc.gpsimd.memset(w3diag(0, g), 1.0)
            for g in range(GROUPS):
                uTg = uT[:, :, g * GQ:(g + 1) * GQ]
                resid = bass.AP(tensor=uTg.tensor, offset=uTg.offset,
                                ap=[list(uTg.ap[0]), [BL, NBLK], [1, GQ]])
                for m in range(NL):
                    lane = w3diag(LIMB0 + 8 * m, g)
                    nc.vector.tensor_scalar(
                        out=lane, in0=resid, scalar1=1.0 / LSC[m], scalar2=None,
                        op0=ALU.mult)
                    if m < NL - 1:
                        back = small.tile([128, NBLK, GQ], FP32, tag="lback")
                        nc.vector.tensor_scalar(
                            out=back, in0=lane, scalar1=LSC[m], scalar2=None,
                            op0=ALU.mult)
                        nres = small.tile([128, NBLK, GQ], FP32, tag=f"lres{m % 2}")
                        nc.vector.tensor_tensor(
                            out=nres, in0=resid, in1=back, op=ALU.subtract)
                        resid = nres[:]

            # ---- pairwise phase ----
            pa_cm, pa_tile = {}, {}
            for g in (1, 0):
                pa_cm[g] = tc.tile_pool(name=f"ps_pa{g}", bufs=1, space="PSUM")
                pool = pa_cm[g].__enter__()
                pa_tile[g] = pool.tile([LANES, N], FP32, tag=f"pa{g}", name=f"pa{g}")
            first = {(g, c): True for g in range(GROUPS) for c in range(2)}
            left = {0: GQ * 4, 1: GQ * 4}

            ub_tile = {}

            def issue_ub(q):
                ub = bcast.tile([128, N], FP32, tag="ub", name=f"ub{q}")
                row = (q // GQ) * 32 + (q % GQ)
                eng = {"S": nc.sync, "P": nc.gpsimd, "A": nc.scalar}[UBQ[q]]
                for c in range(2):
                    cs = slice(c * 512, (c + 1) * 512)
                    base = u_dram[row:row + 1, cs]
                    src = bass.AP(tensor=base.tensor, offset=base.offset,
                                  ap=[[0, 128], [1, 512]])
                    eng.dma_start(out=ub[:, cs], in_=src)
                ub_tile[q] = ub

            def emit_query(q):
                g = q // GQ
                ub = ub_tile[q]
                for tp in range(4):
                    sd2 = scrS.tile([128, 2, N], FP8, tag="sd2", name=f"sd2_{q}_{tp}")
                    for th in range(2):
                        t = 2 * tp + th
                        e = ENG_T[q][t]
                        if e == "A":
                            nc.scalar.activation(
                                out=sd2[:, th, :], in_=ub, func=AFT.Sign,
                                bias=uT[:, t, q:q + 1], scale=-1.0)
                        else:
                            eng = nc.vector if e == "D" else nc.gpsimd
                            eng.tensor_scalar(
                                out=sd2[:, th, :], in0=ub,
                                scalar1=uT[:, t, q:q + 1], scalar2=0.0,
                                op0=ALU.subtract, op1=ALU.is_gt)
                    left[g] -= 1
                    for c in range(2):
                        cs = slice(c * 512, (c + 1) * 512)
                        nc.tensor.matmul(
                            pa_tile[g][:, cs], lhsT=W3[:, tp, :, :, q],
                            rhs=sd2[:, :, cs],
                            start=first[(g, c)], stop=left[g] == 0,
                            perf_mode=PM.DoubleRow)
                        first[(g, c)] = False

            pm_cm, pm_tile, po_cm, fin_state = {}, {}, {}, {}

            def finalize(g):
                pa = pa_tile[g]
                h0, h1 = slice(0, 512), slice(512, 1024)
                # c2 = u .* counts (rows 32:40); Lsc = s_m * limb rows (64:104)
                # halves in separate stack tiles so FdL h0 starts early.
                nc.vector.tensor_tensor(
                    out=stack[g][0][32:40, :], in0=pa[0:8, h0],
                    in1=u_sb[g * 32:g * 32 + 8, h0], op=ALU.mult)
                nc.scalar.activation(
                    out=stack[g][0][64:104, :], in_=pa[64:104, h0], func=AFT.Copy,
                    scale=lsc_sb)
                nc.vector.tensor_tensor(
                    out=stack[g][1][32:40, :], in0=pa[0:8, h1],
                    in1=u_sb[g * 32:g * 32 + 8, h1], op=ALU.mult)
                nc.scalar.activation(
                    out=stack[g][1][64:104, :], in_=pa[64:104, h1], func=AFT.Copy,
                    scale=lsc_sb)
                pa_cm[g].__exit__(None, None, None)
                pm_cm[g] = tc.tile_pool(name=f"ps_pm{g}", bufs=1, space="PSUM")
                pmp = pm_cm[g].__enter__()
                pm = pmp.tile([GQ * TOPK, N], FP32, tag=f"pm{g}", name=f"pm{g}")
                pm_tile[g] = pm
                # 128-col chunks: PE ramps on useful work; chunk 0/4 carry
                # start=True (pending-zero covers the whole 2KB region).
                for c in range(8):
                    cs = slice(c * 128, (c + 1) * 128)
                    hcs = slice((c % 4) * 128, (c % 4) * 128 + 128)
                    nc.tensor.matmul(
                        pm[:, cs], lhsT=fdl_sb[g], rhs=stack[g][c // 4][:, hcs],
                        start=(c % 4 == 0), stop=(c % 4 == 3),
                        skip_group_check=True)
                nmx0 = small.tile([GQ * TOPK, 1], FP32, tag="nmx0")
                nc.vector.tensor_reduce(
                    out=nmx0, in_=pm[:, h0], axis=mybir.AxisListType.X, op=ALU.max,
                    negate=True)
                nmx1 = small.tile([GQ * TOPK, 1], FP32, tag="nmx1")
                nc.vector.tensor_reduce(
                    out=nmx1, in_=pm[:, h1], axis=mybir.AxisListType.X, op=ALU.max,
                    negate=True)
                nmx = small.tile([GQ * TOPK, 1], FP32, tag="nmx")
                nc.vector.tensor_tensor(out=nmx, in0=nmx0, in1=nmx1, op=ALU.min)
                exps = consts.tile([GQ * TOPK, N], BF16, name=f"exps{g}")
                den = small.tile([GQ * TOPK, 1], FP32, tag="den")
                nc.scalar.activation(
                    out=exps, in_=pm, func=AFT.Exp, bias=nmx, scale=1.0,
                    accum_out=den)
                pm_cm[g].__exit__(None, None, None)
                fin_state[g] = (exps, den)

            def finalize_b(g):
                exps, den = fin_state[g]
                rden = small.tile([GQ * TOPK, 1], FP32, tag="rden")
                nc.vector.reciprocal(rden, den)
                gr = small.tile([GQ * TOPK, GQ], BF16, tag="gr")
                nc.vector.tensor_scalar(
                    out=gr, in0=g_sb, scalar1=rden, scalar2=None, op0=ALU.mult)
                po_cm[g] = tc.tile_pool(name=f"ps_po{g}", bufs=1, space="PSUM")
                pop = po_cm[g].__enter__()
                og = consts.tile([GQ, N], FP32, name=f"og{g}")
                for c in range(2):
                    cs = slice(c * 512, (c + 1) * 512)
                    po = pop.tile([GQ, 512], FP32, tag=f"po{g}_{c}", name=f"po{g}_{c}")
                    nc.tensor.matmul(
                        po, lhsT=gr, rhs=exps[:, cs], start=True, stop=True)
                    if g == 0 or c == 0:
                        nc.vector.tensor_copy(og[:, cs], po)
                    else:
                        nc.scalar.activation(out=og[:, cs], in_=po, func=AFT.Copy)
                    deng = nc.sync if (g == 0 or c == 0) else nc.gpsimd
                    deng.dma_start(
                        out=out_t[g * GQ:(g + 1) * GQ, cs], in_=og[:, cs])
                po_cm[g].__exit__(None, None, None)

            # queries 0 and 2 bootstrap via PE broadcast (no DRAM round-trip)
            bc_cm = tc.tile_pool(name="ps_bc", bufs=1, space="PSUM")
            bc_pool = bc_cm.__enter__()
            for q, ceng in ((0, nc.scalar), (2, nc.vector)):
                bc = bc_pool.tile([128, N], FP32, tag="bc", name=f"bc{q}")
                ub = bcast.tile([128, N], FP32, tag="ub", name=f"ub{q}")
                for c in range(2):
                    cs = slice(c * 512, (c + 1) * 512)
                    urow = u_sb[0:1, cs] if q == 0 else u2row[:, cs]
                    nc.tensor.matmul(
                        bc[:, cs], lhsT=ones1x128, rhs=urow,
                        start=True, stop=True)
                    if ceng is nc.scalar:
                        nc.scalar.activation(out=ub[:, cs], in_=bc[:, cs], func=AFT.Copy)
                    else:
                        ceng.tensor_copy(ub[:, cs], bc[:, cs])
                ub_tile[q] = ub
            bc_cm.__exit__(None, None, None)
            issue_ub(1)
            issue_ub(3)
            for pi, pr in enumerate(PAIRS):
                if pi + 2 < len(PAIRS):
                    for q in PAIRS[pi + 2]:
                        issue_ub(q)
                for q in pr:
                    emit_query(q)
                if pi == FIN_A:
                    finalize(0)
                if pi == FIN_B:
                    finalize_b(0)
            finalize(1)
            finalize_b(1)

            if debug_taps:
                nc.sync.dma_start(out=dbg_u[0:8, :], in_=u_sb[0:8, :])
                nc.sync.dma_start(out=dbg_u[8:16, :], in_=u_sb[32:40, :])
                for g in range(GROUPS):
                    for c in range(2):
                        cs = slice(c * 512, (c + 1) * 512)
                        nc.sync.dma_start(
                            out=dbg_stk[g * 104:(g + 1) * 104, cs], in_=stack[g][c])

    nc.compile()
    return nc


_CACHE = {}


def _get_nc():
    if "nc" not in _CACHE:
        _CACHE["nc"] = _build_nc()
    return _CACHE["nc"]


def _in_maps(query, neighbors):
    query = np.ascontiguousarray(query, dtype=np.float32)
    neighbors = np.ascontiguousarray(neighbors, dtype=np.float32)
    return [
        {"query": query[c * BL:(c + 1) * BL], "neighbors": neighbors}
        for c in range(NCORES)
    ]


def _run(query, neighbors, **kw):
    nc = _get_nc()
    res = run_bass_kernel_spmd(nc, _in_maps(query, neighbors), list(range(NCORES)), **kw)
    out = np.concatenate([res.results[c]["out"] for c in range(NCORES)], axis=0)
    return out, res


def kernel(query, neighbors):
    out, _ = _run(query, neighbors)
    return out


def run_profiled(query, neighbors, **kw):
    out, res = _run(query, neighbors, trace=True, **kw)
    return out, res


# revision 4
# speedup vs baseline: 1.2965x; 1.0259x over previous
"""Trainium2 Bass kernel for deterministic NeuralSort soft-kNN (DKNN), v2.

Math (per query b over N neighbors):
    s_j   = -||q_b - x_j||^2
    A_j   = sum_i |s_j - s_i|
    P[r,j]= softmax_j(scaling[r] * s_j - A_j),  r = 0..K-1, scaling[r] = N+1-2(r+1)
    out_j = sum_r P[r,j]

Reductions:
  * s_j = u_j - ||q_b||^2 with u_j = 2 q_b.x_j - ||x_j||^2; the ||q||^2 term
    cancels (constant in j for A; constant per softmax row otherwise).
  * A_j = u_j*(2 r_j - N) - 2 L_j + T with r_j = #{i: u_i < u_j} and
    L_j = sum_{u_i < u_j} u_i.  T is constant per query => cancels in the
    row softmax and is dropped.
  * r_j / L_j come from a 0/1 compare matrix reduced on TensorE: the compare
    tiles are fp8 (exact 0/1 or +-1), u_i is decomposed into 5 exact fp8e4m3
    limbs (scales 4*16^-m), and the reduce runs as fp8 DoubleRow matmuls
    (2 k-tiles per pass, 0.5 cyc/col) accumulating counts + limb sums in
    fp32 PSUM.
  * ScalarE generates sign(u_i - u_j) tiles (func=Sign, scale=-1), Vector/
    GpSimd generate is_gt tiles; the convention difference is absorbed in
    host-constant coefficients of the final fp32 matmul.
  * P_max[80, N] for a group of 8 queries is ONE fp32 matmul per half:
    lhsT [56, 80] host consts over a stacked rhs [c2(8); Lsc(40); u(8)]
    where c2 = u .* counts, Lsc = s_m * limb-sum rows.

Sharding: data-parallel over B=128 queries across 8 cores (16 each, two
groups of 8 for softmax finalization); neighbors replicated.
"""

import numpy as np
import ml_dtypes

import concourse.bass as bass
import concourse.bacc as bacc
import concourse.tile as tile
from concourse import mybir
from concourse.masks import make_identity
from concourse.bass_utils import run_bass_kernel_spmd

AFT = mybir.ActivationFunctionType
ALU = mybir.AluOpType
FP32 = mybir.dt.float32
BF16 = mybir.dt.bfloat16
FP8 = mybir.dt.float8e4
PM = mybir.MatmulPerfMode

B, N, D, TOPK = 128, 1024, 128, 10
NCORES = 8
BL = B // NCORES          # 16 queries per core
NBLK = N // 128           # 8 row-blocks of the pairwise matrix
GROUPS = 2
GQ = BL // GROUPS         # 8 queries per softmax group
NL = 5                    # fp8 limbs per u value
LANES = 104               # pa rows: counts 0:8, (pad), limbs 64:104
LIMB0 = 64                # first limb lane
LSC = [4.0 * 2.0 ** (-4 * m) for m in range(NL)]

# Per-(query, t-block) engine for the compare-tile generation.
# A = ScalarE (Sign, +-1), D = VectorE (is_gt 0/1), P = GpSimd (is_gt 0/1).
ACT_Q = (0, 1, 8, 9, 15)
SIGN_BL = {0: (0, 1), 1: (0, 1, 7)}   # sign-convention bl per group
_NDH = 4  # number of DVE-heavy (5/3) queries
_DH_POOL = [2, 10, 3, 11, 5, 13, 6, 14]
ENG_T = {}
for _q in range(BL):
    if _q in ACT_Q:
        ENG_T[_q] = "AAAAAAAA"
    elif _q in _DH_POOL[:_NDH]:
        ENG_T[_q] = "DDDDDPPP"
    else:
        ENG_T[_q] = "DDDDPPPP"

PAIRS = ((0, 2), (1, 3), (4, 5), (6, 7), (8, 10), (9, 11), (12, 13), (14, 15))
FIN_A, FIN_B = 4, 6
# DMA queue for each query's u broadcast: S = sync, P = gpsimd
UBQ = {q: "S" for q in range(BL)}


def _host_consts():
    scaling = (N + 1 - 2.0 * (np.arange(TOPK) + 1)).astype(np.float64)
    # FdL [56, 80]: stack rows 0:8 u, 8:16 c2 (u .* counts), 16:56 Lsc (limb
    # sums, lane m,b at 16+8m+b).  Column (bl*10+r) computes
    # scaling[r]*u_j - A_j (+ const, dropped).
    # gt-convention (counts r_j, limb sums L):    -A = -2*(u.*r) + N*u + 2*Lsum
    # sign-convention (C' = -sum sgn, S' = -S):   -A = +(u.*C') - Lsum'
    fdl = np.zeros((GROUPS, 104, 80), np.float64)
    for g in range(GROUPS):
        for bl in range(GQ):
            is_sign = bl in SIGN_BL[g]
            for r in range(TOPK):
                col = bl * TOPK + r
                fdl[g, bl, col] = scaling[r] + (0.0 if is_sign else float(N))
                fdl[g, 32 + bl, col] = 1.0 if is_sign else -2.0
                for m in range(NL):
                    fdl[g, 64 + 8 * m + bl, col] = -1.0 if is_sign else 2.0
    # lscalev [40, 1]: Lsc = s_m * pa_limb_row
    lsc = np.zeros((40, 1), np.float32)
    for m in range(NL):
        for bl in range(GQ):
            lsc[8 * m + bl, 0] = LSC[m]
    G = np.zeros((GQ * TOPK, GQ), np.float32)
    for bl in range(GQ):
        G[bl * TOPK:(bl + 1) * TOPK, bl] = 1.0
    zeros_w3 = np.zeros((1, 4 * 2 * LANES * BL), ml_dtypes.float8_e4m3)
    return fdl.astype(np.float32), lsc, G, zeros_w3


def _build_nc(debug_taps=False):
    nc = bacc.Bacc(None, target_bir_lowering=False)

    q_in = nc.dram_tensor("query", [BL, D], FP32, kind="ExternalInput")
    x_in = nc.dram_tensor("neighbors", [N, D], FP32, kind="ExternalInput")
    out_t = nc.dram_tensor("out", [BL, N], FP32, kind="ExternalOutput")
    if debug_taps:
        dbg_u = nc.dram_tensor("dbg_u", [BL, N], FP32, kind="ExternalOutput")
        dbg_pa = nc.dram_tensor("dbg_pa", [GROUPS * LANES, N], FP32, kind="ExternalOutput")
        dbg_stk = nc.dram_tensor("dbg_stk", [GROUPS * 104, N], FP32, kind="ExternalOutput")
        dbg_pm = nc.dram_tensor("dbg_pm", [GROUPS * 80, N], FP32, kind="ExternalOutput")

    FdL, lscv, G, zw3 = _host_consts()
    fdl_in = [nc.inline_tensor(np.ascontiguousarray(FdL[g]), f"fdl{g}")
              for g in range(GROUPS)]
    lsc_in = nc.inline_tensor(lscv, "lscv")
    g_in = nc.inline_tensor(G, "lhs_g")
    zw3_in = nc.inline_tensor(zw3, "zw3")

    with tile.TileContext(nc) as tc:
        with (
            tc.tile_pool(name="consts", bufs=1) as consts,
            tc.tile_pool(name="xp", bufs=1) as xp,
            tc.tile_pool(name="bcast", bufs=12) as bcast,
            tc.tile_pool(name="scrS", bufs=24) as scrS,
            tc.tile_pool(name="small", bufs=8) as small,
            tc.tile_pool(name="dramp", bufs=1, space="DRAM") as dramp,
        ):
            ident = consts.tile([128, 128], FP32)
            make_identity(nc, ident)
            ones128 = consts.tile([128, 1], FP32)
            nc.vector.memset(ones128, 1.0)
            ones1xb = consts.tile([1, BL], FP32)
            nc.vector.memset(ones1xb, 1.0)
            ones1x128 = consts.tile([1, 128], FP32)
            nc.vector.memset(ones1x128, 1.0)
            ones1x72 = consts.tile([1, 72], FP32)
            nc.vector.memset(ones1x72, 0.0)
            nc.vector.memset(ones1x72[:, 0:8], 1.0)
            nc.vector.memset(ones1x72[:, 32:40], 1.0)
            nc.vector.memset(ones1x72[:, 64:65], 1.0)

            # ---- inputs in; W3 zero skeleton on Pool queue ----
            x_sb = xp.tile([128, NBLK, D], FP32)
            xv = x_in[:].rearrange("(t p) d -> p t d", p=128)
            half = NBLK // 2
            nc.sync.dma_start(out=x_sb[:, :half, :], in_=xv[:, :half, :])
            q_sb = small.tile([BL, D], FP32)
            nc.sync.dma_start(out=q_sb, in_=q_in[:])
            nc.sync.dma_start(out=x_sb[:, half:, :], in_=xv[:, half:, :])

            # W3 [128, tpair(4), tp(2), lane(48), q(16)] fp8
            W3 = consts.tile([128, 4, 2, LANES, BL], FP8)
            zsrc = zw3_in[:]
            zap = bass.AP(tensor=zsrc.tensor, offset=zsrc.offset,
                          ap=[[0, 128], [1, 4 * 2 * LANES * BL]])
            nc.sync.dma_start(
                out=W3[:].rearrange("p a b l q -> p (a b l q)"), in_=zap)

            fdl_sb = [consts.tile([104, 80], FP32, name=f"fdl_sb{g}")
                      for g in range(GROUPS)]
            for g in range(GROUPS):
                nc.sync.dma_start(out=fdl_sb[g], in_=fdl_in[g][:])
            lsc_sb = consts.tile([40, 1], FP32)
            nc.sync.dma_start(out=lsc_sb, in_=lsc_in[:])
            lscB = consts.tile([8, 1], FP32)
            nc.sync.dma_start(out=lscB, in_=lsc_in[32:40])
            g_sb = consts.tile([GQ * TOPK, GQ], FP32)
            nc.sync.dma_start(out=g_sb, in_=g_in[:])

            # ---- transposes: xT[d, j], q2T = (2Q)^T ----
            xT = xp.tile([128, N], FP32)
            q2T = consts.tile([128, BL], FP32)
            q2Tw = consts.tile([128, 72], FP32)
            with tc.tile_pool(name="ps_tr", bufs=2, space="PSUM") as ps_tr:
                for t in range(half):
                    ptr = ps_tr.tile([128, 128], FP32)
                    nc.tensor.transpose(ptr, x_sb[:, t, :], ident)
                    nc.vector.tensor_copy(xT[:, t * 128:(t + 1) * 128], ptr)
                pqt = ps_tr.tile([128, BL], FP32, tag="pqt")
                nc.tensor.transpose(pqt, q_sb, ident[:BL, :BL])
                nc.scalar.activation(out=q2T, in_=pqt, func=AFT.Copy, scale=2.0)
                nc.vector.memset(q2Tw[:, 8:32], 0.0)
                nc.vector.memset(q2Tw[:, 40:64], 0.0)
                nc.vector.memset(q2Tw[:, 65:72], 0.0)
                nc.scalar.activation(out=q2Tw[:, 0:8], in_=pqt[:, 0:8],
                                     func=AFT.Copy, scale=2.0)
                nc.scalar.activation(out=q2Tw[:, 32:40], in_=pqt[:, 8:16],
                                     func=AFT.Copy, scale=2.0)
                nc.scalar.activation(out=q2Tw[:, 64:65], in_=pqt[:, 2:3],
                                     func=AFT.Copy, scale=2.0)
                for t in range(half, NBLK):
                    ptr = ps_tr.tile([128, 128], FP32)
                    nc.tensor.transpose(ptr, x_sb[:, t, :], ident)
                    nc.vector.tensor_copy(xT[:, t * 128:(t + 1) * 128], ptr)

            # ---- row norms: negx2[1, j] = -||x_j||^2 ----
            sq = xp.tile([128, N], FP32)
            negx2 = consts.tile([1, N], FP32)
            with tc.tile_pool(name="ps_x2", bufs=1, space="PSUM") as ps_x2:
                px2 = ps_x2.tile([1, N], FP32)
                for c in range(2):
                    cs = slice(c * 512, (c + 1) * 512)
                    nc.scalar.activation(out=sq[:, cs], in_=xT[:, cs], func=AFT.Square)
                    nc.tensor.matmul(
                        px2[:, cs], lhsT=ones128, rhs=sq[:, cs], start=True, stop=True)
                    nc.scalar.activation(
                        out=negx2[:, cs], in_=px2[:, cs], func=AFT.Copy, scale=-1.0)

            # ---- u row form (u_sb -> u_dram) and uT column form ----
            u_sb = consts.tile([40, N], FP32)   # rows 0:8 = q0..7, 32:40 = q8..15
            uT = consts.tile([128, NBLK, BL], FP32)   # uT[p, t, b] = u[b, t*128+p]
            u_dram = dramp.tile([40, N], FP32)
            with tc.tile_pool(name="ps_u", bufs=1, space="PSUM") as ps_u, \
                 tc.tile_pool(name="ps_qt", bufs=2, space="PSUM") as ps_qt:
                pu = ps_u.tile([72, N], FP32)
                for c in range(2):
                    cs = slice(c * 512, (c + 1) * 512)
                    nc.tensor.matmul(
                        pu[:, cs], lhsT=q2Tw, rhs=xT[:, cs], start=True, stop=False)
                    nc.tensor.matmul(
                        pu[:, cs], lhsT=ones1x72, rhs=negx2[:, cs],
                        start=False, stop=True)
                    nc.scalar.activation(out=u_sb[:, cs], in_=pu[0:40, cs], func=AFT.Copy)
                    nc.sync.dma_start(out=u_dram[:, cs], in_=u_sb[:, cs])
                u2row = consts.tile([1, N], FP32, name="u2row")
                nc.vector.tensor_copy(u2row, pu[64:65, :])
                for t in range(NBLK):
                    put = ps_qt.tile([128, BL], FP32, tag="put")
                    nc.tensor.matmul(
                        put, lhsT=xT[:, t * 128:(t + 1) * 128], rhs=q2T,
                        start=True, stop=False)
                    nc.tensor.matmul(
                        put, lhsT=negx2[:, t * 128:(t + 1) * 128], rhs=ones1xb,
                        start=False, stop=True)
                    nc.vector.tensor_copy(uT[:, t, :], put)

            # ---- stacked rhs tiles (u rows placed early) ----
            stack = [[consts.tile([104, 512], FP32, name=f"stack{g}_{c}")
                      for c in range(2)] for g in range(GROUPS)]
            for g in range(GROUPS):
                for c in range(2):
                    cs = slice(c * 512, (c + 1) * 512)
                    nc.gpsimd.memset(stack[g][c][0:32, :], 0.0)
                    nc.gpsimd.memset(stack[g][c][32:64, :], 0.0)
                    nc.gpsimd.tensor_copy(stack[g][c][0:8, :],
                                          u_sb[g * 32:g * 32 + 8, cs])

            # ---- W3 diagonal lanes: counts + 5 fp8 limbs of uT ----
            # diag view for (lane-base lb, group g): [128, t(8), b(8)]
            def w3diag(lb, g):
                v = W3[:]
                return bass.AP(tensor=v.tensor, offset=v.offset + lb * BL + g * GQ,
                               ap=[list(v.ap[0]), [LANES * BL, NBLK], [BL + 1, GQ]])

            for g in range(GROUPS):
                nc.gpsimd.memset(w3diag(0, g), 1.0)
            for g in range(GROUPS):
                uTg = uT[:, :, g * GQ:(g + 1) * GQ]
                resid = bass.AP(tensor=uTg.tensor, offset=uTg.offset,
                                ap=[list(uTg.ap[0]), [BL, NBLK], [1, GQ]])
                for m in range(NL):
                    lane = w3diag(LIMB0 + 8 * m, g)
                    nc.vector.tensor_scalar(
                        out=lane, in0=resid, scalar1=1.0 / LSC[m], scalar2=None,
                        op0=ALU.mult)
                    if m < NL - 1:
                        back = small.tile([128, NBLK, GQ], FP32, tag="lback")
                        nc.vector.tensor_scalar(
                            out=back, in0=lane, scalar1=LSC[m], scalar2=None,
                            op0=ALU.mult)
                        nres = small.tile([128, NBLK, GQ], FP32, tag=f"lres{m % 2}")
                        nc.vector.tensor_tensor(
                            out=nres, in0=resid, in1=back, op=ALU.subtract)
                        resid = nres[:]

            # ---- pairwise phase ----
            pa_cm, pa_tile = {}, {}
            for g in (1, 0):
                pa_cm[g] = tc.tile_pool(name=f"ps_pa{g}", bufs=1, space="PSUM")
                pool = pa_cm[g].__enter__()
                pa_tile[g] = pool.tile([LANES, N], FP32, tag=f"pa{g}", name=f"pa{g}")
            first = {(g, c): True for g in range(GROUPS) for c in range(2)}
            left = {0: GQ * 4, 1: GQ * 4}

            ub_tile = {}

            def issue_ub(q):
                ub = bcast.tile([128, N], FP32, tag="ub", name=f"ub{q}")
                row = (q // GQ) * 32 + (q % GQ)
                eng = {"S": nc.sync, "P": nc.gpsimd, "A": nc.scalar}[UBQ[q]]
                for c in range(2):
                    cs = slice(c * 512, (c + 1) * 512)
                    base = u_dram[row:row + 1, cs]
                    src = bass.AP(tensor=base.tensor, offset=base.offset,
                                  ap=[[0, 128], [1, 512]])
                    eng.dma_start(out=ub[:, cs], in_=src)
                ub_tile[q] = ub

            def emit_query(q):
                g = q // GQ
                ub = ub_tile[q]
                for tp in range(4):
                    sd2 = scrS.tile([128, 2, N], FP8, tag="sd2", name=f"sd2_{q}_{tp}")
                    for th in range(2):
                        t = 2 * tp + th
                        e = ENG_T[q][t]
                        if e == "A":
                            nc.scalar.activation(
                                out=sd2[:, th, :], in_=ub, func=AFT.Sign,
                                bias=uT[:, t, q:q + 1], scale=-1.0)
                        else:
                            eng = nc.vector if e == "D" else nc.gpsimd
                            eng.tensor_scalar(
                                out=sd2[:, th, :], in0=ub,
                                scalar1=uT[:, t, q:q + 1], scalar2=0.0,
                                op0=ALU.subtract, op1=ALU.is_gt)
                    left[g] -= 1
                    for c in range(2):
                        cs = slice(c * 512, (c + 1) * 512)
                        nc.tensor.matmul(
                            pa_tile[g][:, cs], lhsT=W3[:, tp, :, :, q],
                            rhs=sd2[:, :, cs],
                            start=first[(g, c)], stop=left[g] == 0,
                            perf_mode=PM.DoubleRow)
                        first[(g, c)] = False

            pm_cm, pm_tile, po_cm, fin_state = {}, {}, {}, {}

            def finalize(g):
                pa = pa_tile[g]
                h0, h1 = slice(0, 512), slice(512, 1024)
                # c2 = u .* counts (rows 32:40); Lsc = s_m * limb rows (64:104)
                # halves in separate stack tiles so FdL h0 starts early.
                nc.vector.tensor_tensor(
                    out=stack[g][0][32:40, :], in0=pa[0:8, h0],
                    in1=u_sb[g * 32:g * 32 + 8, h0], op=ALU.mult)
                nc.scalar.activation(
                    out=stack[g][0][64:104, :], in_=pa[64:104, h0], func=AFT.Copy,
                    scale=lsc_sb)
                nc.vector.tensor_tensor(
                    out=stack[g][1][32:40, :], in0=pa[0:8, h1],
                    in1=u_sb[g * 32:g * 32 + 8, h1], op=ALU.mult)
                nc.scalar.activation(
                    out=stack[g][1][64:104, :], in_=pa[64:104, h1], func=AFT.Copy,
                    scale=lsc_sb)
                pa_cm[g].__exit__(None, None, None)
                pm_cm[g] = tc.tile_pool(name=f"ps_pm{g}", bufs=1, space="PSUM")
                pmp = pm_cm[g].__enter__()
                pm = pmp.tile([GQ * TOPK, N], FP32, tag=f"pm{g}", name=f"pm{g}")
                pm_tile[g] = pm
                # 128-col chunks: PE ramps on useful work; chunk 0/4 carry
                # start=True (pending-zero covers the whole 2KB region).
                for c in range(8):
                    cs = slice(c * 128, (c + 1) * 128)
                    hcs = slice((c % 4) * 128, (c % 4) * 128 + 128)
                    nc.tensor.matmul(
                        pm[:, cs], lhsT=fdl_sb[g], rhs=stack[g][c // 4][:, hcs],
                        start=(c % 4 == 0), stop=(c % 4 == 3),
                        skip_group_check=True)
                nmx0 = small.tile([GQ * TOPK, 1], FP32, tag="nmx0")
                nc.vector.tensor_reduce(
                    out=nmx0, in_=pm[:, h0], axis=mybir.AxisListType.X, op=ALU.max,
                    negate=True)
                nmx1 = small.tile([GQ * TOPK, 1], FP32, tag="nmx1")
                nc.vector.tensor_reduce(
                    out=nmx1, in_=pm[:, h1], axis=mybir.AxisListType.X, op=ALU.max,
                    negate=True)
                nmx = small.tile([GQ * TOPK, 1], FP32, tag="nmx")
                nc.vector.tensor_tensor(out=nmx, in0=nmx0, in1=nmx1, op=ALU.min)
                exps = consts.tile([GQ * TOPK, N], BF16, name=f"exps{g}")
                den = small.tile([GQ * TOPK, 1], FP32, tag="den")
                nc.scalar.activation(
                    out=exps, in_=pm, func=AFT.Exp, bias=nmx, scale=1.0,
                    accum_out=den)
                pm_cm[g].__exit__(None, None, None)
                fin_state[g] = (exps, den)

            def finalize_b(g):
                exps, den = fin_state[g]
                rden = small.tile([GQ * TOPK, 1], FP32, tag="rden")
                nc.vector.reciprocal(rden, den)
                gr = small.tile([GQ * TOPK, GQ], BF16, tag="gr")
                nc.vector.tensor_scalar(
                    out=gr, in0=g_sb, scalar1=rden, scalar2=None, op0=ALU.mult)
                po_cm[g] = tc.tile_pool(name=f"ps_po{g}", bufs=1, space="PSUM")
                pop = po_cm[g].__enter__()
                og = consts.tile([GQ, N], FP32, name=f"og{g}")
                for c in range(2):
                    cs = slice(c * 512, (c + 1) * 512)
                    po = pop.tile([GQ, 512], FP32, tag=f"po{g}_{c}", name=f"po{g}_{c}")
                    nc.tensor.matmul(
                        po, lhsT=gr, rhs=exps[:, cs], start=True, stop=True)
                    if g == 0 or c == 0:
                        nc.vector.tensor_copy(og[:, cs], po)
                    else:
                        nc.scalar.activation(out=og[:, cs], in_=po, func=AFT.Copy)
                    deng = nc.sync if (g == 0 or c == 0) else nc.gpsimd
                    deng.dma_start(
                        out=out_t[g * GQ:(g + 1) * GQ, cs], in_=og[:, cs])
                po_cm[g].__exit__(None, None, None)

            # queries 0 and 2 bootstrap via PE broadcast (no DRAM round-trip)
            bc_cm = tc.tile_pool(name="ps_bc", bufs=1, space="PSUM")
            bc_pool = bc_cm.__enter__()
            for q, ceng in ((0, nc.scalar), (2, nc.vector)):
                bc = bc_pool.tile([128, N], FP32, tag="bc", name=f"bc{q}")
                ub = bcast.tile([128, N], FP32, tag="ub", name=f"ub{q}")
                for c in range(2):
                    cs = slice(c * 512, (c + 1) * 512)
                    urow = u_sb[0:1, cs] if q == 0 else u2row[:, cs]
                    nc.tensor.matmul(
                        bc[:, cs], lhsT=ones1x128, rhs=urow,
                        start=True, stop=True)
                    if ceng is nc.scalar:
                        nc.scalar.activation(out=ub[:, cs], in_=bc[:, cs], func=AFT.Copy)
                    else:
                        ceng.tensor_copy(ub[:, cs], bc[:, cs])
                ub_tile[q] = ub
            bc_cm.__exit__(None, None, None)
            issue_ub(1)
            issue_ub(3)
            for pi, pr in enumerate(PAIRS):
                if pi + 2 < len(PAIRS):
                    for q in PAIRS[pi + 2]:
                        issue_ub(q)
                for q in pr:
                    emit_query(q)
                if pi == FIN_A:
                    finalize(0)
                if pi == FIN_B:
                    finalize_b(0)
            finalize(1)
            finalize_b(1)

            if debug_taps:
                nc.sync.dma_start(out=dbg_u[0:8, :], in_=u_sb[0:8, :])
                nc.sync.dma_start(out=dbg_u[8:16, :], in_=u_sb[32:40, :])
                for g in range(GROUPS):
                    for c in range(2):
                        cs = slice(c * 512, (c + 1) * 512)
                        nc.sync.dma_start(
                            out=dbg_stk[g * 104:(g + 1) * 104, cs], in_=stack[g][c])

    nc.compile()
    return nc


_CACHE = {}


def _get_nc():
    if "nc" not in _CACHE:
        _CACHE["nc"] = _build_nc()
    return _CACHE["nc"]


def _in_maps(query, neighbors):
    query = np.ascontiguousarray(query, dtype=np.float32)
    neighbors = np.ascontiguousarray(neighbors, dtype=np.float32)
    return [
        {"query": query[c * BL:(c + 1) * BL], "neighbors": neighbors}
        for c in range(NCORES)
    ]


def _run(query, neighbors, **kw):
    nc = _get_nc()
    res = run_bass_kernel_spmd(nc, _in_maps(query, neighbors), list(range(NCORES)), **kw)
    out = np.concatenate([res.results[c]["out"] for c in range(NCORES)], axis=0)
    return out, res


def kernel(query, neighbors):
    out, _ = _run(query, neighbors)
    return out


def run_profiled(query, neighbors, **kw):
    out, res = _run(query, neighbors, trace=True, **kw)
    return out, res


# revision 5
# speedup vs baseline: 1.2987x; 1.0017x over previous
"""Trainium2 Bass kernel for deterministic NeuralSort soft-kNN (DKNN), v2.

Math (per query b over N neighbors):
    s_j   = -||q_b - x_j||^2
    A_j   = sum_i |s_j - s_i|
    P[r,j]= softmax_j(scaling[r] * s_j - A_j),  r = 0..K-1, scaling[r] = N+1-2(r+1)
    out_j = sum_r P[r,j]

Reductions:
  * s_j = u_j - ||q_b||^2 with u_j = 2 q_b.x_j - ||x_j||^2; the ||q||^2 term
    cancels (constant in j for A; constant per softmax row otherwise).
  * A_j = u_j*(2 r_j - N) - 2 L_j + T with r_j = #{i: u_i < u_j} and
    L_j = sum_{u_i < u_j} u_i.  T is constant per query => cancels in the
    row softmax and is dropped.
  * r_j / L_j come from a 0/1 compare matrix reduced on TensorE: the compare
    tiles are fp8 (exact 0/1 or +-1), u_i is decomposed into 5 exact fp8e4m3
    limbs (scales 4*16^-m), and the reduce runs as fp8 DoubleRow matmuls
    (2 k-tiles per pass, 0.5 cyc/col) accumulating counts + limb sums in
    fp32 PSUM.
  * ScalarE generates sign(u_i - u_j) tiles (func=Sign, scale=-1), Vector/
    GpSimd generate is_gt tiles; the convention difference is absorbed in
    host-constant coefficients of the final fp32 matmul.
  * P_max[80, N] for a group of 8 queries is ONE fp32 matmul per half:
    lhsT [56, 80] host consts over a stacked rhs [c2(8); Lsc(40); u(8)]
    where c2 = u .* counts, Lsc = s_m * limb-sum rows.

Sharding: data-parallel over B=128 queries across 8 cores (16 each, two
groups of 8 for softmax finalization); neighbors replicated.
"""

import numpy as np
import ml_dtypes

import concourse.bass as bass
import concourse.bacc as bacc
import concourse.tile as tile
from concourse import mybir
from concourse.masks import make_identity
from concourse.bass_utils import run_bass_kernel_spmd

AFT = mybir.ActivationFunctionType
ALU = mybir.AluOpType
FP32 = mybir.dt.float32
BF16 = mybir.dt.bfloat16
FP8 = mybir.dt.float8e4
PM = mybir.MatmulPerfMode

B, N, D, TOPK = 128, 1024, 128, 10
NCORES = 8
BL = B // NCORES          # 16 queries per core
NBLK = N // 128           # 8 row-blocks of the pairwise matrix
GROUPS = 2
GQ = BL // GROUPS         # 8 queries per softmax group
NL = 5                    # fp8 limbs per u value
LANES = 104               # pa rows: counts 0:8, (pad), limbs 64:104
LIMB0 = 64                # first limb lane
LSC = [4.0 * 2.0 ** (-4 * m) for m in range(NL)]

# Per-(query, t-block) engine for the compare-tile generation.
# A = ScalarE (Sign, +-1), D = VectorE (is_gt 0/1), P = GpSimd (is_gt 0/1).
ACT_Q = (0, 1, 8, 9, 15)
SIGN_BL = {0: (0, 1), 1: (0, 1, 7)}   # sign-convention bl per group
_NDH = 3  # number of DVE-heavy (5/3) queries
_DH_POOL = [2, 10, 3, 11, 5, 13, 6, 14]
ENG_T = {}
for _q in range(BL):
    if _q in ACT_Q:
        ENG_T[_q] = "AAAAAAAA"
    elif _q in _DH_POOL[:_NDH]:
        ENG_T[_q] = "DDDDDPPP"
    else:
        ENG_T[_q] = "DDDDPPPP"

PAIRS = ((0, 2), (1, 3), (4, 5), (6, 7), (8, 10), (9, 11), (12, 13), (14, 15))
FIN_A, FIN_B = 4, 6
# DMA queue for each query's u broadcast: S = sync, P = gpsimd
UBQ = {q: "S" for q in range(BL)}


def _host_consts():
    scaling = (N + 1 - 2.0 * (np.arange(TOPK) + 1)).astype(np.float64)
    # FdL [56, 80]: stack rows 0:8 u, 8:16 c2 (u .* counts), 16:56 Lsc (limb
    # sums, lane m,b at 16+8m+b).  Column (bl*10+r) computes
    # scaling[r]*u_j - A_j (+ const, dropped).
    # gt-convention (counts r_j, limb sums L):    -A = -2*(u.*r) + N*u + 2*Lsum
    # sign-convention (C' = -sum sgn, S' = -S):   -A = +(u.*C') - Lsum'
    fdl = np.zeros((GROUPS, 104, 80), np.float64)
    for g in range(GROUPS):
        for bl in range(GQ):
            is_sign = bl in SIGN_BL[g]
            for r in range(TOPK):
                col = bl * TOPK + r
                fdl[g, bl, col] = scaling[r] + (0.0 if is_sign else float(N))
                fdl[g, 32 + bl, col] = 1.0 if is_sign else -2.0
                for m in range(NL):
                    fdl[g, 64 + 8 * m + bl, col] = -1.0 if is_sign else 2.0
    # lscalev [40, 1]: Lsc = s_m * pa_limb_row
    lsc = np.zeros((40, 1), np.float32)
    for m in range(NL):
        for bl in range(GQ):
            lsc[8 * m + bl, 0] = LSC[m]
    G = np.zeros((GQ * TOPK, GQ), np.float32)
    for bl in range(GQ):
        G[bl * TOPK:(bl + 1) * TOPK, bl] = 1.0
    zeros_w3 = np.zeros((1, 4 * 2 * LANES * BL), ml_dtypes.float8_e4m3)
    return fdl.astype(np.float32), lsc, G, zeros_w3


def _build_nc(debug_taps=False):
    nc = bacc.Bacc(None, target_bir_lowering=False)

    q_in = nc.dram_tensor("query", [BL, D], FP32, kind="ExternalInput")
    x_in = nc.dram_tensor("neighbors", [N, D], FP32, kind="ExternalInput")
    out_t = nc.dram_tensor("out", [BL, N], FP32, kind="ExternalOutput")
    if debug_taps:
        dbg_u = nc.dram_tensor("dbg_u", [BL, N], FP32, kind="ExternalOutput")
        dbg_pa = nc.dram_tensor("dbg_pa", [GROUPS * LANES, N], FP32, kind="ExternalOutput")
        dbg_stk = nc.dram_tensor("dbg_stk", [GROUPS * 104, N], FP32, kind="ExternalOutput")
        dbg_pm = nc.dram_tensor("dbg_pm", [GROUPS * 80, N], FP32, kind="ExternalOutput")

    FdL, lscv, G, zw3 = _host_consts()
    fdl_in = [nc.inline_tensor(np.ascontiguousarray(FdL[g]), f"fdl{g}")
              for g in range(GROUPS)]
    lsc_in = nc.inline_tensor(lscv, "lscv")
    g_in = nc.inline_tensor(G, "lhs_g")
    zw3_in = nc.inline_tensor(zw3, "zw3")

    with tile.TileContext(nc) as tc:
        with (
            tc.tile_pool(name="consts", bufs=1) as consts,
            tc.tile_pool(name="xp", bufs=1) as xp,
            tc.tile_pool(name="bcast", bufs=12) as bcast,
            tc.tile_pool(name="scrS", bufs=24) as scrS,
            tc.tile_pool(name="small", bufs=8) as small,
            tc.tile_pool(name="dramp", bufs=1, space="DRAM") as dramp,
        ):
            ident = consts.tile([128, 128], FP32)
            make_identity(nc, ident)
            ones128 = consts.tile([128, 1], FP32)
            nc.vector.memset(ones128, 1.0)
            ones1xb = consts.tile([1, BL], FP32)
            nc.vector.memset(ones1xb, 1.0)
            ones1x128 = consts.tile([1, 128], FP32)
            nc.vector.memset(ones1x128, 1.0)
            ones1x72 = consts.tile([1, 72], FP32)
            nc.vector.memset(ones1x72, 0.0)
            nc.vector.memset(ones1x72[:, 0:8], 1.0)
            nc.vector.memset(ones1x72[:, 32:40], 1.0)
            nc.vector.memset(ones1x72[:, 64:65], 1.0)

            # ---- inputs in; W3 zero skeleton on Pool queue ----
            x_sb = xp.tile([128, NBLK, D], FP32)
            xv = x_in[:].rearrange("(t p) d -> p t d", p=128)
            half = NBLK // 2
            nc.sync.dma_start(out=x_sb[:, :half, :], in_=xv[:, :half, :])
            q_sb = small.tile([BL, D], FP32)
            nc.sync.dma_start(out=q_sb, in_=q_in[:])
            nc.sync.dma_start(out=x_sb[:, half:, :], in_=xv[:, half:, :])

            # W3 [128, tpair(4), tp(2), lane(48), q(16)] fp8
            W3 = consts.tile([128, 4, 2, LANES, BL], FP8)
            zsrc = zw3_in[:]
            zap = bass.AP(tensor=zsrc.tensor, offset=zsrc.offset,
                          ap=[[0, 128], [1, 4 * 2 * LANES * BL]])
            nc.sync.dma_start(
                out=W3[:].rearrange("p a b l q -> p (a b l q)"), in_=zap)

            fdl_sb = [consts.tile([104, 80], FP32, name=f"fdl_sb{g}")
                      for g in range(GROUPS)]
            for g in range(GROUPS):
                nc.sync.dma_start(out=fdl_sb[g], in_=fdl_in[g][:])
            lsc_sb = consts.tile([40, 1], FP32)
            nc.sync.dma_start(out=lsc_sb, in_=lsc_in[:])
            lscB = consts.tile([8, 1], FP32)
            nc.sync.dma_start(out=lscB, in_=lsc_in[32:40])
            g_sb = consts.tile([GQ * TOPK, GQ], FP32)
            nc.sync.dma_start(out=g_sb, in_=g_in[:])

            # ---- transposes: xT[d, j], q2T = (2Q)^T ----
            xT = xp.tile([128, N], FP32)
            q2T = consts.tile([128, BL], FP32)
            q2Tw = consts.tile([128, 72], FP32)
            with tc.tile_pool(name="ps_tr", bufs=2, space="PSUM") as ps_tr:
                for t in range(half):
                    ptr = ps_tr.tile([128, 128], FP32)
                    nc.tensor.transpose(ptr, x_sb[:, t, :], ident)
                    nc.vector.tensor_copy(xT[:, t * 128:(t + 1) * 128], ptr)
                pqt = ps_tr.tile([128, BL], FP32, tag="pqt")
                nc.tensor.transpose(pqt, q_sb, ident[:BL, :BL])
                nc.scalar.activation(out=q2T, in_=pqt, func=AFT.Copy, scale=2.0)
                nc.vector.memset(q2Tw[:, 8:32], 0.0)
                nc.vector.memset(q2Tw[:, 40:64], 0.0)
                nc.vector.memset(q2Tw[:, 65:72], 0.0)
                nc.scalar.activation(out=q2Tw[:, 0:8], in_=pqt[:, 0:8],
                                     func=AFT.Copy, scale=2.0)
                nc.scalar.activation(out=q2Tw[:, 32:40], in_=pqt[:, 8:16],
                                     func=AFT.Copy, scale=2.0)
                nc.scalar.activation(out=q2Tw[:, 64:65], in_=pqt[:, 2:3],
                                     func=AFT.Copy, scale=2.0)
                for t in range(half, NBLK):
                    ptr = ps_tr.tile([128, 128], FP32)
                    nc.tensor.transpose(ptr, x_sb[:, t, :], ident)
                    nc.vector.tensor_copy(xT[:, t * 128:(t + 1) * 128], ptr)

            # ---- row norms: negx2[1, j] = -||x_j||^2 ----
            sq = xp.tile([128, N], FP32)
            negx2 = consts.tile([1, N], FP32)
            with tc.tile_pool(name="ps_x2", bufs=1, space="PSUM") as ps_x2:
                px2 = ps_x2.tile([1, N], FP32)
                for c in range(2):
                    cs = slice(c * 512, (c + 1) * 512)
                    nc.scalar.activation(out=sq[:, cs], in_=xT[:, cs], func=AFT.Square)
                    nc.tensor.matmul(
                        px2[:, cs], lhsT=ones128, rhs=sq[:, cs], start=True, stop=True)
                    nc.scalar.activation(
                        out=negx2[:, cs], in_=px2[:, cs], func=AFT.Copy, scale=-1.0)

            # ---- u row form (u_sb -> u_dram) and uT column form ----
            ub_tile = {}
            u_sb = consts.tile([40, N], FP32)   # rows 0:8 = q0..7, 32:40 = q8..15
            uT = consts.tile([128, NBLK, BL], FP32)   # uT[p, t, b] = u[b, t*128+p]
            u_dram = dramp.tile([40, N], FP32)
            with tc.tile_pool(name="ps_u", bufs=1, space="PSUM") as ps_u, \
                 tc.tile_pool(name="ps_qt", bufs=2, space="PSUM") as ps_qt:
                pu = ps_u.tile([72, N], FP32)
                for c in range(2):
                    cs = slice(c * 512, (c + 1) * 512)
                    nc.tensor.matmul(
                        pu[:, cs], lhsT=q2Tw, rhs=xT[:, cs], start=True, stop=False)
                    nc.tensor.matmul(
                        pu[:, cs], lhsT=ones1x72, rhs=negx2[:, cs],
                        start=False, stop=True)
                    nc.scalar.activation(out=u_sb[:, cs], in_=pu[0:40, cs], func=AFT.Copy)
                    nc.sync.dma_start(out=u_dram[:, cs], in_=u_sb[:, cs])
                u2row = consts.tile([1, N], FP32, name="u2row")
                nc.vector.tensor_copy(u2row, pu[64:65, :])
                bc_cm = tc.tile_pool(name="ps_bc", bufs=1, space="PSUM")
                bc_pool = bc_cm.__enter__()

                def bootstrap(q, ceng):
                    bc = bc_pool.tile([128, N], FP32, tag="bc", name=f"bc{q}")
                    ub = bcast.tile([128, N], FP32, tag="ub", name=f"ub{q}")
                    for c in range(2):
                        cs = slice(c * 512, (c + 1) * 512)
                        urow = u_sb[0:1, cs] if q == 0 else u2row[:, cs]
                        nc.tensor.matmul(
                            bc[:, cs], lhsT=ones1x128, rhs=urow,
                            start=True, stop=True)
                        if ceng is nc.scalar:
                            nc.scalar.activation(out=ub[:, cs], in_=bc[:, cs],
                                                 func=AFT.Copy)
                        else:
                            ceng.tensor_copy(ub[:, cs], bc[:, cs])
                    ub_tile[q] = ub

                bootstrap(0, nc.scalar)
                for t in range(NBLK):
                    if t == half:
                        bootstrap(2, nc.vector)
                    put = ps_qt.tile([128, BL], FP32, tag="put")
                    nc.tensor.matmul(
                        put, lhsT=xT[:, t * 128:(t + 1) * 128], rhs=q2T,
                        start=True, stop=False)
                    nc.tensor.matmul(
                        put, lhsT=negx2[:, t * 128:(t + 1) * 128], rhs=ones1xb,
                        start=False, stop=True)
                    nc.vector.tensor_copy(uT[:, t, :], put)
                bc_cm.__exit__(None, None, None)

            # ---- stacked rhs tiles (u rows placed early) ----
            stack = [[consts.tile([104, 512], FP32, name=f"stack{g}_{c}")
                      for c in range(2)] for g in range(GROUPS)]
            for g in range(GROUPS):
                for c in range(2):
                    cs = slice(c * 512, (c + 1) * 512)
                    nc.gpsimd.memset(stack[g][c][0:32, :], 0.0)
                    nc.gpsimd.memset(stack[g][c][32:64, :], 0.0)
                    nc.gpsimd.tensor_copy(stack[g][c][0:8, :],
                                          u_sb[g * 32:g * 32 + 8, cs])

            # ---- W3 diagonal lanes: counts + 5 fp8 limbs of uT ----
            # diag view for (lane-base lb, group g): [128, t(8), b(8)]
            def w3diag(lb, g):
                v = W3[:]
                return bass.AP(tensor=v.tensor, offset=v.offset + lb * BL + g * GQ,
                               ap=[list(v.ap[0]), [LANES * BL, NBLK], [BL + 1, GQ]])

            for g in range(GROUPS):
                nc.gpsimd.memset(w3diag(0, g), 1.0)
            for g in range(GROUPS):
                uTg = uT[:, :, g * GQ:(g + 1) * GQ]
                resid = bass.AP(tensor=uTg.tensor, offset=uTg.offset,
                                ap=[list(uTg.ap[0]), [BL, NBLK], [1, GQ]])
                for m in range(NL):
                    lane = w3diag(LIMB0 + 8 * m, g)
                    nc.vector.tensor_scalar(
                        out=lane, in0=resid, scalar1=1.0 / LSC[m], scalar2=None,
                        op0=ALU.mult)
                    if m < NL - 1:
                        back = small.tile([128, NBLK, GQ], FP32, tag="lback")
                        nc.vector.tensor_scalar(
                            out=back, in0=lane, scalar1=LSC[m], scalar2=None,
                            op0=ALU.mult)
                        nres = small.tile([128, NBLK, GQ], FP32, tag=f"lres{m % 2}")
                        nc.vector.tensor_tensor(
                            out=nres, in0=resid, in1=back, op=ALU.subtract)
                        resid = nres[:]

            # ---- pairwise phase ----
            pa_cm, pa_tile = {}, {}
            for g in (1, 0):
                pa_cm[g] = tc.tile_pool(name=f"ps_pa{g}", bufs=1, space="PSUM")
                pool = pa_cm[g].__enter__()
                pa_tile[g] = pool.tile([LANES, N], FP32, tag=f"pa{g}", name=f"pa{g}")
            first = {(g, c): True for g in range(GROUPS) for c in range(2)}
            left = {0: GQ * 4, 1: GQ * 4}

            def issue_ub(q):
                ub = bcast.tile([128, N], FP32, tag="ub", name=f"ub{q}")
                row = (q // GQ) * 32 + (q % GQ)
                eng = {"S": nc.sync, "P": nc.gpsimd, "A": nc.scalar}[UBQ[q]]
                for c in range(2):
                    cs = slice(c * 512, (c + 1) * 512)
                    base = u_dram[row:row + 1, cs]
                    src = bass.AP(tensor=base.tensor, offset=base.offset,
                                  ap=[[0, 128], [1, 512]])
                    eng.dma_start(out=ub[:, cs], in_=src)
                ub_tile[q] = ub

            def emit_query(q):
                g = q // GQ
                ub = ub_tile[q]
                for tp in range(4):
                    sd2 = scrS.tile([128, 2, N], FP8, tag="sd2", name=f"sd2_{q}_{tp}")
                    for th in range(2):
                        t = 2 * tp + th
                        e = ENG_T[q][t]
                        if e == "A":
                            nc.scalar.activation(
                                out=sd2[:, th, :], in_=ub, func=AFT.Sign,
                                bias=uT[:, t, q:q + 1], scale=-1.0)
                        else:
                            eng = nc.vector if e == "D" else nc.gpsimd
                            eng.tensor_scalar(
                                out=sd2[:, th, :], in0=ub,
                                scalar1=uT[:, t, q:q + 1], scalar2=0.0,
                                op0=ALU.subtract, op1=ALU.is_gt)
                    left[g] -= 1
                    for c in range(2):
                        cs = slice(c * 512, (c + 1) * 512)
                        nc.tensor.matmul(
                            pa_tile[g][:, cs], lhsT=W3[:, tp, :, :, q],
                            rhs=sd2[:, :, cs],
                            start=first[(g, c)], stop=left[g] == 0,
                            perf_mode=PM.DoubleRow)
                        first[(g, c)] = False

            pm_cm, pm_tile, po_cm, fin_state = {}, {}, {}, {}

            def finalize(g):
                pa = pa_tile[g]
                h0, h1 = slice(0, 512), slice(512, 1024)
                # c2 = u .* counts (rows 32:40); Lsc = s_m * limb rows (64:104)
                # halves in separate stack tiles so FdL h0 starts early.
                nc.vector.tensor_tensor(
                    out=stack[g][0][32:40, :], in0=pa[0:8, h0],
                    in1=u_sb[g * 32:g * 32 + 8, h0], op=ALU.mult)
                nc.scalar.activation(
                    out=stack[g][0][64:104, :], in_=pa[64:104, h0], func=AFT.Copy,
                    scale=lsc_sb)
                nc.vector.tensor_tensor(
                    out=stack[g][1][32:40, :], in0=pa[0:8, h1],
                    in1=u_sb[g * 32:g * 32 + 8, h1], op=ALU.mult)
                nc.scalar.activation(
                    out=stack[g][1][64:104, :], in_=pa[64:104, h1], func=AFT.Copy,
                    scale=lsc_sb)
                pa_cm[g].__exit__(None, None, None)
                pm_cm[g] = tc.tile_pool(name=f"ps_pm{g}", bufs=1, space="PSUM")
                pmp = pm_cm[g].__enter__()
                pm = pmp.tile([GQ * TOPK, N], FP32, tag=f"pm{g}", name=f"pm{g}")
                pm_tile[g] = pm
                # 128-col chunks: PE ramps on useful work; chunk 0/4 carry
                # start=True (pending-zero covers the whole 2KB region).
                for c in range(8):
                    cs = slice(c * 128, (c + 1) * 128)
                    hcs = slice((c % 4) * 128, (c % 4) * 128 + 128)
                    nc.tensor.matmul(
                        pm[:, cs], lhsT=fdl_sb[g], rhs=stack[g][c // 4][:, hcs],
                        start=(c % 4 == 0), stop=(c % 4 == 3),
                        skip_group_check=True)
                nmx0 = small.tile([GQ * TOPK, 1], FP32, tag="nmx0")
                nc.vector.tensor_reduce(
                    out=nmx0, in_=pm[:, h0], axis=mybir.AxisListType.X, op=ALU.max,
                    negate=True)
                nmx1 = small.tile([GQ * TOPK, 1], FP32, tag="nmx1")
                nc.vector.tensor_reduce(
                    out=nmx1, in_=pm[:, h1], axis=mybir.AxisListType.X, op=ALU.max,
                    negate=True)
                nmx = small.tile([GQ * TOPK, 1], FP32, tag="nmx")
                nc.vector.tensor_tensor(out=nmx, in0=nmx0, in1=nmx1, op=ALU.min)
                exps = consts.tile([GQ * TOPK, N], BF16, name=f"exps{g}")
                den = small.tile([GQ * TOPK, 1], FP32, tag="den")
                nc.scalar.activation(
                    out=exps, in_=pm, func=AFT.Exp, bias=nmx, scale=1.0,
                    accum_out=den)
                pm_cm[g].__exit__(None, None, None)
                fin_state[g] = (exps, den)

            def finalize_b(g):
                exps, den = fin_state[g]
                rden = small.tile([GQ * TOPK, 1], FP32, tag="rden")
                nc.vector.reciprocal(rden, den)
                gr = small.tile([GQ * TOPK, GQ], BF16, tag="gr")
                nc.vector.tensor_scalar(
                    out=gr, in0=g_sb, scalar1=rden, scalar2=None, op0=ALU.mult)
                po_cm[g] = tc.tile_pool(name=f"ps_po{g}", bufs=1, space="PSUM")
                pop = po_cm[g].__enter__()
                og = consts.tile([GQ, N], FP32, name=f"og{g}")
                for c in range(2):
                    cs = slice(c * 512, (c + 1) * 512)
                    po = pop.tile([GQ, 512], FP32, tag=f"po{g}_{c}", name=f"po{g}_{c}")
                    nc.tensor.matmul(
                        po, lhsT=gr, rhs=exps[:, cs], start=True, stop=True)
                    if g == 0 or c == 0:
                        nc.vector.tensor_copy(og[:, cs], po)
                    else:
                        nc.scalar.activation(out=og[:, cs], in_=po, func=AFT.Copy)
                    deng = nc.sync if (g == 0 or c == 0) else nc.gpsimd
                    deng.dma_start(
                        out=out_t[g * GQ:(g + 1) * GQ, cs], in_=og[:, cs])
                po_cm[g].__exit__(None, None, None)

            issue_ub(1)
            issue_ub(3)
            for pi, pr in enumerate(PAIRS):
                if pi + 2 < len(PAIRS):
                    for q in PAIRS[pi + 2]:
                        issue_ub(q)
                for q in pr:
                    emit_query(q)
                if pi == FIN_A:
                    finalize(0)
                if pi == FIN_B:
                    finalize_b(0)
            finalize(1)
            finalize_b(1)

            if debug_taps:
                nc.sync.dma_start(out=dbg_u[0:8, :], in_=u_sb[0:8, :])
                nc.sync.dma_start(out=dbg_u[8:16, :], in_=u_sb[32:40, :])
                for g in range(GROUPS):
                    for c in range(2):
                        cs = slice(c * 512, (c + 1) * 512)
                        nc.sync.dma_start(
                            out=dbg_stk[g * 104:(g + 1) * 104, cs], in_=stack[g][c])

    nc.compile()
    return nc


_CACHE = {}


def _get_nc():
    if "nc" not in _CACHE:
        _CACHE["nc"] = _build_nc()
    return _CACHE["nc"]


def _in_maps(query, neighbors):
    query = np.ascontiguousarray(query, dtype=np.float32)
    neighbors = np.ascontiguousarray(neighbors, dtype=np.float32)
    return [
        {"query": query[c * BL:(c + 1) * BL], "neighbors": neighbors}
        for c in range(NCORES)
    ]


def _run(query, neighbors, **kw):
    nc = _get_nc()
    res = run_bass_kernel_spmd(nc, _in_maps(query, neighbors), list(range(NCORES)), **kw)
    out = np.concatenate([res.results[c]["out"] for c in range(NCORES)], axis=0)
    return out, res


def kernel(query, neighbors):
    out, _ = _run(query, neighbors)
    return out


def run_profiled(query, neighbors, **kw):
    out, res = _run(query, neighbors, trace=True, **kw)
    return out, res


# revision 6
# speedup vs baseline: 1.3154x; 1.0129x over previous
"""Trainium2 Bass kernel for deterministic NeuralSort soft-kNN (DKNN), v2.

Math (per query b over N neighbors):
    s_j   = -||q_b - x_j||^2
    A_j   = sum_i |s_j - s_i|
    P[r,j]= softmax_j(scaling[r] * s_j - A_j),  r = 0..K-1, scaling[r] = N+1-2(r+1)
    out_j = sum_r P[r,j]

Reductions:
  * s_j = u_j - ||q_b||^2 with u_j = 2 q_b.x_j - ||x_j||^2; the ||q||^2 term
    cancels (constant in j for A; constant per softmax row otherwise).
  * A_j = u_j*(2 r_j - N) - 2 L_j + T with r_j = #{i: u_i < u_j} and
    L_j = sum_{u_i < u_j} u_i.  T is constant per query => cancels in the
    row softmax and is dropped.
  * r_j / L_j come from a 0/1 compare matrix reduced on TensorE: the compare
    tiles are fp8 (exact 0/1 or +-1), u_i is decomposed into 5 exact fp8e4m3
    limbs (scales 4*16^-m), and the reduce runs as fp8 DoubleRow matmuls
    (2 k-tiles per pass, 0.5 cyc/col) accumulating counts + limb sums in
    fp32 PSUM.
  * ScalarE generates sign(u_i - u_j) tiles (func=Sign, scale=-1), Vector/
    GpSimd generate is_gt tiles; the convention difference is absorbed in
    host-constant coefficients of the final fp32 matmul.
  * P_max[80, N] for a group of 8 queries is ONE fp32 matmul per half:
    lhsT [56, 80] host consts over a stacked rhs [c2(8); Lsc(40); u(8)]
    where c2 = u .* counts, Lsc = s_m * limb-sum rows.

Sharding: data-parallel over B=128 queries across 8 cores (16 each, two
groups of 8 for softmax finalization); neighbors replicated.
"""

import numpy as np
import ml_dtypes

import concourse.bass as bass
import concourse.bacc as bacc
import concourse.tile as tile
from concourse import mybir
from concourse.masks import make_identity
from concourse.bass_utils import run_bass_kernel_spmd

AFT = mybir.ActivationFunctionType
ALU = mybir.AluOpType
FP32 = mybir.dt.float32
BF16 = mybir.dt.bfloat16
FP8 = mybir.dt.float8e4
PM = mybir.MatmulPerfMode

B, N, D, TOPK = 128, 1024, 128, 10
NCORES = 8
BL = B // NCORES          # 16 queries per core
NBLK = N // 128           # 8 row-blocks of the pairwise matrix
GROUPS = 2
GQ = BL // GROUPS         # 8 queries per softmax group
NL = 5                    # fp8 limbs per u value
LANES = 104               # pa rows: counts 0:8, (pad), limbs 64:104
LIMB0 = 64                # first limb lane
LSC = [4.0 * 2.0 ** (-4 * m) for m in range(NL)]

# Per-(query, t-block) engine for the compare-tile generation.
# A = ScalarE (Sign, +-1), D = VectorE (is_gt 0/1), P = GpSimd (is_gt 0/1).
ACT_Q = (0, 1, 8, 9, 15)
SIGN_BL = {0: (0, 1), 1: (0, 1, 7)}   # sign-convention bl per group
_NDH = 3  # number of DVE-heavy (5/3) queries
_DH_POOL = [2, 10, 3, 11, 5, 13, 6, 14]
ENG_T = {}
for _q in range(BL):
    if _q in ACT_Q:
        ENG_T[_q] = "AAAAAAAA"
    elif _q in _DH_POOL[:_NDH]:
        ENG_T[_q] = "DDDDDPPP"
    else:
        ENG_T[_q] = "DDDDPPPP"

PAIRS = ((0, 2), (1, 3), (4, 5), (6, 7), (8, 10), (9, 11), (12, 13), (14, 15))
FIN_A, FIN_B = 4, 6
# DMA queue for each query's u broadcast: S = sync, P = gpsimd
UBQ = {q: "S" for q in range(BL)}


def _host_consts():
    scaling = (N + 1 - 2.0 * (np.arange(TOPK) + 1)).astype(np.float64)
    # FdL [56, 80]: stack rows 0:8 u, 8:16 c2 (u .* counts), 16:56 Lsc (limb
    # sums, lane m,b at 16+8m+b).  Column (bl*10+r) computes
    # scaling[r]*u_j - A_j (+ const, dropped).
    # gt-convention (counts r_j, limb sums L):    -A = -2*(u.*r) + N*u + 2*Lsum
    # sign-convention (C' = -sum sgn, S' = -S):   -A = +(u.*C') - Lsum'
    fdl = np.zeros((GROUPS, 104, 80), np.float64)
    for g in range(GROUPS):
        for bl in range(GQ):
            is_sign = bl in SIGN_BL[g]
            for r in range(TOPK):
                col = bl * TOPK + r
                fdl[g, bl, col] = scaling[r] + (0.0 if is_sign else float(N))
                fdl[g, 32 + bl, col] = 1.0 if is_sign else -2.0
                for m in range(NL):
                    fdl[g, 64 + 8 * m + bl, col] = -1.0 if is_sign else 2.0
    # lscalev [40, 1]: Lsc = s_m * pa_limb_row
    lsc = np.zeros((40, 1), np.float32)
    for m in range(NL):
        for bl in range(GQ):
            lsc[8 * m + bl, 0] = LSC[m]
    G = np.zeros((GQ * TOPK, GQ), np.float32)
    for bl in range(GQ):
        G[bl * TOPK:(bl + 1) * TOPK, bl] = 1.0
    zeros_w3 = np.zeros((1, 4 * 2 * LANES * BL), ml_dtypes.float8_e4m3)
    return fdl.astype(np.float32), lsc, G, zeros_w3


def _build_nc(debug_taps=False):
    nc = bacc.Bacc(None, target_bir_lowering=False)

    q_in = nc.dram_tensor("query", [BL, D], FP32, kind="ExternalInput")
    x_in = nc.dram_tensor("neighbors", [N, D], FP32, kind="ExternalInput")
    out_t = nc.dram_tensor("out", [BL, N], FP32, kind="ExternalOutput")
    if debug_taps:
        dbg_u = nc.dram_tensor("dbg_u", [BL, N], FP32, kind="ExternalOutput")
        dbg_pa = nc.dram_tensor("dbg_pa", [GROUPS * LANES, N], FP32, kind="ExternalOutput")
        dbg_stk = nc.dram_tensor("dbg_stk", [GROUPS * 104, N], FP32, kind="ExternalOutput")
        dbg_pm = nc.dram_tensor("dbg_pm", [GROUPS * 80, N], FP32, kind="ExternalOutput")

    FdL, lscv, G, zw3 = _host_consts()
    fdl_in = [nc.inline_tensor(np.ascontiguousarray(FdL[g]), f"fdl{g}")
              for g in range(GROUPS)]
    lsc_in = nc.inline_tensor(lscv, "lscv")
    g_in = nc.inline_tensor(G, "lhs_g")
    zw3_in = nc.inline_tensor(zw3, "zw3")

    with tile.TileContext(nc) as tc:
        with (
            tc.tile_pool(name="consts", bufs=1) as consts,
            tc.tile_pool(name="xp", bufs=1) as xp,
            tc.tile_pool(name="bcast", bufs=12) as bcast,
            tc.tile_pool(name="scrS", bufs=24) as scrS,
            tc.tile_pool(name="small", bufs=8) as small,
            tc.tile_pool(name="dramp", bufs=1, space="DRAM") as dramp,
        ):
            ident = consts.tile([128, 128], FP32)
            make_identity(nc, ident)
            ones128 = consts.tile([128, 1], FP32)
            nc.vector.memset(ones128, 1.0)
            ones1xb = consts.tile([1, BL], FP32)
            nc.vector.memset(ones1xb, 1.0)
            ones1x128 = consts.tile([1, 128], FP32)
            nc.vector.memset(ones1x128, 1.0)
            ones1x72 = consts.tile([1, 72], FP32)
            nc.vector.memset(ones1x72, 0.0)
            nc.vector.memset(ones1x72[:, 0:8], 1.0)
            nc.vector.memset(ones1x72[:, 32:40], 1.0)
            nc.vector.memset(ones1x72[:, 64:65], 1.0)

            # ---- inputs in; W3 zero skeleton on Pool queue ----
            x_sb = xp.tile([128, NBLK, D], FP32)
            xv = x_in[:].rearrange("(t p) d -> p t d", p=128)
            half = NBLK // 2
            nc.sync.dma_start(out=x_sb[:, :half, :], in_=xv[:, :half, :])
            q_sb = small.tile([BL, D], FP32)
            nc.sync.dma_start(out=q_sb, in_=q_in[:])
            nc.sync.dma_start(out=x_sb[:, half:, :], in_=xv[:, half:, :])

            # W3 [128, tpair(4), tp(2), lane(48), q(16)] fp8
            W3 = consts.tile([128, 4, 2, LANES, BL], FP8)
            zsrc = zw3_in[:]
            zap = bass.AP(tensor=zsrc.tensor, offset=zsrc.offset,
                          ap=[[0, 128], [1, 4 * 2 * LANES * BL]])
            nc.sync.dma_start(
                out=W3[:].rearrange("p a b l q -> p (a b l q)"), in_=zap)

            fdl_sb = [consts.tile([104, 80], FP32, name=f"fdl_sb{g}")
                      for g in range(GROUPS)]
            for g in range(GROUPS):
                nc.sync.dma_start(out=fdl_sb[g], in_=fdl_in[g][:])
            lsc_sb = consts.tile([40, 1], FP32)
            nc.sync.dma_start(out=lsc_sb, in_=lsc_in[:])
            lscB = consts.tile([8, 1], FP32)
            nc.sync.dma_start(out=lscB, in_=lsc_in[32:40])
            g_sb = consts.tile([GQ * TOPK, GQ], FP32)
            nc.sync.dma_start(out=g_sb, in_=g_in[:])

            # ---- transposes: xT[d, j], q2T = (2Q)^T ----
            xT = xp.tile([128, N], FP32)
            q2T = consts.tile([128, BL], FP32)
            q2Tw = consts.tile([128, 72], FP32)
            with tc.tile_pool(name="ps_tr", bufs=2, space="PSUM") as ps_tr:
                for t in range(half):
                    ptr = ps_tr.tile([128, 128], FP32)
                    nc.tensor.transpose(ptr, x_sb[:, t, :], ident)
                    nc.vector.tensor_copy(xT[:, t * 128:(t + 1) * 128], ptr)
                pqt = ps_tr.tile([128, BL], FP32, tag="pqt")
                nc.tensor.transpose(pqt, q_sb, ident[:BL, :BL])
                nc.scalar.activation(out=q2T, in_=pqt, func=AFT.Copy, scale=2.0)
                nc.vector.memset(q2Tw[:, 8:32], 0.0)
                nc.vector.memset(q2Tw[:, 40:64], 0.0)
                nc.vector.memset(q2Tw[:, 65:72], 0.0)
                nc.vector.tensor_scalar(out=q2Tw[:, 0:8], in0=pqt[:, 0:8],
                                        scalar1=2.0, scalar2=None, op0=ALU.mult)
                nc.vector.tensor_scalar(out=q2Tw[:, 32:40], in0=pqt[:, 8:16],
                                        scalar1=2.0, scalar2=None, op0=ALU.mult)
                nc.vector.tensor_scalar(out=q2Tw[:, 64:65], in0=pqt[:, 2:3],
                                        scalar1=2.0, scalar2=None, op0=ALU.mult)
                for t in range(half, NBLK):
                    ptr = ps_tr.tile([128, 128], FP32)
                    nc.tensor.transpose(ptr, x_sb[:, t, :], ident)
                    nc.vector.tensor_copy(xT[:, t * 128:(t + 1) * 128], ptr)

            # ---- row norms: negx2[1, j] = -||x_j||^2 ----
            sq = xp.tile([128, N], FP32)
            negx2 = consts.tile([1, N], FP32)
            with tc.tile_pool(name="ps_x2", bufs=1, space="PSUM") as ps_x2:
                px2 = ps_x2.tile([1, N], FP32)
                for c in range(2):
                    cs = slice(c * 512, (c + 1) * 512)
                    nc.scalar.activation(out=sq[:, cs], in_=xT[:, cs], func=AFT.Square)
                    nc.tensor.matmul(
                        px2[:, cs], lhsT=ones128, rhs=sq[:, cs], start=True, stop=True)
                    nc.scalar.activation(
                        out=negx2[:, cs], in_=px2[:, cs], func=AFT.Copy, scale=-1.0)

            # ---- u row form (u_sb -> u_dram) and uT column form ----
            ub_tile = {}
            u_sb = consts.tile([40, N], FP32)   # rows 0:8 = q0..7, 32:40 = q8..15
            uT = consts.tile([128, NBLK, BL], FP32)   # uT[p, t, b] = u[b, t*128+p]
            u_dram = dramp.tile([40, N], FP32)
            with tc.tile_pool(name="ps_u", bufs=1, space="PSUM") as ps_u, \
                 tc.tile_pool(name="ps_qt", bufs=2, space="PSUM") as ps_qt:
                pu = ps_u.tile([72, N], FP32)
                for c in range(2):
                    cs = slice(c * 512, (c + 1) * 512)
                    nc.tensor.matmul(
                        pu[:, cs], lhsT=q2Tw, rhs=xT[:, cs], start=True, stop=False)
                    nc.tensor.matmul(
                        pu[:, cs], lhsT=ones1x72, rhs=negx2[:, cs],
                        start=False, stop=True)
                    nc.scalar.activation(out=u_sb[:, cs], in_=pu[0:40, cs], func=AFT.Copy)
                    nc.sync.dma_start(out=u_dram[:, cs], in_=u_sb[:, cs])
                u2row = consts.tile([1, N], FP32, name="u2row")
                nc.vector.tensor_copy(u2row, pu[64:65, :])
                bc_cm = tc.tile_pool(name="ps_bc", bufs=1, space="PSUM")
                bc_pool = bc_cm.__enter__()

                def bootstrap(q, ceng):
                    bc = bc_pool.tile([128, N], FP32, tag="bc", name=f"bc{q}")
                    ub = bcast.tile([128, N], FP32, tag="ub", name=f"ub{q}")
                    for c in range(2):
                        cs = slice(c * 512, (c + 1) * 512)
                        urow = u_sb[0:1, cs] if q == 0 else u2row[:, cs]
                        nc.tensor.matmul(
                            bc[:, cs], lhsT=ones1x128, rhs=urow,
                            start=True, stop=True)
                        if ceng is nc.scalar:
                            nc.scalar.activation(out=ub[:, cs], in_=bc[:, cs],
                                                 func=AFT.Copy)
                        else:
                            ceng.tensor_copy(ub[:, cs], bc[:, cs])
                    ub_tile[q] = ub

                bootstrap(0, nc.scalar)
                for t in range(NBLK):
                    if t == half:
                        bootstrap(2, nc.vector)
                    put = ps_qt.tile([128, BL], FP32, tag="put")
                    nc.tensor.matmul(
                        put, lhsT=xT[:, t * 128:(t + 1) * 128], rhs=q2T,
                        start=True, stop=False)
                    nc.tensor.matmul(
                        put, lhsT=negx2[:, t * 128:(t + 1) * 128], rhs=ones1xb,
                        start=False, stop=True)
                    nc.vector.tensor_copy(uT[:, t, :], put)
                bc_cm.__exit__(None, None, None)

            # ---- stacked rhs tiles (u rows placed early) ----
            stack = [[consts.tile([104, 512], FP32, name=f"stack{g}_{c}")
                      for c in range(2)] for g in range(GROUPS)]
            for g in range(GROUPS):
                for c in range(2):
                    cs = slice(c * 512, (c + 1) * 512)
                    nc.gpsimd.memset(stack[g][c][0:32, :], 0.0)
                    nc.gpsimd.memset(stack[g][c][32:64, :], 0.0)
                    nc.gpsimd.tensor_copy(stack[g][c][0:8, :],
                                          u_sb[g * 32:g * 32 + 8, cs])

            # ---- W3 diagonal lanes: counts + 5 fp8 limbs of uT ----
            # diag view for (lane-base lb, group g): [128, t(8), b(8)]
            def w3diag(lb, g):
                v = W3[:]
                return bass.AP(tensor=v.tensor, offset=v.offset + lb * BL + g * GQ,
                               ap=[list(v.ap[0]), [LANES * BL, NBLK], [BL + 1, GQ]])

            for g in range(GROUPS):
                nc.gpsimd.memset(w3diag(0, g), 1.0)
            for g in range(GROUPS):
                uTg = uT[:, :, g * GQ:(g + 1) * GQ]
                resid = bass.AP(tensor=uTg.tensor, offset=uTg.offset,
                                ap=[list(uTg.ap[0]), [BL, NBLK], [1, GQ]])
                for m in range(NL):
                    lane = w3diag(LIMB0 + 8 * m, g)
                    nc.vector.tensor_scalar(
                        out=lane, in0=resid, scalar1=1.0 / LSC[m], scalar2=None,
                        op0=ALU.mult)
                    if m < NL - 1:
                        back = small.tile([128, NBLK, GQ], FP32, tag="lback")
                        nc.vector.tensor_scalar(
                            out=back, in0=lane, scalar1=LSC[m], scalar2=None,
                            op0=ALU.mult)
                        nres = small.tile([128, NBLK, GQ], FP32, tag=f"lres{m % 2}")
                        nc.vector.tensor_tensor(
                            out=nres, in0=resid, in1=back, op=ALU.subtract)
                        resid = nres[:]

            # ---- pairwise phase ----
            pa_cm, pa_tile = {}, {}
            for g in (1, 0):
                pa_cm[g] = tc.tile_pool(name=f"ps_pa{g}", bufs=1, space="PSUM")
                pool = pa_cm[g].__enter__()
                pa_tile[g] = pool.tile([LANES, N], FP32, tag=f"pa{g}", name=f"pa{g}")
            first = {(g, c): True for g in range(GROUPS) for c in range(2)}
            left = {0: GQ * 4, 1: GQ * 4}

            def issue_ub(q):
                ub = bcast.tile([128, N], FP32, tag="ub", name=f"ub{q}")
                row = (q // GQ) * 32 + (q % GQ)
                eng = {"S": nc.sync, "P": nc.gpsimd, "A": nc.scalar}[UBQ[q]]
                for c in range(2):
                    cs = slice(c * 512, (c + 1) * 512)
                    base = u_dram[row:row + 1, cs]
                    src = bass.AP(tensor=base.tensor, offset=base.offset,
                                  ap=[[0, 128], [1, 512]])
                    eng.dma_start(out=ub[:, cs], in_=src)
                ub_tile[q] = ub

            def emit_query(q):
                g = q // GQ
                ub = ub_tile[q]
                for tp in range(4):
                    sd2 = scrS.tile([128, 2, N], FP8, tag="sd2", name=f"sd2_{q}_{tp}")
                    for th in range(2):
                        t = 2 * tp + th
                        e = ENG_T[q][t]
                        if e == "A":
                            nc.scalar.activation(
                                out=sd2[:, th, :], in_=ub, func=AFT.Sign,
                                bias=uT[:, t, q:q + 1], scale=-1.0)
                        else:
                            eng = nc.vector if e == "D" else nc.gpsimd
                            eng.tensor_scalar(
                                out=sd2[:, th, :], in0=ub,
                                scalar1=uT[:, t, q:q + 1], scalar2=0.0,
                                op0=ALU.subtract, op1=ALU.is_gt)
                    left[g] -= 1
                    for c in range(2):
                        cs = slice(c * 512, (c + 1) * 512)
                        nc.tensor.matmul(
                            pa_tile[g][:, cs], lhsT=W3[:, tp, :, :, q],
                            rhs=sd2[:, :, cs],
                            start=first[(g, c)], stop=left[g] == 0,
                            perf_mode=PM.DoubleRow)
                        first[(g, c)] = False

            pm_cm, pm_tile, po_cm, fin_state = {}, {}, {}, {}

            def finalize(g):
                pa = pa_tile[g]
                h0, h1 = slice(0, 512), slice(512, 1024)
                # c2 = u .* counts (rows 32:40); Lsc = s_m * limb rows (64:104)
                # halves in separate stack tiles so FdL h0 starts early.
                nc.vector.tensor_tensor(
                    out=stack[g][0][32:40, :], in0=pa[0:8, h0],
                    in1=u_sb[g * 32:g * 32 + 8, h0], op=ALU.mult)
                nc.scalar.activation(
                    out=stack[g][0][64:104, :], in_=pa[64:104, h0], func=AFT.Copy,
                    scale=lsc_sb)
                nc.vector.tensor_tensor(
                    out=stack[g][1][32:40, :], in0=pa[0:8, h1],
                    in1=u_sb[g * 32:g * 32 + 8, h1], op=ALU.mult)
                nc.scalar.activation(
                    out=stack[g][1][64:104, :], in_=pa[64:104, h1], func=AFT.Copy,
                    scale=lsc_sb)
                pa_cm[g].__exit__(None, None, None)
                pm_cm[g] = tc.tile_pool(name=f"ps_pm{g}", bufs=1, space="PSUM")
                pmp = pm_cm[g].__enter__()
                if g == 0:
                    pmh = [None, None]
                    pm = pmp.tile([GQ * TOPK, N], FP32, tag=f"pm{g}", name=f"pm{g}")
                else:
                    pmh = [pmp.tile([GQ * TOPK, 512], FP32, tag=f"pm{g}_{c}",
                                    name=f"pm{g}_{c}") for c in range(2)]
                # 128-col chunks: PE ramps on useful work; chunk 0/4 carry
                # start=True (pending-zero covers the whole 2KB region).
                for c in range(8):
                    hcs = slice((c % 4) * 128, (c % 4) * 128 + 128)
                    dst = pm[:, c * 128:(c + 1) * 128] if g == 0 else pmh[c // 4][:, hcs]
                    nc.tensor.matmul(
                        dst, lhsT=fdl_sb[g], rhs=stack[g][c // 4][:, hcs],
                        start=(c % 4 == 0), stop=(c % 4 == 3),
                        skip_group_check=True)
                vh0 = pm[:, h0] if g == 0 else pmh[0][:]
                vh1 = pm[:, h1] if g == 0 else pmh[1][:]
                nmx0 = small.tile([GQ * TOPK, 1], FP32, tag="nmx0")
                nc.vector.tensor_reduce(
                    out=nmx0, in_=vh0, axis=mybir.AxisListType.X, op=ALU.max,
                    negate=True)
                nmx1 = small.tile([GQ * TOPK, 1], FP32, tag="nmx1")
                nc.vector.tensor_reduce(
                    out=nmx1, in_=vh1, axis=mybir.AxisListType.X, op=ALU.max,
                    negate=True)
                nmx = small.tile([GQ * TOPK, 1], FP32, tag="nmx")
                nc.vector.tensor_tensor(out=nmx, in0=nmx0, in1=nmx1, op=ALU.min)
                exps = consts.tile([GQ * TOPK, N], BF16, name=f"exps{g}")
                den = small.tile([GQ * TOPK, 1], FP32, tag="den")
                if g == 0:
                    nc.scalar.activation(
                        out=exps, in_=pm, func=AFT.Exp, bias=nmx, scale=1.0,
                        accum_out=den)
                else:
                    den_h = small.tile([GQ * TOPK, 2], FP32, tag="denh")
                    for c in range(2):
                        cs = slice(c * 512, (c + 1) * 512)
                        nc.scalar.activation(
                            out=exps[:, cs], in_=pmh[c], func=AFT.Exp, bias=nmx,
                            scale=1.0, accum_out=den_h[:, c:c + 1])
                    nc.vector.tensor_tensor(out=den, in0=den_h[:, 0:1],
                                            in1=den_h[:, 1:2], op=ALU.add)
                pm_cm[g].__exit__(None, None, None)
                fin_state[g] = (exps, den)

            def finalize_b(g):
                exps, den = fin_state[g]
                rden = small.tile([GQ * TOPK, 1], FP32, tag="rden")
                nc.vector.reciprocal(rden, den)
                gr = small.tile([GQ * TOPK, GQ], BF16, tag="gr")
                nc.vector.tensor_scalar(
                    out=gr, in0=g_sb, scalar1=rden, scalar2=None, op0=ALU.mult)
                po_cm[g] = tc.tile_pool(name=f"ps_po{g}", bufs=1, space="PSUM")
                pop = po_cm[g].__enter__()
                og = consts.tile([GQ, N], FP32, name=f"og{g}")
                for c in range(2):
                    cs = slice(c * 512, (c + 1) * 512)
                    po = pop.tile([GQ, 512], FP32, tag=f"po{g}_{c}", name=f"po{g}_{c}")
                    nc.tensor.matmul(
                        po, lhsT=gr, rhs=exps[:, cs], start=True, stop=True)
                    if g == 0 or c == 0:
                        nc.vector.tensor_copy(og[:, cs], po)
                    else:
                        nc.scalar.activation(out=og[:, cs], in_=po, func=AFT.Copy)
                    deng = nc.sync if (g == 0 or c == 0) else nc.gpsimd
                    deng.dma_start(
                        out=out_t[g * GQ:(g + 1) * GQ, cs], in_=og[:, cs])
                po_cm[g].__exit__(None, None, None)

            issue_ub(1)
            issue_ub(3)
            for pi, pr in enumerate(PAIRS):
                if pi + 2 < len(PAIRS):
                    for q in PAIRS[pi + 2]:
                        issue_ub(q)
                for q in pr:
                    emit_query(q)
                if pi == FIN_A:
                    finalize(0)
                if pi == FIN_B:
                    finalize_b(0)
            finalize(1)
            finalize_b(1)

            if debug_taps:
                nc.sync.dma_start(out=dbg_u[0:8, :], in_=u_sb[0:8, :])
                nc.sync.dma_start(out=dbg_u[8:16, :], in_=u_sb[32:40, :])
                for g in range(GROUPS):
                    for c in range(2):
                        cs = slice(c * 512, (c + 1) * 512)
                        nc.sync.dma_start(
                            out=dbg_stk[g * 104:(g + 1) * 104, cs], in_=stack[g][c])

    nc.compile()
    return nc


_CACHE = {}


def _get_nc():
    if "nc" not in _CACHE:
        _CACHE["nc"] = _build_nc()
    return _CACHE["nc"]


def _in_maps(query, neighbors):
    query = np.ascontiguousarray(query, dtype=np.float32)
    neighbors = np.ascontiguousarray(neighbors, dtype=np.float32)
    return [
        {"query": query[c * BL:(c + 1) * BL], "neighbors": neighbors}
        for c in range(NCORES)
    ]


def _run(query, neighbors, **kw):
    nc = _get_nc()
    res = run_bass_kernel_spmd(nc, _in_maps(query, neighbors), list(range(NCORES)), **kw)
    out = np.concatenate([res.results[c]["out"] for c in range(NCORES)], axis=0)
    return out, res


def kernel(query, neighbors):
    out, _ = _run(query, neighbors)
    return out


def run_profiled(query, neighbors, **kw):
    out, res = _run(query, neighbors, trace=True, **kw)
    return out, res


# revision 7
# speedup vs baseline: 1.3254x; 1.0076x over previous
"""Trainium2 Bass kernel for deterministic NeuralSort soft-kNN (DKNN), v2.

Math (per query b over N neighbors):
    s_j   = -||q_b - x_j||^2
    A_j   = sum_i |s_j - s_i|
    P[r,j]= softmax_j(scaling[r] * s_j - A_j),  r = 0..K-1, scaling[r] = N+1-2(r+1)
    out_j = sum_r P[r,j]

Reductions:
  * s_j = u_j - ||q_b||^2 with u_j = 2 q_b.x_j - ||x_j||^2; the ||q||^2 term
    cancels (constant in j for A; constant per softmax row otherwise).
  * A_j = u_j*(2 r_j - N) - 2 L_j + T with r_j = #{i: u_i < u_j} and
    L_j = sum_{u_i < u_j} u_i.  T is constant per query => cancels in the
    row softmax and is dropped.
  * r_j / L_j come from a 0/1 compare matrix reduced on TensorE: the compare
    tiles are fp8 (exact 0/1 or +-1), u_i is decomposed into 5 exact fp8e4m3
    limbs (scales 4*16^-m), and the reduce runs as fp8 DoubleRow matmuls
    (2 k-tiles per pass, 0.5 cyc/col) accumulating counts + limb sums in
    fp32 PSUM.
  * ScalarE generates sign(u_i - u_j) tiles (func=Sign, scale=-1), Vector/
    GpSimd generate is_gt tiles; the convention difference is absorbed in
    host-constant coefficients of the final fp32 matmul.
  * P_max[80, N] for a group of 8 queries is ONE fp32 matmul per half:
    lhsT [56, 80] host consts over a stacked rhs [c2(8); Lsc(40); u(8)]
    where c2 = u .* counts, Lsc = s_m * limb-sum rows.

Sharding: data-parallel over B=128 queries across 8 cores (16 each, two
groups of 8 for softmax finalization); neighbors replicated.
"""

import numpy as np
import ml_dtypes

import concourse.bass as bass
import concourse.bacc as bacc
import concourse.tile as tile
from concourse import mybir
from concourse.masks import make_identity
from concourse.bass_utils import run_bass_kernel_spmd

AFT = mybir.ActivationFunctionType
ALU = mybir.AluOpType
FP32 = mybir.dt.float32
BF16 = mybir.dt.bfloat16
FP8 = mybir.dt.float8e4
PM = mybir.MatmulPerfMode

B, N, D, TOPK = 128, 1024, 128, 10
NCORES = 8
BL = B // NCORES          # 16 queries per core
NBLK = N // 128           # 8 row-blocks of the pairwise matrix
GROUPS = 2
GQ = BL // GROUPS         # 8 queries per softmax group
NL = 5                    # fp8 limbs per u value
LANES = 104               # pa rows: counts 0:8, (pad), limbs 64:104
LIMB0 = 64                # first limb lane
LSC = [4.0 * 2.0 ** (-4 * m) for m in range(NL)]

# Per-(query, t-block) engine for the compare-tile generation.
# A = ScalarE (Sign, +-1), D = VectorE (is_gt 0/1), P = GpSimd (is_gt 0/1).
ACT_Q = (0, 1, 8, 9, 15)
SIGN_BL = {0: (0, 1), 1: (0, 1, 7)}   # sign-convention bl per group
_NDH = 3  # number of DVE-heavy (5/3) queries
_DH_POOL = [2, 10, 3, 11, 5, 13, 6, 14]
ENG_T = {}
for _q in range(BL):
    if _q in ACT_Q:
        ENG_T[_q] = "AAAAAAAA"
    elif _q in _DH_POOL[:_NDH]:
        ENG_T[_q] = "DDDDDPPP"
    else:
        ENG_T[_q] = "DDDDPPPP"

PAIRS = ((0, 2), (1, 3), (4, 5), (6, 7), (8, 10), (9, 11), (12, 13), (14, 15))
FIN_A, FIN_B = 4, 6
# DMA queue for each query's u broadcast: S = sync, P = gpsimd
UBQ = {q: "S" for q in range(BL)}


def _host_consts():
    scaling = (N + 1 - 2.0 * (np.arange(TOPK) + 1)).astype(np.float64)
    # FdL [56, 80]: stack rows 0:8 u, 8:16 c2 (u .* counts), 16:56 Lsc (limb
    # sums, lane m,b at 16+8m+b).  Column (bl*10+r) computes
    # scaling[r]*u_j - A_j (+ const, dropped).
    # gt-convention (counts r_j, limb sums L):    -A = -2*(u.*r) + N*u + 2*Lsum
    # sign-convention (C' = -sum sgn, S' = -S):   -A = +(u.*C') - Lsum'
    fdl = np.zeros((GROUPS, 104, 80), np.float64)
    for g in range(GROUPS):
        for bl in range(GQ):
            is_sign = bl in SIGN_BL[g]
            for r in range(TOPK):
                col = bl * TOPK + r
                fdl[g, bl, col] = scaling[r] + (0.0 if is_sign else float(N))
                fdl[g, 32 + bl, col] = 1.0 if is_sign else -2.0
                for m in range(NL):
                    fdl[g, 64 + 8 * m + bl, col] = -1.0 if is_sign else 2.0
    # lscalev [40, 1]: Lsc = s_m * pa_limb_row
    lsc = np.zeros((40, 1), np.float32)
    for m in range(NL):
        for bl in range(GQ):
            lsc[8 * m + bl, 0] = LSC[m]
    G = np.zeros((GQ * TOPK, GQ), np.float32)
    for bl in range(GQ):
        G[bl * TOPK:(bl + 1) * TOPK, bl] = 1.0
    zeros_w3 = np.zeros((1, 4 * 2 * LANES * BL), ml_dtypes.float8_e4m3)
    return fdl.astype(np.float32), lsc, G, zeros_w3


def _build_nc(debug_taps=False):
    nc = bacc.Bacc(None, target_bir_lowering=False)

    q_in = nc.dram_tensor("query", [BL, D], FP32, kind="ExternalInput")
    x_in = nc.dram_tensor("neighbors", [N, D], FP32, kind="ExternalInput")
    out_t = nc.dram_tensor("out", [BL, N], FP32, kind="ExternalOutput")
    if debug_taps:
        dbg_u = nc.dram_tensor("dbg_u", [BL, N], FP32, kind="ExternalOutput")
        dbg_pa = nc.dram_tensor("dbg_pa", [GROUPS * LANES, N], FP32, kind="ExternalOutput")
        dbg_stk = nc.dram_tensor("dbg_stk", [GROUPS * 104, N], FP32, kind="ExternalOutput")
        dbg_pm = nc.dram_tensor("dbg_pm", [GROUPS * 80, N], FP32, kind="ExternalOutput")

    FdL, lscv, G, zw3 = _host_consts()
    fdl_in = [nc.inline_tensor(np.ascontiguousarray(FdL[g]), f"fdl{g}")
              for g in range(GROUPS)]
    lsc_in = nc.inline_tensor(lscv, "lscv")
    g_in = nc.inline_tensor(G, "lhs_g")
    zw3_in = nc.inline_tensor(zw3, "zw3")

    with tile.TileContext(nc) as tc:
        with (
            tc.tile_pool(name="consts", bufs=1) as consts,
            tc.tile_pool(name="xp", bufs=1) as xp,
            tc.tile_pool(name="bcast", bufs=12) as bcast,
            tc.tile_pool(name="scrS", bufs=24) as scrS,
            tc.tile_pool(name="small", bufs=8) as small,
            tc.tile_pool(name="dramp", bufs=1, space="DRAM") as dramp,
        ):
            ident = consts.tile([128, 128], FP32)
            make_identity(nc, ident)
            ones128 = consts.tile([128, 1], FP32)
            nc.vector.memset(ones128, 1.0)
            ones1xb = consts.tile([1, BL], FP32)
            nc.vector.memset(ones1xb, 1.0)
            ones1x128 = consts.tile([1, 128], FP32)
            nc.vector.memset(ones1x128, 1.0)
            ones1x72 = consts.tile([1, 72], FP32)
            nc.vector.memset(ones1x72, 0.0)
            nc.vector.memset(ones1x72[:, 0:8], 1.0)
            nc.vector.memset(ones1x72[:, 32:40], 1.0)
            nc.vector.memset(ones1x72[:, 64:65], 1.0)

            # ---- inputs in; W3 zero skeleton on Pool queue ----
            x_sb = xp.tile([128, NBLK, D], FP32)
            xv = x_in[:].rearrange("(t p) d -> p t d", p=128)
            half = NBLK // 2
            nc.sync.dma_start(out=x_sb[:, :half, :], in_=xv[:, :half, :])
            q_sb = small.tile([BL, D], FP32)
            nc.sync.dma_start(out=q_sb, in_=q_in[:])
            nc.sync.dma_start(out=x_sb[:, half:, :], in_=xv[:, half:, :])

            # W3 [128, tpair(4), tp(2), lane(48), q(16)] fp8
            W3 = consts.tile([128, 4, 2, LANES, BL], FP8)
            zsrc = zw3_in[:]
            zap = bass.AP(tensor=zsrc.tensor, offset=zsrc.offset,
                          ap=[[0, 128], [1, 4 * 2 * LANES * BL]])
            nc.sync.dma_start(
                out=W3[:].rearrange("p a b l q -> p (a b l q)"), in_=zap)

            fdl_sb = [consts.tile([104, 80], FP32, name=f"fdl_sb{g}")
                      for g in range(GROUPS)]
            for g in range(GROUPS):
                nc.sync.dma_start(out=fdl_sb[g], in_=fdl_in[g][:])
            lsc_sb = consts.tile([40, 1], FP32)
            nc.sync.dma_start(out=lsc_sb, in_=lsc_in[:])
            lscB = consts.tile([8, 1], FP32)
            nc.sync.dma_start(out=lscB, in_=lsc_in[32:40])
            g_sb = consts.tile([GQ * TOPK, GQ], FP32)
            nc.sync.dma_start(out=g_sb, in_=g_in[:])

            # ---- transposes: xT[d, j], q2T = (2Q)^T ----
            xT = xp.tile([128, N], FP32)
            q2T = consts.tile([128, BL], FP32)
            q2Tw = consts.tile([128, 72], FP32)
            with tc.tile_pool(name="ps_tr", bufs=2, space="PSUM") as ps_tr:
                for t in range(half):
                    ptr = ps_tr.tile([128, 128], FP32)
                    nc.tensor.transpose(ptr, x_sb[:, t, :], ident)
                    nc.vector.tensor_copy(xT[:, t * 128:(t + 1) * 128], ptr)
                pqt = ps_tr.tile([128, BL], FP32, tag="pqt")
                nc.tensor.transpose(pqt, q_sb, ident[:BL, :BL])
                nc.scalar.activation(out=q2T, in_=pqt, func=AFT.Copy, scale=2.0)
                nc.vector.memset(q2Tw[:, 8:32], 0.0)
                nc.vector.memset(q2Tw[:, 40:64], 0.0)
                nc.vector.memset(q2Tw[:, 65:72], 0.0)
                nc.vector.tensor_scalar(out=q2Tw[:, 0:8], in0=pqt[:, 0:8],
                                        scalar1=2.0, scalar2=None, op0=ALU.mult)
                nc.vector.tensor_scalar(out=q2Tw[:, 32:40], in0=pqt[:, 8:16],
                                        scalar1=2.0, scalar2=None, op0=ALU.mult)
                nc.vector.tensor_scalar(out=q2Tw[:, 64:65], in0=pqt[:, 2:3],
                                        scalar1=2.0, scalar2=None, op0=ALU.mult)
                for t in range(half, NBLK):
                    ptr = ps_tr.tile([128, 128], FP32)
                    nc.tensor.transpose(ptr, x_sb[:, t, :], ident)
                    nc.vector.tensor_copy(xT[:, t * 128:(t + 1) * 128], ptr)

            # ---- row norms: negx2[1, j] = -||x_j||^2 ----
            sq = xp.tile([128, N], FP32)
            negx2 = consts.tile([1, N], FP32)
            with tc.tile_pool(name="ps_x2", bufs=1, space="PSUM") as ps_x2:
                px2 = ps_x2.tile([1, N], FP32)
                for c in range(2):
                    cs = slice(c * 512, (c + 1) * 512)
                    nc.scalar.activation(out=sq[:, cs], in_=xT[:, cs], func=AFT.Square)
                    nc.tensor.matmul(
                        px2[:, cs], lhsT=ones128, rhs=sq[:, cs], start=True, stop=True)
                    nc.scalar.activation(
                        out=negx2[:, cs], in_=px2[:, cs], func=AFT.Copy, scale=-1.0)

            # ---- u row form (u_sb -> u_dram) and uT column form ----
            ub_tile = {}
            u_sb = consts.tile([40, N], FP32)   # rows 0:8 = q0..7, 32:40 = q8..15
            uT = consts.tile([128, NBLK, BL], FP32)   # uT[p, t, b] = u[b, t*128+p]
            u_dram = dramp.tile([40, N], FP32)
            with tc.tile_pool(name="ps_u", bufs=1, space="PSUM") as ps_u, \
                 tc.tile_pool(name="ps_qt", bufs=2, space="PSUM") as ps_qt:
                pu = ps_u.tile([72, N], FP32)
                for c in range(2):
                    cs = slice(c * 512, (c + 1) * 512)
                    nc.tensor.matmul(
                        pu[:, cs], lhsT=q2Tw, rhs=xT[:, cs], start=True, stop=False)
                    nc.tensor.matmul(
                        pu[:, cs], lhsT=ones1x72, rhs=negx2[:, cs],
                        start=False, stop=True)
                    nc.scalar.activation(out=u_sb[:, cs], in_=pu[0:40, cs], func=AFT.Copy)
                    nc.sync.dma_start(out=u_dram[:, cs], in_=u_sb[:, cs])
                u2row = consts.tile([1, N], FP32, name="u2row")
                nc.vector.tensor_copy(u2row, pu[64:65, :])
                bc_cm = tc.tile_pool(name="ps_bc", bufs=1, space="PSUM")
                bc_pool = bc_cm.__enter__()

                def bootstrap(q, ceng):
                    bc = bc_pool.tile([128, N], FP32, tag="bc", name=f"bc{q}")
                    ub = bcast.tile([128, N], FP32, tag="ub", name=f"ub{q}")
                    for c in range(2):
                        cs = slice(c * 512, (c + 1) * 512)
                        urow = u_sb[0:1, cs] if q == 0 else u2row[:, cs]
                        nc.tensor.matmul(
                            bc[:, cs], lhsT=ones1x128, rhs=urow,
                            start=True, stop=True)
                        if ceng is nc.scalar:
                            nc.scalar.activation(out=ub[:, cs], in_=bc[:, cs],
                                                 func=AFT.Copy)
                        else:
                            ceng.tensor_copy(ub[:, cs], bc[:, cs])
                    ub_tile[q] = ub

                bootstrap(0, nc.scalar)
                for t in range(NBLK):
                    if t == half:
                        bootstrap(2, nc.vector)
                    put = ps_qt.tile([128, BL], FP32, tag="put")
                    nc.tensor.matmul(
                        put, lhsT=xT[:, t * 128:(t + 1) * 128], rhs=q2T,
                        start=True, stop=False)
                    nc.tensor.matmul(
                        put, lhsT=negx2[:, t * 128:(t + 1) * 128], rhs=ones1xb,
                        start=False, stop=True)
                    nc.vector.tensor_copy(uT[:, t, :], put)
                bc_cm.__exit__(None, None, None)

            # ---- stacked rhs tiles (u rows placed early) ----
            stack = [[consts.tile([104, 512], FP32, name=f"stack{g}_{c}")
                      for c in range(2)] for g in range(GROUPS)]
            for g in range(GROUPS):
                for c in range(2):
                    cs = slice(c * 512, (c + 1) * 512)
                    nc.gpsimd.memset(stack[g][c][0:32, :], 0.0)
                    nc.gpsimd.memset(stack[g][c][32:64, :], 0.0)
                    nc.gpsimd.tensor_copy(stack[g][c][0:8, :],
                                          u_sb[g * 32:g * 32 + 8, cs])

            # ---- W3 diagonal lanes: counts + 5 fp8 limbs of uT ----
            # diag view for (lane-base lb, group g): [128, t(8), b(8)]
            def w3diag(lb, g):
                v = W3[:]
                return bass.AP(tensor=v.tensor, offset=v.offset + lb * BL + g * GQ,
                               ap=[list(v.ap[0]), [LANES * BL, NBLK], [BL + 1, GQ]])

            for g in range(GROUPS):
                nc.gpsimd.memset(w3diag(0, g), 1.0)
            for g in range(GROUPS):
                uTg = uT[:, :, g * GQ:(g + 1) * GQ]
                resid = bass.AP(tensor=uTg.tensor, offset=uTg.offset,
                                ap=[list(uTg.ap[0]), [BL, NBLK], [1, GQ]])
                for m in range(NL):
                    lane = w3diag(LIMB0 + 8 * m, g)
                    nc.vector.tensor_scalar(
                        out=lane, in0=resid, scalar1=1.0 / LSC[m], scalar2=None,
                        op0=ALU.mult)
                    if m < NL - 1:
                        back = small.tile([128, NBLK, GQ], FP32, tag="lback")
                        nc.vector.tensor_scalar(
                            out=back, in0=lane, scalar1=LSC[m], scalar2=None,
                            op0=ALU.mult)
                        nres = small.tile([128, NBLK, GQ], FP32, tag=f"lres{m % 2}")
                        nc.vector.tensor_tensor(
                            out=nres, in0=resid, in1=back, op=ALU.subtract)
                        resid = nres[:]

            # ---- pairwise phase ----
            pa_cm, pa_tile = {}, {}
            for g in (1, 0):
                pa_cm[g] = tc.tile_pool(name=f"ps_pa{g}", bufs=1, space="PSUM")
                pool = pa_cm[g].__enter__()
                pa_tile[g] = pool.tile([LANES, N], FP32, tag=f"pa{g}", name=f"pa{g}")
            first = {(g, c): True for g in range(GROUPS) for c in range(2)}
            left = {0: GQ * 4, 1: GQ * 4}

            def issue_ub(q):
                ub = bcast.tile([128, N], FP32, tag="ub", name=f"ub{q}")
                row = (q // GQ) * 32 + (q % GQ)
                eng = {"S": nc.sync, "P": nc.gpsimd, "A": nc.scalar}[UBQ[q]]
                for c in range(2):
                    cs = slice(c * 512, (c + 1) * 512)
                    base = u_dram[row:row + 1, cs]
                    src = bass.AP(tensor=base.tensor, offset=base.offset,
                                  ap=[[0, 128], [1, 512]])
                    eng.dma_start(out=ub[:, cs], in_=src)
                ub_tile[q] = ub

            def emit_query(q):
                g = q // GQ
                ub = ub_tile[q]
                for tp in range(4):
                    sd2 = scrS.tile([128, 2, N], FP8, tag="sd2", name=f"sd2_{q}_{tp}")
                    for th in range(2):
                        t = 2 * tp + th
                        e = ENG_T[q][t]
                        if e == "A":
                            nc.scalar.activation(
                                out=sd2[:, th, :], in_=ub, func=AFT.Sign,
                                bias=uT[:, t, q:q + 1], scale=-1.0)
                        else:
                            eng = nc.vector if e == "D" else nc.gpsimd
                            eng.tensor_scalar(
                                out=sd2[:, th, :], in0=ub,
                                scalar1=uT[:, t, q:q + 1], scalar2=0.0,
                                op0=ALU.subtract, op1=ALU.is_gt)
                    left[g] -= 1
                    for c in range(2):
                        cs = slice(c * 512, (c + 1) * 512)
                        nc.tensor.matmul(
                            pa_tile[g][:, cs], lhsT=W3[:, tp, :, :, q],
                            rhs=sd2[:, :, cs],
                            start=first[(g, c)], stop=left[g] == 0,
                            perf_mode=PM.DoubleRow)
                        first[(g, c)] = False

            pm_cm, pm_tile, po_cm, fin_state = {}, {}, {}, {}

            def finalize(g):
                pa = pa_tile[g]
                h0, h1 = slice(0, 512), slice(512, 1024)
                # c2 = u .* counts (rows 32:40); Lsc = s_m * limb rows (64:104)
                # halves in separate stack tiles so FdL h0 starts early.
                nc.vector.tensor_tensor(
                    out=stack[g][0][32:40, :], in0=pa[0:8, h0],
                    in1=u_sb[g * 32:g * 32 + 8, h0], op=ALU.mult)
                nc.scalar.activation(
                    out=stack[g][0][64:104, :], in_=pa[64:104, h0], func=AFT.Copy,
                    scale=lsc_sb)
                nc.vector.tensor_tensor(
                    out=stack[g][1][32:40, :], in0=pa[0:8, h1],
                    in1=u_sb[g * 32:g * 32 + 8, h1], op=ALU.mult)
                nc.scalar.activation(
                    out=stack[g][1][64:104, :], in_=pa[64:104, h1], func=AFT.Copy,
                    scale=lsc_sb)
                pa_cm[g].__exit__(None, None, None)
                pm_cm[g] = tc.tile_pool(name=f"ps_pm{g}", bufs=1, space="PSUM")
                pmp = pm_cm[g].__enter__()
                if g == 0:
                    pmh = [None, None]
                    pm = pmp.tile([GQ * TOPK, N], FP32, tag=f"pm{g}", name=f"pm{g}")
                else:
                    pmh = [pmp.tile([GQ * TOPK, 512], FP32, tag=f"pm{g}_{c}",
                                    name=f"pm{g}_{c}") for c in range(2)]
                # 128-col chunks: PE ramps on useful work; chunk 0/4 carry
                # start=True (pending-zero covers the whole 2KB region).
                for c in range(8):
                    hcs = slice((c % 4) * 128, (c % 4) * 128 + 128)
                    dst = pm[:, c * 128:(c + 1) * 128] if g == 0 else pmh[c // 4][:, hcs]
                    nc.tensor.matmul(
                        dst, lhsT=fdl_sb[g], rhs=stack[g][c // 4][:, hcs],
                        start=(c % 4 == 0), stop=(c % 4 == 3),
                        skip_group_check=True)
                vh0 = pm[:, h0] if g == 0 else pmh[0][:]
                vh1 = pm[:, h1] if g == 0 else pmh[1][:]
                nmx0 = small.tile([GQ * TOPK, 1], FP32, tag="nmx0")
                nc.vector.tensor_reduce(
                    out=nmx0, in_=vh0, axis=mybir.AxisListType.X, op=ALU.max,
                    negate=True)
                nmx1 = small.tile([GQ * TOPK, 1], FP32, tag="nmx1")
                nc.vector.tensor_reduce(
                    out=nmx1, in_=vh1, axis=mybir.AxisListType.X, op=ALU.max,
                    negate=True)
                exps = consts.tile([GQ * TOPK, N], BF16, name=f"exps{g}")
                den = small.tile([GQ * TOPK, 1], FP32, tag="den")
                sc = None
                if g == 0:
                    nmx = small.tile([GQ * TOPK, 1], FP32, tag="nmx")
                    nc.vector.tensor_tensor(out=nmx, in0=nmx0, in1=nmx1, op=ALU.min)
                    nc.scalar.activation(
                        out=exps, in_=pm, func=AFT.Exp, bias=nmx, scale=1.0,
                        accum_out=den)
                else:
                    # per-half bias: exps_h = e^{pm - max_h}; half h is scaled
                    # by s_h = e^{max_h - max_full} <= 1 in the gr factors.
                    den_h = small.tile([GQ * TOPK, 2], FP32, tag="denh")
                    nmxm = small.tile([GQ * TOPK, 1], FP32, tag="nmxm")
                    nc.vector.tensor_tensor(out=nmxm, in0=nmx0, in1=nmx1, op=ALU.min)
                    tdiff = small.tile([GQ * TOPK, 2], FP32, tag="tdiff")
                    nc.vector.tensor_tensor(out=tdiff[:, 0:1], in0=nmxm, in1=nmx0,
                                            op=ALU.subtract)
                    nc.vector.tensor_tensor(out=tdiff[:, 1:2], in0=nmxm, in1=nmx1,
                                            op=ALU.subtract)
                    nc.scalar.activation(
                        out=exps[:, 0:512], in_=pmh[0], func=AFT.Exp, bias=nmx0,
                        scale=1.0, accum_out=den_h[:, 0:1])
                    sc = small.tile([GQ * TOPK, 2], FP32, tag="sc")
                    nc.scalar.activation(out=sc, in_=tdiff, func=AFT.Exp)
                    nc.scalar.activation(
                        out=exps[:, 512:1024], in_=pmh[1], func=AFT.Exp, bias=nmx1,
                        scale=1.0, accum_out=den_h[:, 1:2])
                    d0 = small.tile([GQ * TOPK, 2], FP32, tag="dsc")
                    nc.vector.tensor_tensor(out=d0, in0=den_h, in1=sc, op=ALU.mult)
                    nc.vector.tensor_tensor(out=den, in0=d0[:, 0:1], in1=d0[:, 1:2],
                                            op=ALU.add)
                pm_cm[g].__exit__(None, None, None)
                fin_state[g] = (exps, den, sc)

            def finalize_b(g):
                exps, den, sc = fin_state[g]
                rden = small.tile([GQ * TOPK, 1], FP32, tag="rden")
                nc.vector.reciprocal(rden, den)
                grs = []
                if g == 0:
                    gr = small.tile([GQ * TOPK, GQ], BF16, tag="gr")
                    nc.vector.tensor_scalar(
                        out=gr, in0=g_sb, scalar1=rden, scalar2=None, op0=ALU.mult)
                    grs = [gr, gr]
                else:
                    rs = small.tile([GQ * TOPK, 2], FP32, tag="rs")
                    nc.vector.tensor_scalar(
                        out=rs, in0=sc, scalar1=rden, scalar2=None, op0=ALU.mult)
                    for c in range(2):
                        grc = small.tile([GQ * TOPK, GQ], BF16, tag=f"gr{c}")
                        nc.vector.tensor_scalar(
                            out=grc, in0=g_sb, scalar1=rs[:, c:c + 1], scalar2=None,
                            op0=ALU.mult)
                        grs.append(grc)
                po_cm[g] = tc.tile_pool(name=f"ps_po{g}", bufs=1, space="PSUM")
                pop = po_cm[g].__enter__()
                og = consts.tile([GQ, N], FP32, name=f"og{g}")
                for c in range(2):
                    cs = slice(c * 512, (c + 1) * 512)
                    po = pop.tile([GQ, 512], FP32, tag=f"po{g}_{c}", name=f"po{g}_{c}")
                    nc.tensor.matmul(
                        po, lhsT=grs[c], rhs=exps[:, cs], start=True, stop=True)
                    if g == 0 or c == 0:
                        nc.vector.tensor_copy(og[:, cs], po)
                    else:
                        nc.scalar.activation(out=og[:, cs], in_=po, func=AFT.Copy)
                    deng = nc.sync if (g == 0 or c == 0) else nc.gpsimd
                    deng.dma_start(
                        out=out_t[g * GQ:(g + 1) * GQ, cs], in_=og[:, cs])
                po_cm[g].__exit__(None, None, None)

            issue_ub(1)
            issue_ub(3)
            for pi, pr in enumerate(PAIRS):
                if pi + 2 < len(PAIRS):
                    for q in PAIRS[pi + 2]:
                        issue_ub(q)
                for q in pr:
                    emit_query(q)
                if pi == FIN_A:
                    finalize(0)
                if pi == FIN_B:
                    finalize_b(0)
            finalize(1)
            finalize_b(1)

            if debug_taps:
                nc.sync.dma_start(out=dbg_u[0:8, :], in_=u_sb[0:8, :])
                nc.sync.dma_start(out=dbg_u[8:16, :], in_=u_sb[32:40, :])
                for g in range(GROUPS):
                    for c in range(2):
                        cs = slice(c * 512, (c + 1) * 512)
                        nc.sync.dma_start(
                            out=dbg_stk[g * 104:(g + 1) * 104, cs], in_=stack[g][c])

    nc.compile()
    return nc


_CACHE = {}


def _get_nc():
    if "nc" not in _CACHE:
        _CACHE["nc"] = _build_nc()
    return _CACHE["nc"]


def _in_maps(query, neighbors):
    query = np.ascontiguousarray(query, dtype=np.float32)
    neighbors = np.ascontiguousarray(neighbors, dtype=np.float32)
    return [
        {"query": query[c * BL:(c + 1) * BL], "neighbors": neighbors}
        for c in range(NCORES)
    ]


def _run(query, neighbors, **kw):
    nc = _get_nc()
    res = run_bass_kernel_spmd(nc, _in_maps(query, neighbors), list(range(NCORES)), **kw)
    out = np.concatenate([res.results[c]["out"] for c in range(NCORES)], axis=0)
    return out, res


def kernel(query, neighbors):
    out, _ = _run(query, neighbors)
    return out


def run_profiled(query, neighbors, **kw):
    out, res = _run(query, neighbors, trace=True, **kw)
    return out, res


# revision 8
# speedup vs baseline: 1.3405x; 1.0113x over previous
"""Trainium2 Bass kernel for deterministic NeuralSort soft-kNN (DKNN), v2.

Math (per query b over N neighbors):
    s_j   = -||q_b - x_j||^2
    A_j   = sum_i |s_j - s_i|
    P[r,j]= softmax_j(scaling[r] * s_j - A_j),  r = 0..K-1, scaling[r] = N+1-2(r+1)
    out_j = sum_r P[r,j]

Reductions:
  * s_j = u_j - ||q_b||^2 with u_j = 2 q_b.x_j - ||x_j||^2; the ||q||^2 term
    cancels (constant in j for A; constant per softmax row otherwise).
  * A_j = u_j*(2 r_j - N) - 2 L_j + T with r_j = #{i: u_i < u_j} and
    L_j = sum_{u_i < u_j} u_i.  T is constant per query => cancels in the
    row softmax and is dropped.
  * r_j / L_j come from a 0/1 compare matrix reduced on TensorE: the compare
    tiles are fp8 (exact 0/1 or +-1), u_i is decomposed into 5 exact fp8e4m3
    limbs (scales 4*16^-m), and the reduce runs as fp8 DoubleRow matmuls
    (2 k-tiles per pass, 0.5 cyc/col) accumulating counts + limb sums in
    fp32 PSUM.
  * ScalarE generates sign(u_i - u_j) tiles (func=Sign, scale=-1), Vector/
    GpSimd generate is_gt tiles; the convention difference is absorbed in
    host-constant coefficients of the final fp32 matmul.
  * P_max[80, N] for a group of 8 queries is ONE fp32 matmul per half:
    lhsT [56, 80] host consts over a stacked rhs [c2(8); Lsc(40); u(8)]
    where c2 = u .* counts, Lsc = s_m * limb-sum rows.

Sharding: data-parallel over B=128 queries across 8 cores (16 each, two
groups of 8 for softmax finalization); neighbors replicated.
"""

import numpy as np
import ml_dtypes

import concourse.bass as bass
import concourse.bacc as bacc
import concourse.tile as tile
from concourse import mybir
from concourse.masks import make_identity
from concourse.bass_utils import run_bass_kernel_spmd

AFT = mybir.ActivationFunctionType
ALU = mybir.AluOpType
FP32 = mybir.dt.float32
BF16 = mybir.dt.bfloat16
FP8 = mybir.dt.float8e4
PM = mybir.MatmulPerfMode

B, N, D, TOPK = 128, 1024, 128, 10
NCORES = 8
BL = B // NCORES          # 16 queries per core
NBLK = N // 128           # 8 row-blocks of the pairwise matrix
GROUPS = 2
GQ = BL // GROUPS         # 8 queries per softmax group
NL = 5                    # fp8 limbs per u value
LANES = 104               # pa rows: counts 0:8, (pad), limbs 64:104
LIMB0 = 64                # first limb lane
LSC = [4.0 * 2.0 ** (-4 * m) for m in range(NL)]

# Per-(query, t-block) engine for the compare-tile generation.
# A = ScalarE (Sign, +-1), D = VectorE (is_gt 0/1), P = GpSimd (is_gt 0/1).
ACT_Q = (0, 1, 8, 9, 15)
SIGN_BL = {0: (0, 1), 1: (0, 1, 7)}   # sign-convention bl per group
_NDH = 3  # number of DVE-heavy (5/3) queries
_DH_POOL = [2, 10, 3, 11, 5, 13, 6, 14]
ENG_T = {}
for _q in range(BL):
    if _q in ACT_Q:
        ENG_T[_q] = "AAAAAAAA"
    elif _q in _DH_POOL[:_NDH]:
        ENG_T[_q] = "DDDDDPPP"
    else:
        ENG_T[_q] = "DDDDPPPP"

PAIRS = ((0, 2), (1, 3), (4, 5), (6, 7), (8, 10), (9, 11), (12, 13), (14, 15))
FIN_A, FIN_B = 4, 6
# DMA queue for each query's u broadcast: S = sync, P = gpsimd
UBQ = {q: "S" for q in range(BL)}


def _host_consts():
    scaling = (N + 1 - 2.0 * (np.arange(TOPK) + 1)).astype(np.float64)
    # FdL [56, 80]: stack rows 0:8 u, 8:16 c2 (u .* counts), 16:56 Lsc (limb
    # sums, lane m,b at 16+8m+b).  Column (bl*10+r) computes
    # scaling[r]*u_j - A_j (+ const, dropped).
    # gt-convention (counts r_j, limb sums L):    -A = -2*(u.*r) + N*u + 2*Lsum
    # sign-convention (C' = -sum sgn, S' = -S):   -A = +(u.*C') - Lsum'
    fdl = np.zeros((GROUPS, 104, 80), np.float64)
    for g in range(GROUPS):
        for bl in range(GQ):
            is_sign = bl in SIGN_BL[g]
            for r in range(TOPK):
                col = bl * TOPK + r
                fdl[g, bl, col] = scaling[r] + (0.0 if is_sign else float(N))
                fdl[g, 32 + bl, col] = 1.0 if is_sign else -2.0
                for m in range(NL):
                    fdl[g, 64 + 8 * m + bl, col] = -1.0 if is_sign else 2.0
    # lscalev [40, 1]: Lsc = s_m * pa_limb_row
    lsc = np.zeros((40, 1), np.float32)
    for m in range(NL):
        for bl in range(GQ):
            lsc[8 * m + bl, 0] = LSC[m]
    G = np.zeros((GQ * TOPK, GQ), np.float32)
    for bl in range(GQ):
        G[bl * TOPK:(bl + 1) * TOPK, bl] = 1.0
    zeros_w3 = np.zeros((1, 4 * 2 * LANES * BL), ml_dtypes.float8_e4m3)
    return fdl.astype(np.float32), lsc, G, zeros_w3


def _build_nc(debug_taps=False):
    nc = bacc.Bacc(None, target_bir_lowering=False)

    q_in = nc.dram_tensor("query", [BL, D], FP32, kind="ExternalInput")
    x_in = nc.dram_tensor("neighbors", [N, D], FP32, kind="ExternalInput")
    out_t = nc.dram_tensor("out", [BL, N], FP32, kind="ExternalOutput")
    if debug_taps:
        dbg_u = nc.dram_tensor("dbg_u", [BL, N], FP32, kind="ExternalOutput")
        dbg_pa = nc.dram_tensor("dbg_pa", [GROUPS * LANES, N], FP32, kind="ExternalOutput")
        dbg_stk = nc.dram_tensor("dbg_stk", [GROUPS * 104, N], FP32, kind="ExternalOutput")
        dbg_pm = nc.dram_tensor("dbg_pm", [GROUPS * 80, N], FP32, kind="ExternalOutput")

    FdL, lscv, G, zw3 = _host_consts()
    fdl_in = [nc.inline_tensor(np.ascontiguousarray(FdL[g]), f"fdl{g}")
              for g in range(GROUPS)]
    lsc_in = nc.inline_tensor(lscv, "lscv")
    g_in = nc.inline_tensor(G, "lhs_g")
    zw3_in = nc.inline_tensor(zw3, "zw3")

    with tile.TileContext(nc) as tc:
        with (
            tc.tile_pool(name="consts", bufs=1) as consts,
            tc.tile_pool(name="xp", bufs=1) as xp,
            tc.tile_pool(name="bcast", bufs=12) as bcast,
            tc.tile_pool(name="scrS", bufs=24) as scrS,
            tc.tile_pool(name="small", bufs=8) as small,
            tc.tile_pool(name="dramp", bufs=1, space="DRAM") as dramp,
        ):
            ident = consts.tile([128, 128], FP32)
            make_identity(nc, ident)
            ones128 = consts.tile([128, 1], FP32)
            nc.vector.memset(ones128, 1.0)
            ones1xb = consts.tile([1, BL], FP32)
            nc.vector.memset(ones1xb, 1.0)
            ones1x128 = consts.tile([1, 128], FP32)
            nc.vector.memset(ones1x128, 1.0)
            ones1x72 = consts.tile([1, 72], FP32)
            nc.vector.memset(ones1x72, 0.0)
            nc.vector.memset(ones1x72[:, 0:8], 1.0)
            nc.vector.memset(ones1x72[:, 32:40], 1.0)
            nc.vector.memset(ones1x72[:, 64:65], 1.0)

            # ---- inputs in; W3 zero skeleton on Pool queue ----
            x_sb = xp.tile([128, NBLK, D], FP32)
            xv = x_in[:].rearrange("(t p) d -> p t d", p=128)
            half = NBLK // 2
            nc.sync.dma_start(out=x_sb[:, :half, :], in_=xv[:, :half, :])
            q_sb = small.tile([BL, D], FP32)
            nc.sync.dma_start(out=q_sb, in_=q_in[:])
            nc.sync.dma_start(out=x_sb[:, half:, :], in_=xv[:, half:, :])

            # W3 [128, tpair(4), tp(2), lane(48), q(16)] fp8
            W3 = consts.tile([128, 4, 2, LANES, BL], FP8)
            zsrc = zw3_in[:]
            zap = bass.AP(tensor=zsrc.tensor, offset=zsrc.offset,
                          ap=[[0, 128], [1, 4 * 2 * LANES * BL]])
            nc.sync.dma_start(
                out=W3[:].rearrange("p a b l q -> p (a b l q)"), in_=zap)

            fdl_sb = [consts.tile([104, 80], FP32, name=f"fdl_sb{g}")
                      for g in range(GROUPS)]
            for g in range(GROUPS):
                nc.sync.dma_start(out=fdl_sb[g], in_=fdl_in[g][:])
            lsc_sb = consts.tile([40, 1], FP32)
            nc.sync.dma_start(out=lsc_sb, in_=lsc_in[:])
            lscB = consts.tile([8, 1], FP32)
            nc.sync.dma_start(out=lscB, in_=lsc_in[32:40])
            g_sb = consts.tile([GQ * TOPK, GQ], FP32)
            nc.sync.dma_start(out=g_sb, in_=g_in[:])

            # ---- transposes: xT[d, j], q2T = (2Q)^T ----
            xT = xp.tile([128, N], FP32)
            q2T = consts.tile([128, BL], FP32)
            q2Tw = consts.tile([128, 72], FP32)
            with tc.tile_pool(name="ps_tr", bufs=2, space="PSUM") as ps_tr:
                for t in range(half):
                    ptr = ps_tr.tile([128, 128], FP32)
                    nc.tensor.transpose(ptr, x_sb[:, t, :], ident)
                    nc.vector.tensor_copy(xT[:, t * 128:(t + 1) * 128], ptr)
                pqt = ps_tr.tile([128, BL], FP32, tag="pqt")
                nc.tensor.transpose(pqt, q_sb, ident[:BL, :BL])
                nc.scalar.activation(out=q2T, in_=pqt, func=AFT.Copy, scale=2.0)
                nc.vector.memset(q2Tw[:, 8:32], 0.0)
                nc.vector.memset(q2Tw[:, 40:64], 0.0)
                nc.vector.memset(q2Tw[:, 65:72], 0.0)
                nc.vector.tensor_scalar(out=q2Tw[:, 0:8], in0=pqt[:, 0:8],
                                        scalar1=2.0, scalar2=None, op0=ALU.mult)
                nc.vector.tensor_scalar(out=q2Tw[:, 32:40], in0=pqt[:, 8:16],
                                        scalar1=2.0, scalar2=None, op0=ALU.mult)
                nc.vector.tensor_scalar(out=q2Tw[:, 64:65], in0=pqt[:, 2:3],
                                        scalar1=2.0, scalar2=None, op0=ALU.mult)
                for t in range(half, NBLK):
                    ptr = ps_tr.tile([128, 128], FP32)
                    nc.tensor.transpose(ptr, x_sb[:, t, :], ident)
                    nc.vector.tensor_copy(xT[:, t * 128:(t + 1) * 128], ptr)

            # ---- row norms: negx2[1, j] = -||x_j||^2 ----
            sq = xp.tile([128, N], FP32)
            negx2 = consts.tile([1, N], FP32)
            with tc.tile_pool(name="ps_x2", bufs=1, space="PSUM") as ps_x2:
                px2 = ps_x2.tile([1, N], FP32)
                for c in range(2):
                    cs = slice(c * 512, (c + 1) * 512)
                    nc.scalar.activation(out=sq[:, cs], in_=xT[:, cs], func=AFT.Square)
                    nc.tensor.matmul(
                        px2[:, cs], lhsT=ones128, rhs=sq[:, cs], start=True, stop=True)
                    nc.scalar.activation(
                        out=negx2[:, cs], in_=px2[:, cs], func=AFT.Copy, scale=-1.0)

            # ---- u row form (u_sb -> u_dram) and uT column form ----
            ub_tile = {}
            u_sb = consts.tile([40, N], FP32)   # rows 0:8 = q0..7, 32:40 = q8..15
            uT = consts.tile([128, NBLK, BL], FP32)   # uT[p, t, b] = u[b, t*128+p]
            u_dram = dramp.tile([40, N], FP32)
            with tc.tile_pool(name="ps_u", bufs=1, space="PSUM") as ps_u, \
                 tc.tile_pool(name="ps_qt", bufs=2, space="PSUM") as ps_qt:
                pu = ps_u.tile([72, N], FP32)
                for c in range(2):
                    cs = slice(c * 512, (c + 1) * 512)
                    nc.tensor.matmul(
                        pu[:, cs], lhsT=q2Tw, rhs=xT[:, cs], start=True, stop=False)
                    nc.tensor.matmul(
                        pu[:, cs], lhsT=ones1x72, rhs=negx2[:, cs],
                        start=False, stop=True)
                    nc.scalar.activation(out=u_sb[:, cs], in_=pu[0:40, cs], func=AFT.Copy)
                    nc.sync.dma_start(out=u_dram[:, cs], in_=u_sb[:, cs])
                u2row = consts.tile([1, N], FP32, name="u2row")
                nc.vector.tensor_copy(u2row, pu[64:65, :])
                bc_cm = tc.tile_pool(name="ps_bc", bufs=1, space="PSUM")
                bc_pool = bc_cm.__enter__()

                def bootstrap(q, ceng):
                    bc = bc_pool.tile([128, N], FP32, tag="bc", name=f"bc{q}")
                    ub = bcast.tile([128, N], FP32, tag="ub", name=f"ub{q}")
                    for c in range(2):
                        cs = slice(c * 512, (c + 1) * 512)
                        urow = u_sb[0:1, cs] if q == 0 else u2row[:, cs]
                        nc.tensor.matmul(
                            bc[:, cs], lhsT=ones1x128, rhs=urow,
                            start=True, stop=True)
                        if ceng is nc.scalar:
                            nc.scalar.activation(out=ub[:, cs], in_=bc[:, cs],
                                                 func=AFT.Copy)
                        else:
                            ceng.tensor_copy(ub[:, cs], bc[:, cs])
                    ub_tile[q] = ub

                bootstrap(0, nc.scalar)
                for t in range(NBLK):
                    if t == half:
                        bootstrap(2, nc.vector)
                    put = ps_qt.tile([128, BL], FP32, tag="put")
                    nc.tensor.matmul(
                        put, lhsT=xT[:, t * 128:(t + 1) * 128], rhs=q2T,
                        start=True, stop=False)
                    nc.tensor.matmul(
                        put, lhsT=negx2[:, t * 128:(t + 1) * 128], rhs=ones1xb,
                        start=False, stop=True)
                    nc.vector.tensor_copy(uT[:, t, :], put)
                bc_cm.__exit__(None, None, None)

            # ---- stacked rhs tiles (u rows placed early) ----
            stack = [[consts.tile([104, 512], FP32, name=f"stack{g}_{c}")
                      for c in range(2)] for g in range(GROUPS)]
            for g in range(GROUPS):
                for c in range(2):
                    cs = slice(c * 512, (c + 1) * 512)
                    nc.gpsimd.memset(stack[g][c][0:32, :], 0.0)
                    nc.gpsimd.memset(stack[g][c][32:64, :], 0.0)
                    nc.gpsimd.tensor_copy(stack[g][c][0:8, :],
                                          u_sb[g * 32:g * 32 + 8, cs])

            # ---- W3 diagonal lanes: counts + 5 fp8 limbs of uT ----
            # diag view for (lane-base lb, group g): [128, t(8), b(8)]
            def w3diag(lb, g):
                v = W3[:]
                return bass.AP(tensor=v.tensor, offset=v.offset + lb * BL + g * GQ,
                               ap=[list(v.ap[0]), [LANES * BL, NBLK], [BL + 1, GQ]])

            for g in range(GROUPS):
                nc.gpsimd.memset(w3diag(0, g), 1.0)
            for g in range(GROUPS):
                uTg = uT[:, :, g * GQ:(g + 1) * GQ]
                resid = bass.AP(tensor=uTg.tensor, offset=uTg.offset,
                                ap=[list(uTg.ap[0]), [BL, NBLK], [1, GQ]])
                for m in range(NL):
                    lane = w3diag(LIMB0 + 8 * m, g)
                    nc.vector.tensor_scalar(
                        out=lane, in0=resid, scalar1=1.0 / LSC[m], scalar2=None,
                        op0=ALU.mult)
                    if m < NL - 1:
                        back = small.tile([128, NBLK, GQ], FP32, tag="lback")
                        nc.vector.tensor_scalar(
                            out=back, in0=lane, scalar1=LSC[m], scalar2=None,
                            op0=ALU.mult)
                        nres = small.tile([128, NBLK, GQ], FP32, tag=f"lres{m % 2}")
                        nc.vector.tensor_tensor(
                            out=nres, in0=resid, in1=back, op=ALU.subtract)
                        resid = nres[:]

            # ---- pairwise phase ----
            pa_cm, pa_tile = {}, {}
            for g in (1, 0):
                pa_cm[g] = tc.tile_pool(name=f"ps_pa{g}", bufs=1, space="PSUM")
                pool = pa_cm[g].__enter__()
                pa_tile[g] = pool.tile([LANES, N], FP32, tag=f"pa{g}", name=f"pa{g}")
            first = {(g, c): True for g in range(GROUPS) for c in range(2)}
            left = {0: GQ * 4, 1: GQ * 4}

            def issue_ub(q):
                ub = bcast.tile([128, N], FP32, tag="ub", name=f"ub{q}")
                row = (q // GQ) * 32 + (q % GQ)
                eng = {"S": nc.sync, "P": nc.gpsimd, "A": nc.scalar}[UBQ[q]]
                for c in range(2):
                    cs = slice(c * 512, (c + 1) * 512)
                    base = u_dram[row:row + 1, cs]
                    src = bass.AP(tensor=base.tensor, offset=base.offset,
                                  ap=[[0, 128], [1, 512]])
                    eng.dma_start(out=ub[:, cs], in_=src)
                ub_tile[q] = ub

            def emit_query(q):
                g = q // GQ
                ub = ub_tile[q]
                for tp in range(4):
                    sd2 = scrS.tile([128, 2, N], FP8, tag="sd2", name=f"sd2_{q}_{tp}")
                    for th in range(2):
                        t = 2 * tp + th
                        e = ENG_T[q][t]
                        if e == "A":
                            nc.scalar.activation(
                                out=sd2[:, th, :], in_=ub, func=AFT.Sign,
                                bias=uT[:, t, q:q + 1], scale=-1.0)
                        else:
                            eng = nc.vector if e == "D" else nc.gpsimd
                            eng.tensor_scalar(
                                out=sd2[:, th, :], in0=ub,
                                scalar1=uT[:, t, q:q + 1], scalar2=0.0,
                                op0=ALU.subtract, op1=ALU.is_gt)
                    left[g] -= 1
                    for c in range(2):
                        cs = slice(c * 512, (c + 1) * 512)
                        nc.tensor.matmul(
                            pa_tile[g][:, cs], lhsT=W3[:, tp, :, :, q],
                            rhs=sd2[:, :, cs],
                            start=first[(g, c)], stop=left[g] == 0,
                            perf_mode=PM.DoubleRow)
                        first[(g, c)] = False

            pm_cm, pm_tile, po_cm, fin_state = {}, {}, {}, {}

            def finalize(g):
                pa = pa_tile[g]
                h0, h1 = slice(0, 512), slice(512, 1024)
                # c2 = u .* counts (rows 32:40); Lsc = s_m * limb rows (64:104)
                # halves in separate stack tiles so FdL h0 starts early.
                nc.vector.tensor_tensor(
                    out=stack[g][0][32:40, :], in0=pa[0:8, h0],
                    in1=u_sb[g * 32:g * 32 + 8, h0], op=ALU.mult)
                nc.scalar.activation(
                    out=stack[g][0][64:104, :], in_=pa[64:104, h0], func=AFT.Copy,
                    scale=lsc_sb)
                nc.vector.tensor_tensor(
                    out=stack[g][1][32:40, :], in0=pa[0:8, h1],
                    in1=u_sb[g * 32:g * 32 + 8, h1], op=ALU.mult)
                nc.scalar.activation(
                    out=stack[g][1][64:104, :], in_=pa[64:104, h1], func=AFT.Copy,
                    scale=lsc_sb)
                if g == 0:
                    pa_cm[g].__exit__(None, None, None)
                pm_cm[g] = tc.tile_pool(name=f"ps_pm{g}", bufs=1, space="PSUM")
                pmp = pm_cm[g].__enter__()
                if g == 0:
                    pmh = [None, None]
                    pm = pmp.tile([GQ * TOPK, N], FP32, tag=f"pm{g}", name=f"pm{g}")
                else:
                    pmh = [pmp.tile([GQ * TOPK, 512], FP32, tag=f"pm{g}_{c}",
                                    name=f"pm{g}_{c}") for c in range(2)]
                # 128-col chunks: PE ramps on useful work; chunk 0/4 carry
                # start=True (pending-zero covers the whole 2KB region).
                for c in range(8):
                    hcs = slice((c % 4) * 128, (c % 4) * 128 + 128)
                    dst = pm[:, c * 128:(c + 1) * 128] if g == 0 else pmh[c // 4][:, hcs]
                    nc.tensor.matmul(
                        dst, lhsT=fdl_sb[g], rhs=stack[g][c // 4][:, hcs],
                        start=(c % 4 == 0), stop=(c % 4 == 3),
                        skip_group_check=True)
                vh0 = pm[:, h0] if g == 0 else pmh[0][:]
                vh1 = pm[:, h1] if g == 0 else pmh[1][:]
                nmx0 = small.tile([GQ * TOPK, 1], FP32, tag="nmx0")
                nc.vector.tensor_reduce(
                    out=nmx0, in_=vh0, axis=mybir.AxisListType.X, op=ALU.max,
                    negate=True)
                nmx1 = small.tile([GQ * TOPK, 1], FP32, tag="nmx1")
                nc.vector.tensor_reduce(
                    out=nmx1, in_=vh1, axis=mybir.AxisListType.X, op=ALU.max,
                    negate=True)
                exps = consts.tile([GQ * TOPK, N], BF16, name=f"exps{g}")
                den = small.tile([GQ * TOPK, 1], FP32, tag="den")
                sc = None
                if g == 0:
                    nmx = small.tile([GQ * TOPK, 1], FP32, tag="nmx")
                    nc.vector.tensor_tensor(out=nmx, in0=nmx0, in1=nmx1, op=ALU.min)
                    nc.scalar.activation(
                        out=exps, in_=pm, func=AFT.Exp, bias=nmx, scale=1.0,
                        accum_out=den)
                else:
                    # per-half bias: exps_h = e^{pm - max_h}; half h is scaled
                    # by s_h = e^{max_h - max_full} <= 1 in the gr factors.
                    den_h = small.tile([GQ * TOPK, 2], FP32, tag="denh")
                    nmxm = small.tile([GQ * TOPK, 1], FP32, tag="nmxm")
                    nc.vector.tensor_tensor(out=nmxm, in0=nmx0, in1=nmx1, op=ALU.min)
                    tdiff = small.tile([GQ * TOPK, 2], FP32, tag="tdiff")
                    nc.vector.tensor_tensor(out=tdiff[:, 0:1], in0=nmxm, in1=nmx0,
                                            op=ALU.subtract)
                    nc.vector.tensor_tensor(out=tdiff[:, 1:2], in0=nmxm, in1=nmx1,
                                            op=ALU.subtract)
                    nc.scalar.activation(
                        out=exps[:, 0:512], in_=pmh[0], func=AFT.Exp, bias=nmx0,
                        scale=1.0, accum_out=den_h[:, 0:1])
                    sc = small.tile([GQ * TOPK, 2], FP32, tag="sc")
                    nc.scalar.activation(out=sc, in_=tdiff, func=AFT.Exp)
                    nc.scalar.activation(
                        out=exps[:, 512:1024], in_=pmh[1], func=AFT.Exp, bias=nmx1,
                        scale=1.0, accum_out=den_h[:, 1:2])
                    d0 = small.tile([GQ * TOPK, 2], FP32, tag="dsc")
                    nc.vector.tensor_tensor(out=d0, in0=den_h, in1=sc, op=ALU.mult)
                    nc.vector.tensor_tensor(out=den, in0=d0[:, 0:1], in1=d0[:, 1:2],
                                            op=ALU.add)
                pm_cm[g].__exit__(None, None, None)
                if g == 1:
                    pa_cm[g].__exit__(None, None, None)
                fin_state[g] = (exps, den, sc)

            def finalize_b(g):
                exps, den, sc = fin_state[g]
                rden = small.tile([GQ * TOPK, 1], FP32, tag="rden")
                nc.vector.reciprocal(rden, den)
                grs = []
                if g == 0:
                    gr = small.tile([GQ * TOPK, GQ], BF16, tag="gr")
                    nc.vector.tensor_scalar(
                        out=gr, in0=g_sb, scalar1=rden, scalar2=None, op0=ALU.mult)
                    grs = [gr, gr]
                else:
                    rs = small.tile([GQ * TOPK, 2], FP32, tag="rs")
                    nc.vector.tensor_scalar(
                        out=rs, in0=sc, scalar1=rden, scalar2=None, op0=ALU.mult)
                    for c in range(2):
                        grc = small.tile([GQ * TOPK, GQ], BF16, tag=f"gr{c}")
                        nc.vector.tensor_scalar(
                            out=grc, in0=g_sb, scalar1=rs[:, c:c + 1], scalar2=None,
                            op0=ALU.mult)
                        grs.append(grc)
                po_cm[g] = tc.tile_pool(name=f"ps_po{g}", bufs=1, space="PSUM")
                pop = po_cm[g].__enter__()
                og = consts.tile([GQ, N], FP32, name=f"og{g}")
                for c in range(2):
                    cs = slice(c * 512, (c + 1) * 512)
                    po = pop.tile([GQ, 512], FP32, tag=f"po{g}_{c}", name=f"po{g}_{c}")
                    nc.tensor.matmul(
                        po, lhsT=grs[c], rhs=exps[:, cs], start=True, stop=True)
                    if g == 0 or c == 0:
                        nc.vector.tensor_copy(og[:, cs], po)
                    else:
                        nc.scalar.activation(out=og[:, cs], in_=po, func=AFT.Copy)
                    deng = nc.sync if (g == 0 or c == 0) else nc.gpsimd
                    deng.dma_start(
                        out=out_t[g * GQ:(g + 1) * GQ, cs], in_=og[:, cs])
                po_cm[g].__exit__(None, None, None)

            issue_ub(1)
            issue_ub(3)
            for pi, pr in enumerate(PAIRS):
                if pi + 2 < len(PAIRS):
                    for q in PAIRS[pi + 2]:
                        issue_ub(q)
                for q in pr:
                    emit_query(q)
                if pi == FIN_A:
                    finalize(0)
                if pi == FIN_B:
                    finalize_b(0)
            finalize(1)
            finalize_b(1)

            if debug_taps:
                nc.sync.dma_start(out=dbg_u[0:8, :], in_=u_sb[0:8, :])
                nc.sync.dma_start(out=dbg_u[8:16, :], in_=u_sb[32:40, :])
                for g in range(GROUPS):
                    for c in range(2):
                        cs = slice(c * 512, (c + 1) * 512)
                        nc.sync.dma_start(
                            out=dbg_stk[g * 104:(g + 1) * 104, cs], in_=stack[g][c])

    nc.compile()
    return nc


_CACHE = {}


def _get_nc():
    if "nc" not in _CACHE:
        _CACHE["nc"] = _build_nc()
    return _CACHE["nc"]


def _in_maps(query, neighbors):
    query = np.ascontiguousarray(query, dtype=np.float32)
    neighbors = np.ascontiguousarray(neighbors, dtype=np.float32)
    return [
        {"query": query[c * BL:(c + 1) * BL], "neighbors": neighbors}
        for c in range(NCORES)
    ]


def _run(query, neighbors, **kw):
    nc = _get_nc()
    res = run_bass_kernel_spmd(nc, _in_maps(query, neighbors), list(range(NCORES)), **kw)
    out = np.concatenate([res.results[c]["out"] for c in range(NCORES)], axis=0)
    return out, res


def kernel(query, neighbors):
    out, _ = _run(query, neighbors)
    return out


def run_profiled(query, neighbors, **kw):
    out, res = _run(query, neighbors, trace=True, **kw)
    return out, res
